# revision 5
# baseline (speedup 1.0000x reference)
"""Swin-style transformer block on 8 Trainium2 NeuronCores.

Sharding: data-parallel over batch — each of the 8 cores processes one image
([4096, 768] tokens). All weights replicated per core. No collectives.

Per-core pipeline:
  P1: LN1 statistics over the 50 window-half-chunks (98 tokens each)
  P2: 13 window-pairs: LN1-apply -> PE transpose -> fused QKV (fp32r, N=392)
      -> per-head scores (fp32r window-paired) -> softmax via exp(S)*exp(B)
      with deferred 1/sum -> transpose P -> O^T -> proj + residual -> x2 DRAM
  P4: LN2 statistics (32 token tiles)
  P5: MLP in two hidden-halves: fc1 (feature-major out) + erf-GELU -> fc2 +
      residual; second half accumulated into the output via gpsimd accum-DMA.

Host-side folds (all exact for the given inputs; SCALE=0.125 is binary-exact):
  - ln1 gamma/beta folded into qkv weights/bias; attention SCALE folded into
    the q block of qkv; v-bias folded into the proj bias (softmax rows sum
    to 1); ln2 gamma/beta folded into fc1; rel_pos_bias shipped as
    exp(rel_pos_bias) so softmax(S+B) = exp(S)*expB / sum(...).
"""

import numpy as np
from contextlib import ExitStack

import concourse.bass as bass
import concourse.mybir as mybir
import concourse.tile as tile
from concourse import bacc
from concourse.bass_utils import run_bass_kernel_spmd
from concourse.masks import make_identity

F32 = mybir.dt.float32
F32R = mybir.dt.float32r
AF = mybir.ActivationFunctionType
OP = mybir.AluOpType

DIM, HEADS, WIN, MLP_H = 768, 12, 14, 3072
B, H0, W0 = 8, 64, 64
NTOK = H0 * W0
NW = 5            # windows per image axis (70/14)
NWIN = NW * NW    # 25 windows
WW = WIN * WIN    # 196 tokens per window
HC = 98           # half-window chunk (7 rows x 14 cols)
DH = DIM // HEADS # 64
EPS = 1e-5

# window pairing: 12 pairs + 1 single
PAIRS = [(2 * i, 2 * i + 1) for i in range(12)] + [(24,)]


def _chunk_geom(w, c):
    """Valid-row/col geometry of half-chunk c (0/1) of window w."""
    wi, wj = divmod(w, NW)
    r0 = wi * WIN + c * 7          # first padded-image row of this chunk
    c0 = wj * WIN
    vr = 7 if (wi < 4 or c == 0) else 1   # wi==4 -> rows 56..63 valid (8)
    vc = 14 if wj < 4 else 8
    return r0, c0, vr, vc


def _gather_chunk(nc, dst, dram, w, c):
    """DMA image tokens of half-chunk (w, c) from [4096,768] DRAM into
    dst [98, 768] SBUF tile (partition p = 14*row + col). Pads with zeros."""
    r0, c0, vr, vc = _chunk_geom(w, c)
    if vr < 7 or vc < 14:
        nc.gpsimd.memset(dst[:, :], 0.0)
    if vc == 14:
        src = bass.AP(tensor=dram, offset=(r0 * W0 + c0) * DIM,
                      ap=[[W0 * DIM, vr], [DIM, 14], [1, DIM]])
        nc.sync.dma_start(dst[0:vr * 14, :], src)
    else:
        for r in range(vr):
            src = bass.AP(tensor=dram, offset=((r0 + r) * W0 + c0) * DIM,
                          ap=[[DIM, vc], [1, DIM]])
            nc.sync.dma_start(dst[r * 14:r * 14 + vc, :], src)


def _scatter_chunk(nc, dram, src, w, c):
    """DMA the valid tokens of half-chunk (w, c) from src [98,768] SBUF back
    to token-major [4096,768] DRAM."""
    r0, c0, vr, vc = _chunk_geom(w, c)
    if vc == 14:
        dst = bass.AP(tensor=dram, offset=(r0 * W0 + c0) * DIM,
                      ap=[[W0 * DIM, vr], [DIM, 14], [1, DIM]])
        nc.sync.dma_start(dst, src[0:vr * 14, :])
    else:
        for r in range(vr):
            dst = bass.AP(tensor=dram, offset=((r0 + r) * W0 + c0) * DIM,
                          ap=[[DIM, vc], [1, DIM]])
            nc.sync.dma_start(dst, src[r * 14:r * 14 + vc, :])


def _ln_stats(nc, pool, xt, n_part, statsM, statsR, col, eps_t):
    """bn_stats/bn_aggr over 768 features -> mean/rstd columns."""
    st = pool.tile([128, 3, 6], F32, name="bnstats")
    for g in range(3):
        nc.vector.bn_stats(st[:n_part, g, :], xt[:n_part, g * 256:(g + 1) * 256])
    mv = pool.tile([128, 2], F32, name="bnaggr")
    nc.vector.bn_aggr(mv[:n_part, :], st[:n_part, :, :])
    nc.gpsimd.tensor_copy(statsM[:n_part, col:col + 1], mv[:n_part, 0:1])
    std = pool.tile([128, 1], F32, name="std")
    nc.scalar.activation(std[:n_part, :], mv[:n_part, 1:2], AF.Sqrt,
                         bias=eps_t[:n_part, :], scale=1.0)
    nc.vector.reciprocal(statsR[:n_part, col:col + 1], std[:n_part, :])


def build_program():
    nc = bacc.Bacc(None, target_bir_lowering=False, debug=False)

    x_d = nc.dram_tensor("x", [NTOK, DIM], F32, kind="ExternalInput")
    qkvw_d = nc.dram_tensor("qkvw", [DIM, 3 * DIM], F32R, kind="ExternalInput")
    qkvb_d = nc.dram_tensor("qkvb", [3 * DIM], F32, kind="ExternalInput")
    projw_d = nc.dram_tensor("projw", [DIM, DIM], F32R, kind="ExternalInput")
    projb_d = nc.dram_tensor("projb", [DIM], F32R, kind="ExternalInput")
    expb_d = nc.dram_tensor("expb", [HC, HEADS, 2, WW], F32, kind="ExternalInput")
    fc1w_d = nc.dram_tensor("fc1w", [DIM, MLP_H], F32R, kind="ExternalInput")
    fc1b_d = nc.dram_tensor("fc1b", [MLP_H], F32, kind="ExternalInput")
    fc2w_d = nc.dram_tensor("fc2w", [MLP_H, DIM], F32R, kind="ExternalInput")
    fc2b_d = nc.dram_tensor("fc2b", [DIM], F32R, kind="ExternalInput")

    out_d = nc.dram_tensor("out", [NTOK, DIM], F32, kind="ExternalOutput")
    x2_d = nc.dram_tensor("x2", [NTOK, DIM], F32)  # internal scratch

    with tile.TileContext(nc) as tc:
        with ExitStack() as g:
            # ---------------- global constants / stats ----------------
            consts = g.enter_context(tc.tile_pool(name="consts", bufs=1))
            ident32 = consts.tile([128, 128], F32)
            make_identity(nc, ident32)
            ident_r = consts.tile([128, 128], F32R)
            nc.vector.tensor_copy(ident_r[:, :], ident32[:, :])
            ones32 = consts.tile([1, 128], F32)
            nc.vector.memset(ones32[:, :], 1.0)
            ones_r = consts.tile([1, 128], F32R)
            nc.vector.tensor_copy(ones_r[:, :], ones32[:, :])
            eps_t = consts.tile([128, 1], F32)
            nc.vector.memset(eps_t[:, :], EPS)
            qkvb_sb = consts.tile([128, 12], F32)
            nc.sync.dma_start(
                qkvb_sb[:, :],
                bass.AP(tensor=qkvb_d, offset=0, ap=[[1, 128], [128, 12]]))
            projb_row = consts.tile([1, DIM], F32R)
            nc.sync.dma_start(projb_row[0:1, :],
                              bass.AP(tensor=projb_d, offset=0, ap=[[1, DIM]]))
            fc2b_row = consts.tile([1, DIM], F32R)
            nc.sync.dma_start(fc2b_row[0:1, :],
                              bass.AP(tensor=fc2b_d, offset=0, ap=[[1, DIM]]))
            fc1b_sb = consts.tile([128, 24], F32)
            nc.sync.dma_start(
                fc1b_sb[:, :],
                bass.AP(tensor=fc1b_d, offset=0, ap=[[1, 128], [128, 24]]))
            statsM = consts.tile([HC, 2 * NWIN], F32)
            statsR = consts.tile([HC, 2 * NWIN], F32)
            stats2M = consts.tile([128, 32], F32)
            stats2R = consts.tile([128, 32], F32)

            # ---------------- P1: LN1 stats over window chunks ----------------
            with ExitStack() as s1:
                p1x = s1.enter_context(tc.tile_pool(name="p1x", bufs=4))
                p1s = s1.enter_context(tc.tile_pool(name="p1s", bufs=4))
                for w in range(NWIN):
                    for c in range(2):
                        xt = p1x.tile([HC, DIM], F32, name="p1xt")
                        _gather_chunk(nc, xt, x_d, w, c)
                        _ln_stats(nc, p1s, xt, HC, statsM, statsR, 2 * w + c, eps_t)

            # ---------------- P2: attention over window pairs ----------------
            with ExitStack() as s2:
                wA = s2.enter_context(tc.tile_pool(name="wA", bufs=1))
                qkvw_sb = wA.tile([128, 6, 3 * DIM], F32R)
                nc.sync.dma_start(
                    qkvw_sb[:, :, :],
                    qkvw_d[:].rearrange("(a p) n -> p a n", p=128))
                projw_sb = wA.tile([128, 6, DIM], F32R)
                nc.sync.dma_start(
                    projw_sb[:, :, :],
                    projw_d[:].rearrange("(a p) n -> p a n", p=128))
                expb_sb = wA.tile([HC, HEADS, 2, WW], F32)
                nc.sync.dma_start(expb_sb[:, :, :, :], expb_d[:])

                pxp = s2.enter_context(tc.tile_pool(name="pxp", bufs=1))
                pln = s2.enter_context(tc.tile_pool(name="pln", bufs=1))
                phT = s2.enter_context(tc.tile_pool(name="phT", bufs=1))
                pqk = s2.enter_context(tc.tile_pool(name="pqk", bufs=1))
                pvT = s2.enter_context(tc.tile_pool(name="pvT", bufs=1))
                psm = s2.enter_context(tc.tile_pool(name="psm", bufs=4))
                ppn = s2.enter_context(tc.tile_pool(name="ppn", bufs=2))
                pUT = s2.enter_context(tc.tile_pool(name="pUT", bufs=2))
                pOT = s2.enter_context(tc.tile_pool(name="pOT", bufs=1))
                px2 = s2.enter_context(tc.tile_pool(name="px2", bufs=1))
                pps = s2.enter_context(tc.tile_pool(name="pps", bufs=8, space="PSUM"))

                for pair in PAIRS:
                    nw = len(pair)
                    PW = nw * WW          # 392 or 196
                    nch = 2 * nw          # chunks in pair

                    x_pair = pxp.tile([HC, 4, DIM], F32, name="x_pair")
                    ln_pair = pln.tile([HC, 4, DIM], F32R, name="ln_pair")
                    for wl, w in enumerate(pair):
                        for c in range(2):
                            i = 2 * wl + c
                            _gather_chunk(nc, x_pair[:, i, :], x_d, w, c)
                            col = 2 * w + c
                            nc.vector.tensor_scalar(
                                out=ln_pair[:, i, :], in0=x_pair[:, i, :],
                                scalar1=statsM[:, col:col + 1],
                                scalar2=statsR[:, col:col + 1],
                                op0=OP.subtract, op1=OP.mult)

                    # transpose ln -> hT [128, 6, PW] (feature-major)
                    hT = phT.tile([128, 6, 2 * WW], F32R, name="hT")
                    for k in range(6):
                        ps_t = pps.tile([128, 392], F32R, tag="ps", name="ps_t")
                        for i in range(nch):
                            nc.tensor.transpose(
                                ps_t[:, i * HC:(i + 1) * HC],
                                ln_pair[:HC, i, k * 128:(k + 1) * 128],
                                ident_r[:HC, :HC])
                        nc.vector.tensor_copy(hT[:, k, :PW], ps_t[:, :PW])

                    # fused qk^T: [1536 feats, PW] (q pre-scaled on host)
                    qkT = pqk.tile([128, 12, 2 * WW], F32R, name="qkT")
                    for m in range(12):
                        ps_qk = pps.tile([128, 392], F32, tag="ps", name="ps_qk")
                        for k in range(6):
                            nc.tensor.matmul(
                                ps_qk[:, :PW],
                                qkvw_sb[:, k, m * 128:(m + 1) * 128],
                                hT[:, k, :PW],
                                start=(k == 0), stop=(k == 5))
                        nc.scalar.activation(qkT[:, m, :PW], ps_qk[:, :PW],
                                             AF.Identity, bias=qkvb_sb[:, m:m + 1])

                    # v token-major: [PW, 768]
                    vT = pvT.tile([HC, 4, DIM], F32R, name="vT")
                    for i in range(nch):
                        for n in range(2):
                            ps_v = pps.tile([128, 392], F32, tag="ps", name="ps_v")
                            for k in range(6):
                                nc.tensor.matmul(
                                    ps_v[:HC, :384],
                                    hT[:, k, i * HC:(i + 1) * HC],
                                    qkvw_sb[:, k, 2 * DIM + n * 384:2 * DIM + (n + 1) * 384],
                                    start=(k == 0), stop=(k == 5))
                            nc.vector.tensor_copy(vT[:, i, n * 384:(n + 1) * 384],
                                                  ps_v[:HC, :384])

                    OT = pOT.tile([128, 6, 2 * WW], F32R, name="OT")
                    for h in range(HEADS):
                        qrow = (h % 2) * 64
                        qm = h // 2
                        km = 6 + h // 2
                        Pn = ppn.tile([HC, 4, WW], F32, name="Pn")
                        den = ppn.tile([HC, 4], F32, name="den")
                        rden = ppn.tile([HC, 4], F32, name="rden")
                        for qc in range(nch):
                            wl = qc // 2
                            ps_S = pps.tile([128, 392], F32, tag="ps", name="ps_S")
                            nc.tensor.matmul(
                                ps_S[:HC, :PW],
                                qkT[qrow:qrow + 64, qm, qc * HC:(qc + 1) * HC],
                                qkT[qrow:qrow + 64, km, :PW],
                                start=True, stop=True)
                            U = psm.tile([HC, 2 * WW], F32, name="U")
                            nc.scalar.activation(U[:, :PW], ps_S[:HC, :PW], AF.Exp)
                            P = psm.tile([HC, WW], F32, name="P")
                            nc.vector.scalar_tensor_tensor(
                                out=P[:, :], in0=U[:, wl * WW:(wl + 1) * WW],
                                scalar=1.0, in1=expb_sb[:, h, qc % 2, :],
                                op0=OP.mult, op1=OP.mult,
                                accum_out=den[:, qc:qc + 1])
                            nc.vector.reciprocal(rden[:, qc:qc + 1], den[:, qc:qc + 1])
                            nc.gpsimd.tensor_scalar_mul(
                                Pn[:, qc, :], P[:, :], rden[:, qc:qc + 1])
                        # transpose Pn -> UT [k-local 98 x 2, q of pair]
                        UT = pUT.tile([HC, 2, 2 * WW], F32R, name="UT")
                        for kc in range(2):
                            ps_PT = pps.tile([128, 392], F32, tag="ps", name="ps_PT")
                            for qc in range(nch):
                                nc.tensor.transpose(
                                    ps_PT[:HC, qc * HC:(qc + 1) * HC],
                                    Pn[:HC, qc, kc * HC:(kc + 1) * HC],
                                    ident32[:HC, :HC])
                            nc.scalar.copy(UT[:, kc, :PW], ps_PT[:HC, :PW])
                        # O^T = V^T @ U^T per window
                        for wl in range(nw):
                            ps_O = pps.tile([128, 392], F32, tag="ps", name="ps_O")
                            for kc in range(2):
                                nc.tensor.matmul(
                                    ps_O[:64, :PW],
                                    vT[:, 2 * wl + kc, h * 64:(h + 1) * 64],
                                    UT[:, kc, :PW],
                                    start=(kc == 0), stop=(kc == 1))
                            nc.vector.tensor_copy(
                                OT[qrow:qrow + 64, qm, wl * WW:(wl + 1) * WW],
                                ps_O[:64, wl * WW:(wl + 1) * WW])

                    # proj + residual -> x2
                    x2_pair = px2.tile([HC, 4, DIM], F32, name="x2_pair")
                    for i in range(nch):
                        for n in range(2):
                            ps_pr = pps.tile([128, 392], F32, tag="ps", name="ps_pr")
                            for k in range(6):
                                nc.tensor.matmul(
                                    ps_pr[:HC, :384],
                                    OT[:, k, i * HC:(i + 1) * HC],
                                    projw_sb[:, k, n * 384:(n + 1) * 384],
                                    start=(k == 0), stop=False)
                            nc.tensor.matmul(
                                ps_pr[:HC, :384],
                                ones_r[0:1, :HC],
                                projb_row[0:1, n * 384:(n + 1) * 384],
                                start=False, stop=True)
                            nc.vector.tensor_tensor(
                                out=x2_pair[:, i, n * 384:(n + 1) * 384],
                                in0=ps_pr[:HC, :384],
                                in1=x_pair[:, i, n * 384:(n + 1) * 384],
                                op=OP.add)
                    for wl, w in enumerate(pair):
                        for c in range(2):
                            _scatter_chunk(nc, x2_d, x2_pair[:, 2 * wl + c, :], w, c)

            # ---------------- P4: LN2 stats ----------------
            with ExitStack() as s4:
                p4x = s4.enter_context(tc.tile_pool(name="p4x", bufs=4))
                p4s = s4.enter_context(tc.tile_pool(name="p4s", bufs=4))
                for t in range(32):
                    xt = p4x.tile([128, DIM], F32, name="p4xt")
                    nc.sync.dma_start(xt[:, :], x2_d[t * 128:(t + 1) * 128, :])
                    _ln_stats(nc, p4s, xt, 128, stats2M, stats2R, t, eps_t)

            # ---------------- P5: MLP in two hidden halves ----------------
            with ExitStack() as s5:
                wB = s5.enter_context(tc.tile_pool(name="wB", bufs=1))
                pxt = s5.enter_context(tc.tile_pool(name="pxt", bufs=4))
                pxn = s5.enter_context(tc.tile_pool(name="pxn", bufs=2))
                pxnT = s5.enter_context(tc.tile_pool(name="pxnT", bufs=2))
                pgT = s5.enter_context(tc.tile_pool(name="pgT", bufs=2))
                pout = s5.enter_context(tc.tile_pool(name="pout", bufs=4))
                pps5 = s5.enter_context(tc.tile_pool(name="pps5", bufs=8, space="PSUM"))

                for half in range(2):
                    fc1w_sb = wB.tile([128, 6, MLP_H // 2], F32R, name="fc1w_sb")
                    nc.sync.dma_start(
                        fc1w_sb[:, :, :],
                        fc1w_d[:, half * 1536:(half + 1) * 1536]
                        .rearrange("(a p) n -> p a n", p=128))
                    fc2w_sb = wB.tile([128, 12, DIM], F32R, name="fc2w_sb")
                    nc.sync.dma_start(
                        fc2w_sb[:, :, :],
                        fc2w_d[half * 1536:(half + 1) * 1536, :]
                        .rearrange("(a p) n -> p a n", p=128))

                    for st in range(16):   # super-tiles of 256 tokens
                        x2t = []
                        x2nT = pxnT.tile([128, 6, 256], F32R, name="x2nT")
                        for c in range(2):
                            t = st * 2 + c
                            xt = pxt.tile([128, DIM], F32, name="x2t")
                            nc.sync.dma_start(xt[:, :], x2_d[t * 128:(t + 1) * 128, :])
                            x2t.append(xt)
                            xn = pxn.tile([128, DIM], F32R, name="x2n")
                            nc.vector.tensor_scalar(
                                out=xn[:, :], in0=xt[:, :],
                                scalar1=stats2M[:, t:t + 1],
                                scalar2=stats2R[:, t:t + 1],
                                op0=OP.subtract, op1=OP.mult)
                            for k in range(6):
                                ps_t2 = pps5.tile([128, 384], F32R, tag="ps5",
                                                  name="ps_t2")
                                nc.tensor.transpose(
                                    ps_t2[:, :128], xn[:, k * 128:(k + 1) * 128],
                                    ident_r[:, :])
                                nc.vector.tensor_copy(
                                    x2nT[:, k, c * 128:(c + 1) * 128],
                                    ps_t2[:, :128])
                        gT = pgT.tile([128, 12, 256], F32R, name="gT")
                        for m in range(12):
                            ps_f1 = pps5.tile([128, 384], F32, tag="ps5", name="ps_f1")
                            for k in range(6):
                                nc.tensor.matmul(
                                    ps_f1[:, :256],
                                    fc1w_sb[:, k, m * 128:(m + 1) * 128],
                                    x2nT[:, k, :],
                                    start=(k == 0), stop=(k == 5))
                            nc.scalar.activation(
                                gT[:, m, :], ps_f1[:, :256], AF.Gelu,
                                bias=fc1b_sb[:, half * 12 + m:half * 12 + m + 1])
                        for c in range(2):
                            outt = pout.tile([128, DIM], F32, name="outt")
                            for n in range(2):
                                ps_f2 = pps5.tile([128, 384], F32, tag="ps5",
                                                  name="ps_f2")
                                for m in range(12):
                                    nc.tensor.matmul(
                                        ps_f2[:, :384],
                                        gT[:, m, c * 128:(c + 1) * 128],
                                        fc2w_sb[:, m, n * 384:(n + 1) * 384],
                                        start=(m == 0), stop=(m == 11 and half == 1))
                                if half == 0:
                                    nc.tensor.matmul(
                                        ps_f2[:, :384],
                                        ones_r[0:1, :128],
                                        fc2b_row[0:1, n * 384:(n + 1) * 384],
                                        start=False, stop=True)
                                    nc.vector.tensor_tensor(
                                        out=outt[:, n * 384:(n + 1) * 384],
                                        in0=ps_f2[:, :384],
                                        in1=x2t[c][:, n * 384:(n + 1) * 384],
                                        op=OP.add)
                                else:
                                    nc.vector.tensor_copy(
                                        outt[:, n * 384:(n + 1) * 384],
                                        ps_f2[:, :384])
                            t = st * 2 + c
                            if half == 0:
                                nc.sync.dma_start(
                                    out_d[t * 128:(t + 1) * 128, :], outt[:, :])
                            else:
                                nc.gpsimd.dma_start(
                                    out_d[t * 128:(t + 1) * 128, :], outt[:, :],
                                    accum_op=OP.add)

    nc.compile()
    return nc


_NC_CACHE = {}


def _get_nc():
    if "nc" not in _NC_CACHE:
        _NC_CACHE["nc"] = build_program()
    return _NC_CACHE["nc"]


def _prep_weights(inputs):
    f = lambda k: np.asarray(inputs[k], np.float32)
    x = f("x")
    ln1_g, ln1_b = f("ln1_g"), f("ln1_b")
    ln2_g, ln2_b = f("ln2_g"), f("ln2_b")
    qkv_w, qkv_b = f("qkv_w"), f("qkv_b")
    proj_w, proj_b = f("proj_w"), f("proj_b")
    fc1_w, fc1_b = f("fc1_w"), f("fc1_b")
    fc2_w, fc2_b = f("fc2_w"), f("fc2_b")
    rel = f("rel_pos_bias")
    SCALE = np.float32((DIM // HEADS) ** -0.5)

    # fold ln1 into qkv
    qkv_w_f = qkv_w * ln1_g[None, :]
    qkv_b_f = qkv_w @ ln1_b + qkv_b
    # fold attention scale into q block
    qkv_w_f[:DIM] *= SCALE
    qkv_b_f2 = qkv_b_f.copy()
    qkv_b_f2[:DIM] *= SCALE
    # fold v bias into proj bias (softmax rows sum to 1)
    projb = proj_b + proj_w @ qkv_b_f[2 * DIM:]
    # fold ln2 into fc1
    fc1_w_f = fc1_w * ln2_g[None, :]
    fc1_b_f = fc1_w @ ln2_b + fc1_b

    expb = np.exp(rel).astype(np.float32)          # [12, 196, 196]
    expb = expb.reshape(HEADS, 2, HC, WW).transpose(2, 0, 1, 3).copy()

    return {
        "qkvw": np.ascontiguousarray(qkv_w_f.T),    # [768, 2304]
        "qkvb": qkv_b_f2,
        "projw": np.ascontiguousarray(proj_w.T),    # [768, 768]
        "projb": projb,
        "expb": expb,
        "fc1w": np.ascontiguousarray(fc1_w_f.T),    # [768, 3072]
        "fc1b": fc1_b_f,
        "fc2w": np.ascontiguousarray(fc2_w.T),      # [3072, 768]
        "fc2b": fc2_b,
    }, x


PROFILE = False
LAST_RESULT = None


def kernel(**inputs):
    global LAST_RESULT
    weights, x = _prep_weights(inputs)
    nc = _get_nc()
    in_maps = [dict(weights, x=np.ascontiguousarray(x[i])) for i in range(B)]
    res = run_bass_kernel_spmd(nc, in_maps, core_ids=list(range(B)),
                               trace=PROFILE)
    LAST_RESULT = res
    out = np.stack([res.results[i]["out"] for i in range(B)], axis=0)
    return out.astype(np.float32)


if __name__ == "__main__":
    rng = np.random.default_rng(0)
    ins = {
        "x": rng.standard_normal((B, NTOK, DIM), dtype=np.float32),
        "rel_pos_bias": (rng.standard_normal((HEADS, WW, WW)) * 0.1).astype(np.float32),
        "ln1_g": np.ones(DIM, np.float32), "ln1_b": np.zeros(DIM, np.float32),
        "qkv_w": (rng.standard_normal((3 * DIM, DIM)) * 0.02).astype(np.float32),
        "qkv_b": np.zeros(3 * DIM, np.float32),
        "proj_w": (rng.standard_normal((DIM, DIM)) * 0.02).astype(np.float32),
        "proj_b": np.zeros(DIM, np.float32),
        "ln2_g": np.ones(DIM, np.float32), "ln2_b": np.zeros(DIM, np.float32),
        "fc1_w": (rng.standard_normal((MLP_H, DIM)) * 0.02).astype(np.float32),
        "fc1_b": np.zeros(MLP_H, np.float32),
        "fc2_w": (rng.standard_normal((DIM, MLP_H)) * 0.02).astype(np.float32),
        "fc2_b": np.zeros(DIM, np.float32),
        "H": np.int64(64), "W": np.int64(64),
    }
    out = kernel(**ins)
    print("out", out.shape, out.dtype, np.abs(out).max())


# revision 14
# speedup vs baseline: 1.2690x; 1.2690x over previous
"""Swin-style transformer block on 8 Trainium2 NeuronCores.

Sharding: data-parallel over batch — each of the 8 cores processes one image
([4096, 768] tokens). All weights replicated per core. No collectives.

Per-core pipeline:
  P1: LN1 statistics over the 50 window-half-chunks (98 tokens each)
  P2: 13 window-pairs: LN1-apply -> PE transpose -> fused QKV (fp32r, N=392)
      -> per-head scores (fp32r window-paired) -> softmax via exp(S)*exp(B)
      with deferred 1/sum -> transpose P -> O^T -> proj + residual -> x2 DRAM
  P4: LN2 statistics (32 token tiles)
  P5: MLP in two hidden-halves: fc1 (feature-major out) + erf-GELU -> fc2 +
      residual; second half accumulated into the output via gpsimd accum-DMA.

Host-side folds (all exact for the given inputs; SCALE=0.125 is binary-exact):
  - ln1 gamma/beta folded into qkv weights/bias; attention SCALE folded into
    the q block of qkv; v-bias folded into the proj bias (softmax rows sum
    to 1); ln2 gamma/beta folded into fc1; rel_pos_bias shipped as
    exp(rel_pos_bias) so softmax(S+B) = exp(S)*expB / sum(...).
"""

import numpy as np
from contextlib import ExitStack

import concourse.bass as bass
import concourse.mybir as mybir
import concourse.tile as tile
from concourse import bacc
from concourse.bass_utils import run_bass_kernel_spmd
from concourse.masks import make_identity

F32 = mybir.dt.float32
F32R = mybir.dt.float32r
AF = mybir.ActivationFunctionType
OP = mybir.AluOpType

DIM, HEADS, WIN, MLP_H = 768, 12, 14, 3072
B, H0, W0 = 8, 64, 64
NTOK = H0 * W0
NW = 5            # windows per image axis (70/14)
NWIN = NW * NW    # 25 windows
WW = WIN * WIN    # 196 tokens per window
HC = 98           # half-window chunk (7 rows x 14 cols)
DH = DIM // HEADS # 64
EPS = 1e-5

# window pairing: 12 pairs + 1 single
PAIRS = [(2 * i, 2 * i + 1) for i in range(12)] + [(24,)]


def _chunk_geom(w, c):
    """Valid-row/col geometry of half-chunk c (0/1) of window w."""
    wi, wj = divmod(w, NW)
    r0 = wi * WIN + c * 7          # first padded-image row of this chunk
    c0 = wj * WIN
    vr = 7 if (wi < 4 or c == 0) else 1   # wi==4 -> rows 56..63 valid (8)
    vc = 14 if wj < 4 else 8
    return r0, c0, vr, vc


def _gather_chunk(nc, dst, dram, w, c, eng=None):
    """DMA image tokens of half-chunk (w, c) from [4096,768] DRAM into
    dst [98, 768] SBUF tile (partition p = 14*row + col). Pads with zeros."""
    eng = eng or nc.sync
    r0, c0, vr, vc = _chunk_geom(w, c)
    if vr < 7 or vc < 14:
        nc.gpsimd.memset(dst[:, :], 0.0)
    if vc == 14:
        src = bass.AP(tensor=dram, offset=(r0 * W0 + c0) * DIM,
                      ap=[[W0 * DIM, vr], [DIM, 14], [1, DIM]])
        eng.dma_start(dst[0:vr * 14, :], src)
    else:
        for r in range(vr):
            src = bass.AP(tensor=dram, offset=((r0 + r) * W0 + c0) * DIM,
                          ap=[[DIM, vc], [1, DIM]])
            eng.dma_start(dst[r * 14:r * 14 + vc, :], src)


def _scatter_chunk(nc, dram, src, w, c, eng=None):
    """DMA the valid tokens of half-chunk (w, c) from src [98,768] SBUF back
    to token-major [4096,768] DRAM."""
    eng = eng or nc.sync
    r0, c0, vr, vc = _chunk_geom(w, c)
    if vc == 14:
        dst = bass.AP(tensor=dram, offset=(r0 * W0 + c0) * DIM,
                      ap=[[W0 * DIM, vr], [DIM, 14], [1, DIM]])
        eng.dma_start(dst, src[0:vr * 14, :])
    else:
        for r in range(vr):
            dst = bass.AP(tensor=dram, offset=((r0 + r) * W0 + c0) * DIM,
                          ap=[[DIM, vc], [1, DIM]])
            eng.dma_start(dst, src[r * 14:r * 14 + vc, :])


def _ln_stats(nc, pool, xt, n_part, statsM, statsR, col, eps_t):
    """bn_stats/bn_aggr over 768 features -> mean/rstd columns."""
    st = pool.tile([128, 3, 6], F32, name="bnstats")
    for g in range(3):
        nc.vector.bn_stats(st[:n_part, g, :], xt[:n_part, g * 256:(g + 1) * 256])
    mv = pool.tile([128, 2], F32, name="bnaggr")
    nc.vector.bn_aggr(mv[:n_part, :], st[:n_part, :, :])
    nc.gpsimd.tensor_copy(statsM[:n_part, col:col + 1], mv[:n_part, 0:1])
    std = pool.tile([128, 1], F32, name="std")
    nc.scalar.activation(std[:n_part, :], mv[:n_part, 1:2], AF.Sqrt,
                         bias=eps_t[:n_part, :], scale=1.0)
    nc.vector.reciprocal(statsR[:n_part, col:col + 1], std[:n_part, :])


def build_program():
    nc = bacc.Bacc(None, target_bir_lowering=False, debug=False)

    x_d = nc.dram_tensor("x", [NTOK, DIM], F32, kind="ExternalInput")
    qkvw_d = nc.dram_tensor("qkvw", [DIM, 3 * DIM], F32R, kind="ExternalInput")
    qkvb_d = nc.dram_tensor("qkvb", [3 * DIM], F32, kind="ExternalInput")
    projw_d = nc.dram_tensor("projw", [DIM, DIM], F32R, kind="ExternalInput")
    projb_d = nc.dram_tensor("projb", [DIM], F32R, kind="ExternalInput")
    expb_d = nc.dram_tensor("expb", [HC, HEADS, 2, WW], F32, kind="ExternalInput")
    fc1w_d = nc.dram_tensor("fc1w", [DIM, MLP_H], F32R, kind="ExternalInput")
    fc1b_d = nc.dram_tensor("fc1b", [MLP_H], F32, kind="ExternalInput")
    fc2w_d = nc.dram_tensor("fc2w", [MLP_H, DIM], F32R, kind="ExternalInput")
    fc2b_d = nc.dram_tensor("fc2b", [DIM], F32R, kind="ExternalInput")

    out_d = nc.dram_tensor("out", [NTOK, DIM], F32, kind="ExternalOutput")
    x2_d = nc.dram_tensor("x2", [NTOK, DIM], F32)  # internal scratch

    with tile.TileContext(nc) as tc:
        with ExitStack() as g:
            # ---------------- global constants / stats ----------------
            consts = g.enter_context(tc.tile_pool(name="consts", bufs=1))
            ident32 = consts.tile([128, 128], F32)
            make_identity(nc, ident32)
            ident_r = consts.tile([128, 128], F32R)
            nc.vector.tensor_copy(ident_r[:, :], ident32[:, :])
            ones32 = consts.tile([33, 128], F32)
            nc.vector.memset(ones32[:, :], 1.0)
            ones_r = consts.tile([33, 128], F32R)
            nc.vector.tensor_copy(ones_r[:, :], ones32[:, :])
            eps_t = consts.tile([128, 1], F32)
            nc.vector.memset(eps_t[:, :], EPS)
            qkvb_sb = consts.tile([128, 12], F32)
            nc.sync.dma_start(
                qkvb_sb[:, :],
                bass.AP(tensor=qkvb_d, offset=0, ap=[[1, 128], [128, 12]]))
            bias2 = consts.tile([33, DIM], F32R)
            nc.sync.dma_start(bias2[0:1, :],
                              bass.AP(tensor=projb_d, offset=0, ap=[[1, DIM]]))
            nc.sync.dma_start(bias2[32:33, :],
                              bass.AP(tensor=fc2b_d, offset=0, ap=[[1, DIM]]))
            fc1b_sb = consts.tile([128, 24], F32)
            nc.sync.dma_start(
                fc1b_sb[:, :],
                bass.AP(tensor=fc1b_d, offset=0, ap=[[1, 128], [128, 24]]))
            stats2M = consts.tile([128, 32], F32)
            stats2R = consts.tile([128, 32], F32)

            # ---------------- P2: attention over window pairs ----------------
            with ExitStack() as s2:
                wA = s2.enter_context(tc.tile_pool(name="wA", bufs=1))
                qkvw_sb = wA.tile([128, 6, 3 * DIM], F32R)
                for kk in range(6):
                    nc.sync.dma_start(
                        qkvw_sb[:, kk, :],
                        qkvw_d[kk * 128:(kk + 1) * 128, :])
                projw_sb = wA.tile([128, 6, DIM], F32R)
                nc.sync.dma_start(
                    projw_sb[:, :, :],
                    projw_d[:].rearrange("(a p) n -> p a n", p=128))
                expb_sb = wA.tile([HC, HEADS, 2, WW], F32)
                nc.sync.dma_start(expb_sb[:, :, :, :], expb_d[:])

                # natural_log_exp_and_others: exp (softmax) + ln/exp (rstd)
                nc.scalar.add_instruction(mybir.InstLoadActFuncSet(
                    name=nc.get_next_instruction_name(), ins=[], outs=[],
                    act_func_set_id=6))

                pxp = s2.enter_context(tc.tile_pool(name="pxp", bufs=2))
                pln = s2.enter_context(tc.tile_pool(name="pln", bufs=1))
                phT = s2.enter_context(tc.tile_pool(name="phT", bufs=1))
                pqk = s2.enter_context(tc.tile_pool(name="pqk", bufs=1))
                pvT = s2.enter_context(tc.tile_pool(name="pvT", bufs=1))
                psm = s2.enter_context(tc.tile_pool(name="psm", bufs=3))
                ppn = s2.enter_context(tc.tile_pool(name="ppn", bufs=2))
                pUT = s2.enter_context(tc.tile_pool(name="pUT", bufs=1))
                pOT = s2.enter_context(tc.tile_pool(name="pOT", bufs=1))
                px2 = s2.enter_context(tc.tile_pool(name="px2", bufs=3))
                pps = s2.enter_context(tc.tile_pool(name="pps", bufs=8, space="PSUM"))

                for pair in PAIRS:
                    nw = len(pair)
                    PW = nw * WW          # 392 or 196
                    nch = 2 * nw          # chunks in pair

                    x_pair = pxp.tile([HC, 4, DIM], F32, name="x_pair")
                    ln_pair = pln.tile([HC, 4, DIM], F32R, name="ln_pair")
                    for wl, w in enumerate(pair):
                        for c in range(2):
                            i = 2 * wl + c
                            _gather_chunk(nc, x_pair[:, i, :], x_d, w, c,
                                          eng=nc.gpsimd)
                            # LN1 stats inline: rstd = exp(-0.5*ln(var+eps))
                            st = psm.tile([HC, 3, 6], F32, name="bst")
                            for gg in range(3):
                                nc.vector.bn_stats(
                                    st[:, gg, :],
                                    x_pair[:, i, gg * 256:(gg + 1) * 256])
                            mv = psm.tile([HC, 2], F32, name="bmv")
                            nc.vector.bn_aggr(mv[:, :], st[:, :, :])
                            rstd = psm.tile([HC, 2], F32, name="rstd")
                            nc.scalar.activation(rstd[:, 0:1], mv[:, 1:2],
                                                 AF.Ln, bias=eps_t[:HC, :])
                            nc.scalar.activation(rstd[:, 1:2], rstd[:, 0:1],
                                                 AF.Exp, scale=-0.5)
                            nc.vector.tensor_scalar(
                                out=ln_pair[:, i, :], in0=x_pair[:, i, :],
                                scalar1=mv[:, 0:1],
                                scalar2=rstd[:, 1:2],
                                op0=OP.subtract, op1=OP.mult)

                    # transpose ln -> hT [128, 6, PW] (feature-major)
                    hT = phT.tile([128, 6, 2 * WW], F32R, name="hT")
                    for k in range(6):
                        ps_t = pps.tile([128, 392], F32R, tag="ps", name="ps_t")
                        for i in range(nch):
                            nc.tensor.transpose(
                                ps_t[:, i * HC:(i + 1) * HC],
                                ln_pair[:HC, i, k * 128:(k + 1) * 128],
                                ident_r[:HC, :HC])
                        nc.vector.tensor_copy(hT[:, k, :PW], ps_t[:, :PW])

                    # fused qk^T: [1536 feats, PW] (q pre-scaled on host)
                    qkT = pqk.tile([128, 12, 2 * WW], F32R, name="qkT")
                    for m in range(12):
                        ps_qk = pps.tile([128, 392], F32, tag="ps", name="ps_qk")
                        for k in range(6):
                            nc.tensor.matmul(
                                ps_qk[:, :PW],
                                qkvw_sb[:, k, m * 128:(m + 1) * 128],
                                hT[:, k, :PW],
                                start=(k == 0), stop=(k == 5))
                        nc.scalar.activation(qkT[:, m, :PW], ps_qk[:, :PW],
                                             AF.Identity, bias=qkvb_sb[:, m:m + 1])

                    # v token-major: [PW, 768]
                    vT = pvT.tile([HC, 4, DIM], F32R, name="vT")
                    for i in range(nch):
                        for n in range(2):
                            ps_v = pps.tile([128, 392], F32, tag="ps", name="ps_v")
                            for k in range(6):
                                nc.tensor.matmul(
                                    ps_v[:HC, :384],
                                    hT[:, k, i * HC:(i + 1) * HC],
                                    qkvw_sb[:, k, 2 * DIM + n * 384:2 * DIM + (n + 1) * 384],
                                    start=(k == 0), stop=(k == 5))
                            nc.vector.tensor_copy(vT[:, i, n * 384:(n + 1) * 384],
                                                  ps_v[:HC, :384])

                    OT = pOT.tile([128, 6, 2 * WW], F32R, name="OT")

                    def head_stage_a(h):
                        """scores + softmax -> normalized Pn for head h."""
                        qrow = (h % 2) * 64
                        qm = h // 2
                        km = 6 + h // 2
                        Pn = ppn.tile([HC, 4, WW], F32, name="Pn")
                        den = ppn.tile([HC, 4], F32, name="den")
                        rden = ppn.tile([HC, 4], F32, name="rden")
                        for qc in range(nch):
                            wl = qc // 2
                            ps_S = pps.tile([128, 392], F32, tag="ps", name="ps_S")
                            nc.tensor.matmul(
                                ps_S[:HC, :PW],
                                qkT[qrow:qrow + 64, qm, qc * HC:(qc + 1) * HC],
                                qkT[qrow:qrow + 64, km, :PW],
                                start=True, stop=True)
                            U = psm.tile([HC, 2 * WW], F32, name="U")
                            nc.scalar.activation(U[:, :PW], ps_S[:HC, :PW], AF.Exp)
                            P = psm.tile([HC, WW], F32, name="P")
                            nc.vector.scalar_tensor_tensor(
                                out=P[:, :], in0=U[:, wl * WW:(wl + 1) * WW],
                                scalar=1.0, in1=expb_sb[:, h, qc % 2, :],
                                op0=OP.mult, op1=OP.mult,
                                accum_out=den[:, qc:qc + 1])
                            nc.vector.reciprocal(rden[:, qc:qc + 1],
                                                 den[:, qc:qc + 1])
                            nc.gpsimd.tensor_scalar_mul(
                                Pn[:, qc, :], P[:, :], rden[:, qc:qc + 1])
                        return Pn

                    def head_stage_b(h, Pn):
                        """transpose Pn -> UT, then O^T = V^T @ U^T."""
                        qrow = (h % 2) * 64
                        qm = h // 2
                        UT = pUT.tile([HC, 2, 2 * WW], F32R, name="UT")
                        for kc in range(2):
                            ps_PT = pps.tile([128, 392], F32, tag="ps", name="ps_PT")
                            for qc in range(nch):
                                nc.tensor.transpose(
                                    ps_PT[:HC, qc * HC:(qc + 1) * HC],
                                    Pn[:HC, qc, kc * HC:(kc + 1) * HC],
                                    ident32[:HC, :HC])
                            nc.scalar.copy(UT[:, kc, :PW], ps_PT[:HC, :PW])
                        for wl in range(nw):
                            ps_O = pps.tile([128, 392], F32, tag="ps", name="ps_O")
                            for kc in range(2):
                                nc.tensor.matmul(
                                    ps_O[:64, :PW],
                                    vT[:, 2 * wl + kc, h * 64:(h + 1) * 64],
                                    UT[:, kc, :PW],
                                    start=(kc == 0), stop=(kc == 1))
                            nc.vector.tensor_copy(
                                OT[qrow:qrow + 64, qm, wl * WW:(wl + 1) * WW],
                                ps_O[:64, wl * WW:(wl + 1) * WW])

                    # 2-stage pipeline over heads: S(h+1) issues before PT/O(h)
                    prev = None
                    for h in range(HEADS):
                        Pn = head_stage_a(h)
                        if prev is not None:
                            head_stage_b(h - 1, prev)
                        prev = Pn
                    head_stage_b(HEADS - 1, prev)

                    # proj + residual -> x2 (per chunk, scatter immediately)
                    for i in range(nch):
                        x2c = px2.tile([HC, DIM], F32, name="x2c")
                        for n in range(2):
                            ps_pr = pps.tile([128, 392], F32, tag="ps", name="ps_pr")
                            for k in range(6):
                                nc.tensor.matmul(
                                    ps_pr[:HC, :384],
                                    OT[:, k, i * HC:(i + 1) * HC],
                                    projw_sb[:, k, n * 384:(n + 1) * 384],
                                    start=(k == 0), stop=False)
                            nc.tensor.matmul(
                                ps_pr[:HC, :384],
                                ones_r[0:1, :HC],
                                bias2[0:1, n * 384:(n + 1) * 384],
                                start=False, stop=True)
                            nc.vector.tensor_tensor(
                                out=x2c[:, n * 384:(n + 1) * 384],
                                in0=ps_pr[:HC, :384],
                                in1=x_pair[:, i, n * 384:(n + 1) * 384],
                                op=OP.add)
                        _scatter_chunk(nc, x2_d, x2c[:, :], pair[i // 2], i % 2,
                                       eng=nc.sync)

            # ---------------- P4 + P5 (MLP) ----------------
            with ExitStack() as s5:
                # weight pool first so the half-0 prefetch DMAs overlap the
                # LN2-stats pass (no address reuse WAR on P4's pools)
                wB = s5.enter_context(tc.tile_pool(name="wB", bufs=1))

                def load_fc_weights(half):
                    fc1w_sb = wB.tile([128, 6, MLP_H // 2], F32R, name="fc1w_sb")
                    nc.gpsimd.dma_start(
                        fc1w_sb[:, :, :],
                        fc1w_d[:, half * 1536:(half + 1) * 1536]
                        .rearrange("(a p) n -> p a n", p=128))
                    fc2w_sb = wB.tile([128, 12, DIM], F32R, name="fc2w_sb")
                    nc.gpsimd.dma_start(
                        fc2w_sb[:, :, :],
                        fc2w_d[half * 1536:(half + 1) * 1536, :]
                        .rearrange("(a p) n -> p a n", p=128))
                    return fc1w_sb, fc2w_sb

                w_half0 = load_fc_weights(0)

                pxt = s5.enter_context(tc.tile_pool(name="pxt", bufs=4))
                pxn = s5.enter_context(tc.tile_pool(name="pxn", bufs=2))
                pxnT = s5.enter_context(tc.tile_pool(name="pxnT", bufs=2))
                pgT = s5.enter_context(tc.tile_pool(name="pgT", bufs=2))
                pout = s5.enter_context(tc.tile_pool(name="pout", bufs=4))
                pps5 = s5.enter_context(tc.tile_pool(name="pps5", bufs=8, space="PSUM"))

                # LN2 stats (ACT: Sqrt table; loads alternate scalar/sync)
                with ExitStack() as s4:
                    p4x = s4.enter_context(tc.tile_pool(name="p4x", bufs=4))
                    p4s = s4.enter_context(tc.tile_pool(name="p4s", bufs=4))
                    for t in range(32):
                        xt = p4x.tile([128, DIM], F32, name="p4xt")
                        eng = nc.scalar if t % 2 == 0 else nc.sync
                        eng.dma_start(xt[:, :], x2_d[t * 128:(t + 1) * 128, :])
                        _ln_stats(nc, p4s, xt, 128, stats2M, stats2R, t, eps_t)

                for half in range(2):
                    fc1w_sb, fc2w_sb = w_half0 if half == 0 else load_fc_weights(1)

                    for st in range(16):   # super-tiles of 256 tokens
                        x2t = []
                        x2nT = pxnT.tile([128, 6, 256], F32R, name="x2nT")
                        for c in range(2):
                            t = st * 2 + c
                            xt = pxt.tile([128, DIM], F32, name="x2t")
                            eng = nc.scalar if t % 2 == 0 else nc.sync
                            eng.dma_start(xt[:, :], x2_d[t * 128:(t + 1) * 128, :])
                            x2t.append(xt)
                            xn = pxn.tile([128, DIM], F32R, name="x2n")
                            nc.vector.tensor_scalar(
                                out=xn[:, :], in0=xt[:, :],
                                scalar1=stats2M[:, t:t + 1],
                                scalar2=stats2R[:, t:t + 1],
                                op0=OP.subtract, op1=OP.mult)
                            for k in range(6):
                                ps_t2 = pps5.tile([128, 384], F32R, tag="ps5",
                                                  name="ps_t2")
                                nc.tensor.transpose(
                                    ps_t2[:, :128], xn[:, k * 128:(k + 1) * 128],
                                    ident_r[:, :])
                                nc.vector.tensor_copy(
                                    x2nT[:, k, c * 128:(c + 1) * 128],
                                    ps_t2[:, :128])
                        gT = pgT.tile([128, 12, 256], F32R, name="gT")
                        for m in range(12):
                            ps_f1 = pps5.tile([128, 384], F32, tag="ps5", name="ps_f1")
                            for k in range(6):
                                nc.tensor.matmul(
                                    ps_f1[:, :256],
                                    fc1w_sb[:, k, m * 128:(m + 1) * 128],
                                    x2nT[:, k, :],
                                    start=(k == 0), stop=(k == 5))
                            nc.scalar.activation(
                                gT[:, m, :], ps_f1[:, :256], AF.Gelu,
                                bias=fc1b_sb[:, half * 12 + m:half * 12 + m + 1])
                        for c in range(2):
                            outt = pout.tile([128, DIM], F32, name="outt")
                            for n in range(2):
                                ps_f2 = pps5.tile([128, 384], F32, tag="ps5",
                                                  name="ps_f2")
                                for m in range(12):
                                    nc.tensor.matmul(
                                        ps_f2[:, :384],
                                        gT[:, m, c * 128:(c + 1) * 128],
                                        fc2w_sb[:, m, n * 384:(n + 1) * 384],
                                        start=(m == 0), stop=(m == 11 and half == 1))
                                if half == 0:
                                    nc.tensor.matmul(
                                        ps_f2[:, :384],
                                        ones_r[32:33, :128],
                                        bias2[32:33, n * 384:(n + 1) * 384],
                                        start=False, stop=True)
                                    nc.vector.tensor_tensor(
                                        out=outt[:, n * 384:(n + 1) * 384],
                                        in0=ps_f2[:, :384],
                                        in1=x2t[c][:, n * 384:(n + 1) * 384],
                                        op=OP.add)
                                else:
                                    nc.vector.tensor_copy(
                                        outt[:, n * 384:(n + 1) * 384],
                                        ps_f2[:, :384])
                            t = st * 2 + c
                            if half == 0:
                                nc.sync.dma_start(
                                    out_d[t * 128:(t + 1) * 128, :], outt[:, :])
                            else:
                                nc.gpsimd.dma_start(
                                    out_d[t * 128:(t + 1) * 128, :], outt[:, :],
                                    accum_op=OP.add)

    nc.compile()
    return nc


_NC_CACHE = {}


def _get_nc():
    if "nc" not in _NC_CACHE:
        _NC_CACHE["nc"] = build_program()
    return _NC_CACHE["nc"]


def _prep_weights(inputs):
    f = lambda k: np.asarray(inputs[k], np.float32)
    x = f("x")
    ln1_g, ln1_b = f("ln1_g"), f("ln1_b")
    ln2_g, ln2_b = f("ln2_g"), f("ln2_b")
    qkv_w, qkv_b = f("qkv_w"), f("qkv_b")
    proj_w, proj_b = f("proj_w"), f("proj_b")
    fc1_w, fc1_b = f("fc1_w"), f("fc1_b")
    fc2_w, fc2_b = f("fc2_w"), f("fc2_b")
    rel = f("rel_pos_bias")
    SCALE = np.float32((DIM // HEADS) ** -0.5)

    # fold ln1 into qkv
    qkv_w_f = qkv_w * ln1_g[None, :]
    qkv_b_f = qkv_w @ ln1_b + qkv_b
    # fold attention scale into q block
    qkv_w_f[:DIM] *= SCALE
    qkv_b_f2 = qkv_b_f.copy()
    qkv_b_f2[:DIM] *= SCALE
    # fold v bias into proj bias (softmax rows sum to 1)
    projb = proj_b + proj_w @ qkv_b_f[2 * DIM:]
    # fold ln2 into fc1
    fc1_w_f = fc1_w * ln2_g[None, :]
    fc1_b_f = fc1_w @ ln2_b + fc1_b

    expb = np.exp(rel).astype(np.float32)          # [12, 196, 196]
    expb = expb.reshape(HEADS, 2, HC, WW).transpose(2, 0, 1, 3).copy()

    return {
        "qkvw": np.ascontiguousarray(qkv_w_f.T),    # [768, 2304]
        "qkvb": qkv_b_f2,
        "projw": np.ascontiguousarray(proj_w.T),    # [768, 768]
        "projb": projb,
        "expb": expb,
        "fc1w": np.ascontiguousarray(fc1_w_f.T),    # [768, 3072]
        "fc1b": fc1_b_f,
        "fc2w": np.ascontiguousarray(fc2_w.T),      # [3072, 768]
        "fc2b": fc2_b,
    }, x


PROFILE = False
LAST_RESULT = None


def kernel(**inputs):
    global LAST_RESULT
    weights, x = _prep_weights(inputs)
    nc = _get_nc()
    in_maps = [dict(weights, x=np.ascontiguousarray(x[i])) for i in range(B)]
    res = run_bass_kernel_spmd(nc, in_maps, core_ids=list(range(B)),
                               trace=PROFILE)
    LAST_RESULT = res
    out = np.stack([res.results[i]["out"] for i in range(B)], axis=0)
    return out.astype(np.float32)


if __name__ == "__main__":
    rng = np.random.default_rng(0)
    ins = {
        "x": rng.standard_normal((B, NTOK, DIM), dtype=np.float32),
        "rel_pos_bias": (rng.standard_normal((HEADS, WW, WW)) * 0.1).astype(np.float32),
        "ln1_g": np.ones(DIM, np.float32), "ln1_b": np.zeros(DIM, np.float32),
        "qkv_w": (rng.standard_normal((3 * DIM, DIM)) * 0.02).astype(np.float32),
        "qkv_b": np.zeros(3 * DIM, np.float32),
        "proj_w": (rng.standard_normal((DIM, DIM)) * 0.02).astype(np.float32),
        "proj_b": np.zeros(DIM, np.float32),
        "ln2_g": np.ones(DIM, np.float32), "ln2_b": np.zeros(DIM, np.float32),
        "fc1_w": (rng.standard_normal((MLP_H, DIM)) * 0.02).astype(np.float32),
        "fc1_b": np.zeros(MLP_H, np.float32),
        "fc2_w": (rng.standard_normal((DIM, MLP_H)) * 0.02).astype(np.float32),
        "fc2_b": np.zeros(DIM, np.float32),
        "H": np.int64(64), "W": np.int64(64),
    }
    out = kernel(**ins)
    print("out", out.shape, out.dtype, np.abs(out).max())


# revision 16
# speedup vs baseline: 1.3187x; 1.0392x over previous
"""Swin-style transformer block on 8 Trainium2 NeuronCores.

Sharding: data-parallel over batch — each of the 8 cores processes one image
([4096, 768] tokens). All weights replicated per core. No collectives.

Per-core pipeline:
  P1: LN1 statistics over the 50 window-half-chunks (98 tokens each)
  P2: 13 window-pairs: LN1-apply -> PE transpose -> fused QKV (fp32r, N=392)
      -> per-head scores (fp32r window-paired) -> softmax via exp(S)*exp(B)
      with deferred 1/sum -> transpose P -> O^T -> proj + residual -> x2 DRAM
  P4: LN2 statistics (32 token tiles)
  P5: MLP in two hidden-halves: fc1 (feature-major out) + erf-GELU -> fc2 +
      residual; second half accumulated into the output via gpsimd accum-DMA.

Host-side folds (all exact for the given inputs; SCALE=0.125 is binary-exact):
  - ln1 gamma/beta folded into qkv weights/bias; attention SCALE folded into
    the q block of qkv; v-bias folded into the proj bias (softmax rows sum
    to 1); ln2 gamma/beta folded into fc1; rel_pos_bias shipped as
    exp(rel_pos_bias) so softmax(S+B) = exp(S)*expB / sum(...).
"""

import numpy as np
from contextlib import ExitStack

import concourse.bass as bass
import concourse.mybir as mybir
import concourse.tile as tile
from concourse import bacc
from concourse.bass_utils import run_bass_kernel_spmd
from concourse.masks import make_identity

F32 = mybir.dt.float32
F32R = mybir.dt.float32r
AF = mybir.ActivationFunctionType
OP = mybir.AluOpType

DIM, HEADS, WIN, MLP_H = 768, 12, 14, 3072
B, H0, W0 = 8, 64, 64
NTOK = H0 * W0
NW = 5            # windows per image axis (70/14)
NWIN = NW * NW    # 25 windows
WW = WIN * WIN    # 196 tokens per window
HC = 98           # half-window chunk (7 rows x 14 cols)
DH = DIM // HEADS # 64
EPS = 1e-5

# window pairing: 12 pairs + 1 single
PAIRS = [(2 * i, 2 * i + 1) for i in range(12)] + [(24,)]


def _chunk_geom(w, c):
    """Valid-row/col geometry of half-chunk c (0/1) of window w."""
    wi, wj = divmod(w, NW)
    r0 = wi * WIN + c * 7          # first padded-image row of this chunk
    c0 = wj * WIN
    vr = 7 if (wi < 4 or c == 0) else 1   # wi==4 -> rows 56..63 valid (8)
    vc = 14 if wj < 4 else 8
    return r0, c0, vr, vc


def _gather_chunk(nc, dst, dram, w, c, eng=None):
    """DMA image tokens of half-chunk (w, c) from [4096,768] DRAM into
    dst [98, 768] SBUF tile (partition p = 14*row + col). Pads with zeros."""
    eng = eng or nc.sync
    r0, c0, vr, vc = _chunk_geom(w, c)
    if vr < 7 or vc < 14:
        nc.gpsimd.memset(dst[:, :], 0.0)
    if vc == 14:
        src = bass.AP(tensor=dram, offset=(r0 * W0 + c0) * DIM,
                      ap=[[W0 * DIM, vr], [DIM, 14], [1, DIM]])
        eng.dma_start(dst[0:vr * 14, :], src)
    else:
        for r in range(vr):
            src = bass.AP(tensor=dram, offset=((r0 + r) * W0 + c0) * DIM,
                          ap=[[DIM, vc], [1, DIM]])
            eng.dma_start(dst[r * 14:r * 14 + vc, :], src)


def _scatter_chunk(nc, dram, src, w, c, eng=None):
    """DMA the valid tokens of half-chunk (w, c) from src [98,768] SBUF back
    to token-major [4096,768] DRAM."""
    eng = eng or nc.sync
    r0, c0, vr, vc = _chunk_geom(w, c)
    if vc == 14:
        dst = bass.AP(tensor=dram, offset=(r0 * W0 + c0) * DIM,
                      ap=[[W0 * DIM, vr], [DIM, 14], [1, DIM]])
        eng.dma_start(dst, src[0:vr * 14, :])
    else:
        for r in range(vr):
            dst = bass.AP(tensor=dram, offset=((r0 + r) * W0 + c0) * DIM,
                          ap=[[DIM, vc], [1, DIM]])
            eng.dma_start(dst, src[r * 14:r * 14 + vc, :])


def _ln_stats(nc, pool, xt, n_part, statsM, statsR, col, eps_t):
    """bn_stats/bn_aggr over 768 features -> mean/rstd columns."""
    st = pool.tile([128, 3, 6], F32, name="bnstats")
    for g in range(3):
        nc.vector.bn_stats(st[:n_part, g, :], xt[:n_part, g * 256:(g + 1) * 256])
    mv = pool.tile([128, 2], F32, name="bnaggr")
    nc.vector.bn_aggr(mv[:n_part, :], st[:n_part, :, :])
    nc.gpsimd.tensor_copy(statsM[:n_part, col:col + 1], mv[:n_part, 0:1])
    std = pool.tile([128, 1], F32, name="std")
    nc.scalar.activation(std[:n_part, :], mv[:n_part, 1:2], AF.Sqrt,
                         bias=eps_t[:n_part, :], scale=1.0)
    nc.vector.reciprocal(statsR[:n_part, col:col + 1], std[:n_part, :])


def build_program():
    nc = bacc.Bacc(None, target_bir_lowering=False, debug=False)

    x_d = nc.dram_tensor("x", [NTOK, DIM], F32, kind="ExternalInput")
    qkvw_d = nc.dram_tensor("qkvw", [DIM, 3 * DIM], F32R, kind="ExternalInput")
    qkvb_d = nc.dram_tensor("qkvb", [3 * DIM], F32, kind="ExternalInput")
    projw_d = nc.dram_tensor("projw", [DIM, DIM], F32R, kind="ExternalInput")
    projb_d = nc.dram_tensor("projb", [DIM], F32R, kind="ExternalInput")
    expb_d = nc.dram_tensor("expb", [HC, HEADS, 2, WW], F32, kind="ExternalInput")
    fc1w_d = nc.dram_tensor("fc1w", [DIM, MLP_H], F32R, kind="ExternalInput")
    fc1b_d = nc.dram_tensor("fc1b", [MLP_H], F32, kind="ExternalInput")
    fc2w_d = nc.dram_tensor("fc2w", [MLP_H, DIM], F32R, kind="ExternalInput")
    fc2b_d = nc.dram_tensor("fc2b", [DIM], F32R, kind="ExternalInput")

    out_d = nc.dram_tensor("out", [NTOK, DIM], F32, kind="ExternalOutput")
    x2_d = nc.dram_tensor("x2", [NTOK, DIM], F32)  # internal scratch

    with tile.TileContext(nc) as tc:
        with ExitStack() as g:
            # ---------------- global constants / stats ----------------
            consts = g.enter_context(tc.tile_pool(name="consts", bufs=1))
            ident32 = consts.tile([128, 128], F32)
            make_identity(nc, ident32)
            ident_r = consts.tile([128, 128], F32R)
            nc.vector.tensor_copy(ident_r[:, :], ident32[:, :])
            ones32 = consts.tile([33, 128], F32)
            nc.vector.memset(ones32[:, :], 1.0)
            ones_r = consts.tile([33, 128], F32R)
            nc.vector.tensor_copy(ones_r[:, :], ones32[:, :])
            eps_t = consts.tile([128, 1], F32)
            nc.vector.memset(eps_t[:, :], EPS)
            qkvb_sb = consts.tile([128, 12], F32)
            nc.sync.dma_start(
                qkvb_sb[:, :],
                bass.AP(tensor=qkvb_d, offset=0, ap=[[1, 128], [128, 12]]))
            bias2 = consts.tile([33, DIM], F32R)
            nc.sync.dma_start(bias2[0:1, :],
                              bass.AP(tensor=projb_d, offset=0, ap=[[1, DIM]]))
            nc.sync.dma_start(bias2[32:33, :],
                              bass.AP(tensor=fc2b_d, offset=0, ap=[[1, DIM]]))
            fc1b_sb = consts.tile([128, 24], F32)
            nc.sync.dma_start(
                fc1b_sb[:, :],
                bass.AP(tensor=fc1b_d, offset=0, ap=[[1, 128], [128, 24]]))
            stats2M = consts.tile([128, 32], F32)
            stats2R = consts.tile([128, 32], F32)

            # ---------------- P2: attention over window pairs ----------------
            with ExitStack() as s2:
                wA = s2.enter_context(tc.tile_pool(name="wA", bufs=1))
                qkvw_sb = wA.tile([128, 6, 3 * DIM], F32R)
                for kk in range(6):
                    eng = (nc.sync, nc.scalar, nc.gpsimd)[kk % 3]
                    eng.dma_start(
                        qkvw_sb[:, kk, :],
                        qkvw_d[kk * 128:(kk + 1) * 128, :])
                projw_sb = wA.tile([128, 6, DIM], F32R)
                nc.sync.dma_start(
                    projw_sb[:, :, :],
                    projw_d[:].rearrange("(a p) n -> p a n", p=128))
                expb_sb = wA.tile([HC, HEADS, 2, WW], F32)
                nc.sync.dma_start(expb_sb[:, :, :, :], expb_d[:])

                # natural_log_exp_and_others: exp (softmax) + ln/exp (rstd)
                nc.scalar.add_instruction(mybir.InstLoadActFuncSet(
                    name=nc.get_next_instruction_name(), ins=[], outs=[],
                    act_func_set_id=6))

                pxp = s2.enter_context(tc.tile_pool(name="pxp", bufs=2))
                pln = s2.enter_context(tc.tile_pool(name="pln", bufs=1))
                phT = s2.enter_context(tc.tile_pool(name="phT", bufs=1))
                pqk = s2.enter_context(tc.tile_pool(name="pqk", bufs=1))
                pvT = s2.enter_context(tc.tile_pool(name="pvT", bufs=1))
                psm = s2.enter_context(tc.tile_pool(name="psm", bufs=3))
                ppn = s2.enter_context(tc.tile_pool(name="ppn", bufs=2))
                pUT = s2.enter_context(tc.tile_pool(name="pUT", bufs=1))
                pOT = s2.enter_context(tc.tile_pool(name="pOT", bufs=1))
                px2 = s2.enter_context(tc.tile_pool(name="px2", bufs=3))
                pps = s2.enter_context(tc.tile_pool(name="pps", bufs=8, space="PSUM"))

                for pair in PAIRS:
                    nw = len(pair)
                    PW = nw * WW          # 392 or 196
                    nch = 2 * nw          # chunks in pair

                    x_pair = pxp.tile([HC, 4, DIM], F32, name="x_pair")
                    ln_pair = pln.tile([HC, 4, DIM], F32R, name="ln_pair")
                    for wl, w in enumerate(pair):
                        for c in range(2):
                            i = 2 * wl + c
                            _gather_chunk(nc, x_pair[:, i, :], x_d, w, c,
                                          eng=nc.gpsimd)
                            # LN1 stats inline: rstd = exp(-0.5*ln(var+eps))
                            st = psm.tile([HC, 3, 6], F32, name="bst")
                            for gg in range(3):
                                nc.vector.bn_stats(
                                    st[:, gg, :],
                                    x_pair[:, i, gg * 256:(gg + 1) * 256])
                            mv = psm.tile([HC, 2], F32, name="bmv")
                            nc.vector.bn_aggr(mv[:, :], st[:, :, :])
                            rstd = psm.tile([HC, 2], F32, name="rstd")
                            nc.scalar.activation(rstd[:, 0:1], mv[:, 1:2],
                                                 AF.Ln, bias=eps_t[:HC, :])
                            nc.scalar.activation(rstd[:, 1:2], rstd[:, 0:1],
                                                 AF.Exp, scale=-0.5)
                            nc.vector.tensor_scalar(
                                out=ln_pair[:, i, :], in0=x_pair[:, i, :],
                                scalar1=mv[:, 0:1],
                                scalar2=rstd[:, 1:2],
                                op0=OP.subtract, op1=OP.mult)

                    # transpose ln -> hT [128, 6, PW] (feature-major)
                    hT = phT.tile([128, 6, 2 * WW], F32R, name="hT")
                    for k in range(6):
                        ps_t = pps.tile([128, 392], F32R, tag="ps", name="ps_t")
                        for i in range(nch):
                            nc.tensor.transpose(
                                ps_t[:, i * HC:(i + 1) * HC],
                                ln_pair[:HC, i, k * 128:(k + 1) * 128],
                                ident_r[:HC, :HC])
                        nc.vector.tensor_copy(hT[:, k, :PW], ps_t[:, :PW])

                    # fused qk^T: [1536 feats, PW] (q pre-scaled on host)
                    qkT = pqk.tile([128, 12, 2 * WW], F32R, name="qkT")
                    for m in range(12):
                        ps_qk = pps.tile([128, 392], F32, tag="ps", name="ps_qk")
                        for k in range(6):
                            nc.tensor.matmul(
                                ps_qk[:, :PW],
                                qkvw_sb[:, k, m * 128:(m + 1) * 128],
                                hT[:, k, :PW],
                                start=(k == 0), stop=(k == 5))
                        nc.scalar.activation(qkT[:, m, :PW], ps_qk[:, :PW],
                                             AF.Identity, bias=qkvb_sb[:, m:m + 1])

                    # v token-major: [PW, 768]
                    vT = pvT.tile([HC, 4, DIM], F32R, name="vT")
                    for i in range(nch):
                        for n in range(2):
                            ps_v = pps.tile([128, 392], F32, tag="ps", name="ps_v")
                            for k in range(6):
                                nc.tensor.matmul(
                                    ps_v[:HC, :384],
                                    hT[:, k, i * HC:(i + 1) * HC],
                                    qkvw_sb[:, k, 2 * DIM + n * 384:2 * DIM + (n + 1) * 384],
                                    start=(k == 0), stop=(k == 5))
                            nc.vector.tensor_copy(vT[:, i, n * 384:(n + 1) * 384],
                                                  ps_v[:HC, :384])

                    OT = pOT.tile([128, 6, 2 * WW], F32R, name="OT")

                    def head_stage_a(h):
                        """scores + softmax -> normalized Pn for head h."""
                        qrow = (h % 2) * 64
                        qm = h // 2
                        km = 6 + h // 2
                        Pn = ppn.tile([HC, 4, WW], F32, name="Pn")
                        den = ppn.tile([HC, 4], F32, name="den")
                        rden = ppn.tile([HC, 4], F32, name="rden")
                        for qc in range(nch):
                            wl = qc // 2
                            ps_S = pps.tile([128, 392], F32, tag="ps", name="ps_S")
                            nc.tensor.matmul(
                                ps_S[:HC, :PW],
                                qkT[qrow:qrow + 64, qm, qc * HC:(qc + 1) * HC],
                                qkT[qrow:qrow + 64, km, :PW],
                                start=True, stop=True)
                            U = psm.tile([HC, 2 * WW], F32, name="U")
                            nc.scalar.activation(U[:, :PW], ps_S[:HC, :PW], AF.Exp)
                            P = psm.tile([HC, WW], F32, name="P")
                            nc.vector.scalar_tensor_tensor(
                                out=P[:, :], in0=U[:, wl * WW:(wl + 1) * WW],
                                scalar=1.0, in1=expb_sb[:, h, qc % 2, :],
                                op0=OP.mult, op1=OP.mult,
                                accum_out=den[:, qc:qc + 1])
                            nc.vector.reciprocal(rden[:, qc:qc + 1],
                                                 den[:, qc:qc + 1])
                            nc.gpsimd.tensor_scalar_mul(
                                Pn[:, qc, :], P[:, :], rden[:, qc:qc + 1])
                        return Pn

                    def head_stage_b(h, Pn):
                        """transpose Pn -> UT, then O^T = V^T @ U^T."""
                        qrow = (h % 2) * 64
                        qm = h // 2
                        UT = pUT.tile([HC, 2, 2 * WW], F32R, name="UT")
                        for kc in range(2):
                            ps_PT = pps.tile([128, 392], F32, tag="ps", name="ps_PT")
                            for qc in range(nch):
                                nc.tensor.transpose(
                                    ps_PT[:HC, qc * HC:(qc + 1) * HC],
                                    Pn[:HC, qc, kc * HC:(kc + 1) * HC],
                                    ident32[:HC, :HC])
                            nc.scalar.copy(UT[:, kc, :PW], ps_PT[:HC, :PW])
                        for wl in range(nw):
                            ps_O = pps.tile([128, 392], F32, tag="ps", name="ps_O")
                            for kc in range(2):
                                nc.tensor.matmul(
                                    ps_O[:64, :PW],
                                    vT[:, 2 * wl + kc, h * 64:(h + 1) * 64],
                                    UT[:, kc, :PW],
                                    start=(kc == 0), stop=(kc == 1))
                            nc.vector.tensor_copy(
                                OT[qrow:qrow + 64, qm, wl * WW:(wl + 1) * WW],
                                ps_O[:64, wl * WW:(wl + 1) * WW])

                    # 2-stage pipeline over heads: S(h+1) issues before PT/O(h)
                    prev = None
                    for h in range(HEADS):
                        Pn = head_stage_a(h)
                        if prev is not None:
                            head_stage_b(h - 1, prev)
                        prev = Pn
                    head_stage_b(HEADS - 1, prev)

                    # proj + residual -> x2 (per chunk, scatter immediately)
                    for i in range(nch):
                        x2c = px2.tile([HC, DIM], F32, name="x2c")
                        for n in range(2):
                            ps_pr = pps.tile([128, 392], F32, tag="ps", name="ps_pr")
                            for k in range(6):
                                nc.tensor.matmul(
                                    ps_pr[:HC, :384],
                                    OT[:, k, i * HC:(i + 1) * HC],
                                    projw_sb[:, k, n * 384:(n + 1) * 384],
                                    start=(k == 0), stop=False)
                            nc.tensor.matmul(
                                ps_pr[:HC, :384],
                                ones_r[0:1, :HC],
                                bias2[0:1, n * 384:(n + 1) * 384],
                                start=False, stop=True)
                            nc.vector.tensor_tensor(
                                out=x2c[:, n * 384:(n + 1) * 384],
                                in0=ps_pr[:HC, :384],
                                in1=x_pair[:, i, n * 384:(n + 1) * 384],
                                op=OP.add)
                        _scatter_chunk(nc, x2_d, x2c[:, :], pair[i // 2], i % 2,
                                       eng=nc.sync)

            # ---------------- P4 + P5 (MLP) ----------------
            with ExitStack() as s5:
                # weight pool first so the half-0 prefetch DMAs overlap the
                # LN2-stats pass (no address reuse WAR on P4's pools)
                wB = s5.enter_context(tc.tile_pool(name="wB", bufs=1))

                def load_fc_weights(half):
                    fc1w_sb = wB.tile([128, 6, MLP_H // 2], F32R, name="fc1w_sb")
                    nc.gpsimd.dma_start(
                        fc1w_sb[:, :, :],
                        fc1w_d[:, half * 1536:(half + 1) * 1536]
                        .rearrange("(a p) n -> p a n", p=128))
                    fc2w_sb = wB.tile([128, 12, DIM], F32R, name="fc2w_sb")
                    nc.gpsimd.dma_start(
                        fc2w_sb[:, :, :],
                        fc2w_d[half * 1536:(half + 1) * 1536, :]
                        .rearrange("(a p) n -> p a n", p=128))
                    return fc1w_sb, fc2w_sb

                w_half0 = load_fc_weights(0)

                pxt = s5.enter_context(tc.tile_pool(name="pxt", bufs=4))
                pxn = s5.enter_context(tc.tile_pool(name="pxn", bufs=2))
                pxnT = s5.enter_context(tc.tile_pool(name="pxnT", bufs=2))
                pgT = s5.enter_context(tc.tile_pool(name="pgT", bufs=2))
                pout = s5.enter_context(tc.tile_pool(name="pout", bufs=4))
                pps5 = s5.enter_context(tc.tile_pool(name="pps5", bufs=8, space="PSUM"))


                for half in range(2):
                    fc1w_sb, fc2w_sb = w_half0 if half == 0 else load_fc_weights(1)

                    for st in range(16):   # super-tiles of 256 tokens
                        x2t = []
                        x2nT = pxnT.tile([128, 6, 256], F32R, name="x2nT")
                        for c in range(2):
                            t = st * 2 + c
                            xt = pxt.tile([128, DIM], F32, name="x2t")
                            eng = nc.scalar if t % 2 == 0 else nc.sync
                            eng.dma_start(xt[:, :], x2_d[t * 128:(t + 1) * 128, :])
                            x2t.append(xt)
                            if half == 0:
                                # LN2 stats inline; rsqrt on DVE only
                                # (quake seed + 3 Newton steps, ~1.5e-7 rel)
                                st5 = pxn.tile([128, 3, 6], F32, name="st5")
                                for gg in range(3):
                                    nc.vector.bn_stats(
                                        st5[:, gg, :],
                                        xt[:, gg * 256:(gg + 1) * 256])
                                mv5 = pxn.tile([128, 8], F32, name="mv5")
                                nc.vector.bn_aggr(mv5[:, 0:2], st5[:, :, :])
                                nc.gpsimd.tensor_copy(stats2M[:, t:t + 1],
                                                      mv5[:, 0:1])
                                ve = mv5[:, 2:3]
                                nc.vector.tensor_scalar(
                                    out=ve, in0=mv5[:, 1:2], scalar1=EPS,
                                    scalar2=None, op0=OP.add)
                                yi = mv5[:, 3:4].bitcast(mybir.dt.int32)
                                nc.vector.tensor_scalar(
                                    out=yi, in0=ve.bitcast(mybir.dt.int32),
                                    scalar1=1, scalar2=None,
                                    op0=OP.arith_shift_right)
                                y0 = mv5[:, 4:5].bitcast(mybir.dt.int32)
                                nc.vector.tensor_scalar(
                                    out=y0, in0=yi, scalar1=-1,
                                    scalar2=None, op0=OP.bitwise_xor)
                                nc.vector.tensor_scalar(
                                    out=y0, in0=y0, scalar1=0x5f3759e0,
                                    scalar2=None, op0=OP.add)
                                ya = mv5[:, 4:5]
                                yb = mv5[:, 5:6]
                                t2 = mv5[:, 6:7]
                                w5 = mv5[:, 7:8]
                                for _ in range(3):
                                    nc.vector.tensor_tensor(
                                        out=t2, in0=ya, in1=ya, op=OP.mult)
                                    nc.vector.scalar_tensor_tensor(
                                        out=w5, in0=ve, scalar=-0.5, in1=t2,
                                        op0=OP.mult, op1=OP.mult)
                                    nc.vector.tensor_scalar(
                                        out=w5, in0=w5, scalar1=1.5,
                                        scalar2=None, op0=OP.add)
                                    nc.vector.tensor_tensor(
                                        out=yb, in0=ya, in1=w5, op=OP.mult)
                                    ya, yb = yb, ya
                                nc.gpsimd.tensor_copy(stats2R[:, t:t + 1], ya)
                            xn = pxn.tile([128, DIM], F32R, name="x2n")
                            nc.vector.tensor_scalar(
                                out=xn[:, :], in0=xt[:, :],
                                scalar1=stats2M[:, t:t + 1],
                                scalar2=stats2R[:, t:t + 1],
                                op0=OP.subtract, op1=OP.mult)
                            for k in range(6):
                                ps_t2 = pps5.tile([128, 384], F32R, tag="ps5",
                                                  name="ps_t2")
                                nc.tensor.transpose(
                                    ps_t2[:, :128], xn[:, k * 128:(k + 1) * 128],
                                    ident_r[:, :])
                                nc.vector.tensor_copy(
                                    x2nT[:, k, c * 128:(c + 1) * 128],
                                    ps_t2[:, :128])
                        gT = pgT.tile([128, 12, 256], F32R, name="gT")
                        for m in range(12):
                            ps_f1 = pps5.tile([128, 384], F32, tag="ps5", name="ps_f1")
                            for k in range(6):
                                nc.tensor.matmul(
                                    ps_f1[:, :256],
                                    fc1w_sb[:, k, m * 128:(m + 1) * 128],
                                    x2nT[:, k, :],
                                    start=(k == 0), stop=(k == 5))
                            nc.scalar.activation(
                                gT[:, m, :], ps_f1[:, :256], AF.Gelu,
                                bias=fc1b_sb[:, half * 12 + m:half * 12 + m + 1])
                        for c in range(2):
                            outt = pout.tile([128, DIM], F32, name="outt")
                            for n in range(2):
                                ps_f2 = pps5.tile([128, 384], F32, tag="ps5",
                                                  name="ps_f2")
                                for m in range(12):
                                    nc.tensor.matmul(
                                        ps_f2[:, :384],
                                        gT[:, m, c * 128:(c + 1) * 128],
                                        fc2w_sb[:, m, n * 384:(n + 1) * 384],
                                        start=(m == 0), stop=(m == 11 and half == 1))
                                if half == 0:
                                    nc.tensor.matmul(
                                        ps_f2[:, :384],
                                        ones_r[32:33, :128],
                                        bias2[32:33, n * 384:(n + 1) * 384],
                                        start=False, stop=True)
                                    nc.vector.tensor_tensor(
                                        out=outt[:, n * 384:(n + 1) * 384],
                                        in0=ps_f2[:, :384],
                                        in1=x2t[c][:, n * 384:(n + 1) * 384],
                                        op=OP.add)
                                else:
                                    nc.vector.tensor_copy(
                                        outt[:, n * 384:(n + 1) * 384],
                                        ps_f2[:, :384])
                            t = st * 2 + c
                            if half == 0:
                                nc.sync.dma_start(
                                    out_d[t * 128:(t + 1) * 128, :], outt[:, :])
                            else:
                                nc.gpsimd.dma_start(
                                    out_d[t * 128:(t + 1) * 128, :], outt[:, :],
                                    accum_op=OP.add)

    nc.compile()
    return nc


_NC_CACHE = {}


def _get_nc():
    if "nc" not in _NC_CACHE:
        _NC_CACHE["nc"] = build_program()
    return _NC_CACHE["nc"]


def _prep_weights(inputs):
    f = lambda k: np.asarray(inputs[k], np.float32)
    x = f("x")
    ln1_g, ln1_b = f("ln1_g"), f("ln1_b")
    ln2_g, ln2_b = f("ln2_g"), f("ln2_b")
    qkv_w, qkv_b = f("qkv_w"), f("qkv_b")
    proj_w, proj_b = f("proj_w"), f("proj_b")
    fc1_w, fc1_b = f("fc1_w"), f("fc1_b")
    fc2_w, fc2_b = f("fc2_w"), f("fc2_b")
    rel = f("rel_pos_bias")
    SCALE = np.float32((DIM // HEADS) ** -0.5)

    # fold ln1 into qkv
    qkv_w_f = qkv_w * ln1_g[None, :]
    qkv_b_f = qkv_w @ ln1_b + qkv_b
    # fold attention scale into q block
    qkv_w_f[:DIM] *= SCALE
    qkv_b_f2 = qkv_b_f.copy()
    qkv_b_f2[:DIM] *= SCALE
    # fold v bias into proj bias (softmax rows sum to 1)
    projb = proj_b + proj_w @ qkv_b_f[2 * DIM:]
    # fold ln2 into fc1
    fc1_w_f = fc1_w * ln2_g[None, :]
    fc1_b_f = fc1_w @ ln2_b + fc1_b

    expb = np.exp(rel).astype(np.float32)          # [12, 196, 196]
    expb = expb.reshape(HEADS, 2, HC, WW).transpose(2, 0, 1, 3).copy()

    return {
        "qkvw": np.ascontiguousarray(qkv_w_f.T),    # [768, 2304]
        "qkvb": qkv_b_f2,
        "projw": np.ascontiguousarray(proj_w.T),    # [768, 768]
        "projb": projb,
        "expb": expb,
        "fc1w": np.ascontiguousarray(fc1_w_f.T),    # [768, 3072]
        "fc1b": fc1_b_f,
        "fc2w": np.ascontiguousarray(fc2_w.T),      # [3072, 768]
        "fc2b": fc2_b,
    }, x


PROFILE = False
LAST_RESULT = None


def kernel(**inputs):
    global LAST_RESULT
    weights, x = _prep_weights(inputs)
    nc = _get_nc()
    in_maps = [dict(weights, x=np.ascontiguousarray(x[i])) for i in range(B)]
    res = run_bass_kernel_spmd(nc, in_maps, core_ids=list(range(B)),
                               trace=PROFILE)
    LAST_RESULT = res
    out = np.stack([res.results[i]["out"] for i in range(B)], axis=0)
    return out.astype(np.float32)


if __name__ == "__main__":
    rng = np.random.default_rng(0)
    ins = {
        "x": rng.standard_normal((B, NTOK, DIM), dtype=np.float32),
        "rel_pos_bias": (rng.standard_normal((HEADS, WW, WW)) * 0.1).astype(np.float32),
        "ln1_g": np.ones(DIM, np.float32), "ln1_b": np.zeros(DIM, np.float32),
        "qkv_w": (rng.standard_normal((3 * DIM, DIM)) * 0.02).astype(np.float32),
        "qkv_b": np.zeros(3 * DIM, np.float32),
        "proj_w": (rng.standard_normal((DIM, DIM)) * 0.02).astype(np.float32),
        "proj_b": np.zeros(DIM, np.float32),
        "ln2_g": np.ones(DIM, np.float32), "ln2_b": np.zeros(DIM, np.float32),
        "fc1_w": (rng.standard_normal((MLP_H, DIM)) * 0.02).astype(np.float32),
        "fc1_b": np.zeros(MLP_H, np.float32),
        "fc2_w": (rng.standard_normal((DIM, MLP_H)) * 0.02).astype(np.float32),
        "fc2_b": np.zeros(DIM, np.float32),
        "H": np.int64(64), "W": np.int64(64),
    }
    out = kernel(**ins)
    print("out", out.shape, out.dtype, np.abs(out).max())


# revision 18
# speedup vs baseline: 1.6609x; 1.2594x over previous
"""Swin-style transformer block on 8 Trainium2 NeuronCores.

Sharding: data-parallel over batch — each of the 8 cores processes one image
([4096, 768] tokens). All weights replicated per core. No collectives.

Per-core pipeline:
  P1: LN1 statistics over the 50 window-half-chunks (98 tokens each)
  P2: 13 window-pairs: LN1-apply -> PE transpose -> fused QKV (fp32r, N=392)
      -> per-head scores (fp32r window-paired) -> softmax via exp(S)*exp(B)
      with deferred 1/sum -> transpose P -> O^T -> proj + residual -> x2 DRAM
  P4: LN2 statistics (32 token tiles)
  P5: MLP in two hidden-halves: fc1 (feature-major out) + erf-GELU -> fc2 +
      residual; second half accumulated into the output via gpsimd accum-DMA.

Host-side folds (all exact for the given inputs; SCALE=0.125 is binary-exact):
  - ln1 gamma/beta folded into qkv weights/bias; attention SCALE folded into
    the q block of qkv; v-bias folded into the proj bias (softmax rows sum
    to 1); ln2 gamma/beta folded into fc1; rel_pos_bias shipped as
    exp(rel_pos_bias) so softmax(S+B) = exp(S)*expB / sum(...).
"""

import numpy as np
from contextlib import ExitStack

import concourse.bass as bass
import concourse.mybir as mybir
import concourse.tile as tile
from concourse import bacc
from concourse.bass_utils import run_bass_kernel_spmd
from concourse.masks import make_identity

F32 = mybir.dt.float32
F32R = mybir.dt.float32r
AF = mybir.ActivationFunctionType
OP = mybir.AluOpType

DIM, HEADS, WIN, MLP_H = 768, 12, 14, 3072
B, H0, W0 = 8, 64, 64
NTOK = H0 * W0
NW = 5            # windows per image axis (70/14)
NWIN = NW * NW    # 25 windows
WW = WIN * WIN    # 196 tokens per window
HC = 98           # half-window chunk (7 rows x 14 cols)
DH = DIM // HEADS # 64
EPS = 1e-5

# window pairing: 12 pairs + 1 single
PAIRS = [(2 * i, 2 * i + 1) for i in range(12)] + [(24,)]


def _chunk_geom(w, c):
    """Valid-row/col geometry of half-chunk c (0/1) of window w."""
    wi, wj = divmod(w, NW)
    r0 = wi * WIN + c * 7          # first padded-image row of this chunk
    c0 = wj * WIN
    vr = 7 if (wi < 4 or c == 0) else 1   # wi==4 -> rows 56..63 valid (8)
    vc = 14 if wj < 4 else 8
    return r0, c0, vr, vc


def _gather_chunk(nc, dst, dram, w, c, eng=None):
    """DMA image tokens of half-chunk (w, c) from [4096,768] DRAM into
    dst [98, 768] SBUF tile (partition p = 14*row + col). Pads with zeros."""
    eng = eng or nc.sync
    r0, c0, vr, vc = _chunk_geom(w, c)
    if vr < 7 or vc < 14:
        nc.gpsimd.memset(dst[:, :], 0.0)
    if vc == 14:
        src = bass.AP(tensor=dram, offset=(r0 * W0 + c0) * DIM,
                      ap=[[W0 * DIM, vr], [DIM, 14], [1, DIM]])
        eng.dma_start(dst[0:vr * 14, :], src)
    else:
        for r in range(vr):
            src = bass.AP(tensor=dram, offset=((r0 + r) * W0 + c0) * DIM,
                          ap=[[DIM, vc], [1, DIM]])
            eng.dma_start(dst[r * 14:r * 14 + vc, :], src)


def _scatter_chunk(nc, dram, src, w, c, eng=None):
    """DMA the valid tokens of half-chunk (w, c) from src [98,768] SBUF back
    to token-major [4096,768] DRAM."""
    eng = eng or nc.sync
    r0, c0, vr, vc = _chunk_geom(w, c)
    if vc == 14:
        dst = bass.AP(tensor=dram, offset=(r0 * W0 + c0) * DIM,
                      ap=[[W0 * DIM, vr], [DIM, 14], [1, DIM]])
        eng.dma_start(dst, src[0:vr * 14, :])
    else:
        for r in range(vr):
            dst = bass.AP(tensor=dram, offset=((r0 + r) * W0 + c0) * DIM,
                          ap=[[DIM, vc], [1, DIM]])
            eng.dma_start(dst, src[r * 14:r * 14 + vc, :])


def _ln_stats(nc, pool, xt, n_part, statsM, statsR, col, eps_t):
    """bn_stats/bn_aggr over 768 features -> mean/rstd columns."""
    st = pool.tile([128, 3, 6], F32, name="bnstats")
    for g in range(3):
        nc.vector.bn_stats(st[:n_part, g, :], xt[:n_part, g * 256:(g + 1) * 256])
    mv = pool.tile([128, 2], F32, name="bnaggr")
    nc.vector.bn_aggr(mv[:n_part, :], st[:n_part, :, :])
    nc.gpsimd.tensor_copy(statsM[:n_part, col:col + 1], mv[:n_part, 0:1])
    std = pool.tile([128, 1], F32, name="std")
    nc.scalar.activation(std[:n_part, :], mv[:n_part, 1:2], AF.Sqrt,
                         bias=eps_t[:n_part, :], scale=1.0)
    nc.vector.reciprocal(statsR[:n_part, col:col + 1], std[:n_part, :])


def build_program():
    nc = bacc.Bacc(None, target_bir_lowering=False, debug=False)

    x_d = nc.dram_tensor("x", [NTOK, DIM], F32, kind="ExternalInput")
    qkvw_d = nc.dram_tensor("qkvw", [DIM, 3 * DIM], F32R, kind="ExternalInput")
    qkvb_d = nc.dram_tensor("qkvb", [3 * DIM], F32, kind="ExternalInput")
    projw_d = nc.dram_tensor("projw", [DIM, DIM], F32R, kind="ExternalInput")
    projb_d = nc.dram_tensor("projb", [DIM], F32R, kind="ExternalInput")
    expb_d = nc.dram_tensor("expb", [HC, HEADS, 2, WW], F32, kind="ExternalInput")
    fc1w_d = nc.dram_tensor("fc1w", [DIM, MLP_H], F32R, kind="ExternalInput")
    fc1b_d = nc.dram_tensor("fc1b", [MLP_H], F32, kind="ExternalInput")
    fc2w_d = nc.dram_tensor("fc2w", [MLP_H, DIM], F32R, kind="ExternalInput")
    fc2b_d = nc.dram_tensor("fc2b", [DIM], F32R, kind="ExternalInput")

    out_d = nc.dram_tensor("out", [NTOK, DIM], F32, kind="ExternalOutput")
    x2_d = nc.dram_tensor("x2", [NTOK, DIM], F32)  # internal scratch

    with tile.TileContext(nc) as tc:
        with ExitStack() as g:
            # ---------------- global constants / stats ----------------
            consts = g.enter_context(tc.tile_pool(name="consts", bufs=1))
            ident32 = consts.tile([128, 128], F32)
            make_identity(nc, ident32)
            ident_r = consts.tile([128, 128], F32R)
            nc.vector.tensor_copy(ident_r[:, :], ident32[:, :])
            ones32 = consts.tile([33, 128], F32)
            nc.vector.memset(ones32[:, :], 1.0)
            ones_r = consts.tile([33, 128], F32R)
            nc.vector.tensor_copy(ones_r[:, :], ones32[:, :])
            eps_t = consts.tile([128, 1], F32)
            nc.vector.memset(eps_t[:, :], EPS)
            qkvb_sb = consts.tile([128, 12], F32)
            nc.sync.dma_start(
                qkvb_sb[:, :],
                bass.AP(tensor=qkvb_d, offset=0, ap=[[1, 128], [128, 12]]))
            bias2 = consts.tile([33, DIM], F32R)
            nc.sync.dma_start(bias2[0:1, :],
                              bass.AP(tensor=projb_d, offset=0, ap=[[1, DIM]]))
            nc.sync.dma_start(bias2[32:33, :],
                              bass.AP(tensor=fc2b_d, offset=0, ap=[[1, DIM]]))
            fc1b_sb = consts.tile([128, 24], F32)
            nc.sync.dma_start(
                fc1b_sb[:, :],
                bass.AP(tensor=fc1b_d, offset=0, ap=[[1, 128], [128, 24]]))
            stats2M = consts.tile([128, 32], F32)
            stats2R = consts.tile([128, 32], F32)

            # ---------------- P2: attention over window pairs ----------------
            with ExitStack() as s2:
                wA = s2.enter_context(tc.tile_pool(name="wA", bufs=1))
                qkvw_sb = wA.tile([128, 6, 3 * DIM], F32R)
                for kk in range(6):
                    for hh in range(2):
                        eng = (nc.sync, nc.scalar, nc.gpsimd)[(2 * kk + hh) % 3]
                        eng.dma_start(
                            qkvw_sb[:, kk, hh * 1152:(hh + 1) * 1152],
                            qkvw_d[kk * 128:(kk + 1) * 128,
                                   hh * 1152:(hh + 1) * 1152])
                projw_sb = wA.tile([128, 6, DIM], F32R)
                for kk in range(3):
                    eng = (nc.sync, nc.scalar, nc.gpsimd)[kk]
                    nc.sync.dma_start(
                        projw_sb[:, 2 * kk:2 * kk + 2, :],
                        projw_d[kk * 256:(kk + 1) * 256, :]
                        .rearrange("(a p) n -> p a n", p=128))
                expb_sb = wA.tile([HC, HEADS, 2, WW], F32)
                for hh in range(3):
                    eng = (nc.sync, nc.scalar, nc.gpsimd)[hh]
                    eng.dma_start(expb_sb[:, 4 * hh:4 * (hh + 1), :, :],
                                  expb_d[:, 4 * hh:4 * (hh + 1), :, :])

                # natural_log_exp_and_others: exp (softmax) + ln/exp (rstd)
                nc.scalar.add_instruction(mybir.InstLoadActFuncSet(
                    name=nc.get_next_instruction_name(), ins=[], outs=[],
                    act_func_set_id=6))

                pxp = s2.enter_context(tc.tile_pool(name="pxp", bufs=2))
                pln = s2.enter_context(tc.tile_pool(name="pln", bufs=1))
                phT = s2.enter_context(tc.tile_pool(name="phT", bufs=1))
                pqk = s2.enter_context(tc.tile_pool(name="pqk", bufs=1))
                pvT = s2.enter_context(tc.tile_pool(name="pvT", bufs=1))
                psm = s2.enter_context(tc.tile_pool(name="psm", bufs=3))
                ppn = s2.enter_context(tc.tile_pool(name="ppn", bufs=2))
                pUT = s2.enter_context(tc.tile_pool(name="pUT", bufs=1))
                pOT = s2.enter_context(tc.tile_pool(name="pOT", bufs=1))
                px2 = s2.enter_context(tc.tile_pool(name="px2", bufs=3))
                pps = s2.enter_context(tc.tile_pool(name="pps", bufs=8, space="PSUM"))

                for pair in PAIRS:
                    nw = len(pair)
                    PW = nw * WW          # 392 or 196
                    nch = 2 * nw          # chunks in pair

                    x_pair = pxp.tile([HC, 4, DIM], F32, name="x_pair")
                    ln_pair = pln.tile([HC, 4, DIM], F32R, name="ln_pair")
                    for wl, w in enumerate(pair):
                        for c in range(2):
                            i = 2 * wl + c
                            _gather_chunk(nc, x_pair[:, i, :], x_d, w, c,
                                          eng=nc.gpsimd)
                            # LN1 stats inline: rstd = exp(-0.5*ln(var+eps))
                            st = psm.tile([HC, 3, 6], F32, name="bst")
                            for gg in range(3):
                                nc.vector.bn_stats(
                                    st[:, gg, :],
                                    x_pair[:, i, gg * 256:(gg + 1) * 256])
                            mv = psm.tile([HC, 2], F32, name="bmv")
                            nc.vector.bn_aggr(mv[:, :], st[:, :, :])
                            rstd = psm.tile([HC, 2], F32, name="rstd")
                            nc.scalar.activation(rstd[:, 0:1], mv[:, 1:2],
                                                 AF.Ln, bias=eps_t[:HC, :])
                            nc.scalar.activation(rstd[:, 1:2], rstd[:, 0:1],
                                                 AF.Exp, scale=-0.5)
                            nc.vector.tensor_scalar(
                                out=ln_pair[:, i, :], in0=x_pair[:, i, :],
                                scalar1=mv[:, 0:1],
                                scalar2=rstd[:, 1:2],
                                op0=OP.subtract, op1=OP.mult)

                    # transpose ln -> hT [128, 6, PW] (feature-major)
                    hT = phT.tile([128, 6, 2 * WW], F32R, name="hT")
                    for k in range(6):
                        ps_t = pps.tile([128, 392], F32R, tag="ps", name="ps_t")
                        for i in range(nch):
                            nc.tensor.transpose(
                                ps_t[:, i * HC:(i + 1) * HC],
                                ln_pair[:HC, i, k * 128:(k + 1) * 128],
                                ident_r[:HC, :HC])
                        nc.vector.tensor_copy(hT[:, k, :PW], ps_t[:, :PW])

                    # fused qk^T: [1536 feats, PW] (q pre-scaled on host)
                    qkT = pqk.tile([128, 12, 2 * WW], F32R, name="qkT")
                    for m in range(12):
                        ps_qk = pps.tile([128, 392], F32, tag="ps", name="ps_qk")
                        for k in range(6):
                            nc.tensor.matmul(
                                ps_qk[:, :PW],
                                qkvw_sb[:, k, m * 128:(m + 1) * 128],
                                hT[:, k, :PW],
                                start=(k == 0), stop=(k == 5))
                        nc.scalar.activation(qkT[:, m, :PW], ps_qk[:, :PW],
                                             AF.Identity, bias=qkvb_sb[:, m:m + 1])

                    # v token-major: [PW, 768]
                    vT = pvT.tile([HC, 4, DIM], F32R, name="vT")
                    for i in range(nch):
                        for n in range(2):
                            ps_v = pps.tile([128, 392], F32, tag="ps", name="ps_v")
                            for k in range(6):
                                nc.tensor.matmul(
                                    ps_v[:HC, :384],
                                    hT[:, k, i * HC:(i + 1) * HC],
                                    qkvw_sb[:, k, 2 * DIM + n * 384:2 * DIM + (n + 1) * 384],
                                    start=(k == 0), stop=(k == 5))
                            nc.vector.tensor_copy(vT[:, i, n * 384:(n + 1) * 384],
                                                  ps_v[:HC, :384])

                    OT = pOT.tile([128, 6, 2 * WW], F32R, name="OT")

                    def head_stage_a(h):
                        """scores + softmax -> normalized Pn for head h."""
                        qrow = (h % 2) * 64
                        qm = h // 2
                        km = 6 + h // 2
                        Pn = ppn.tile([HC, 4, WW], F32, name="Pn")
                        den = ppn.tile([HC, 4], F32, name="den")
                        rden = ppn.tile([HC, 4], F32, name="rden")
                        for qc in range(nch):
                            wl = qc // 2
                            ps_S = pps.tile([128, 392], F32, tag="ps", name="ps_S")
                            nc.tensor.matmul(
                                ps_S[:HC, :PW],
                                qkT[qrow:qrow + 64, qm, qc * HC:(qc + 1) * HC],
                                qkT[qrow:qrow + 64, km, :PW],
                                start=True, stop=True)
                            U = psm.tile([HC, 2 * WW], F32, name="U")
                            nc.scalar.activation(U[:, :PW], ps_S[:HC, :PW], AF.Exp)
                            P = psm.tile([HC, WW], F32, name="P")
                            nc.vector.scalar_tensor_tensor(
                                out=P[:, :], in0=U[:, wl * WW:(wl + 1) * WW],
                                scalar=1.0, in1=expb_sb[:, h, qc % 2, :],
                                op0=OP.mult, op1=OP.mult,
                                accum_out=den[:, qc:qc + 1])
                            nc.vector.reciprocal(rden[:, qc:qc + 1],
                                                 den[:, qc:qc + 1])
                            nc.vector.tensor_scalar_mul(
                                Pn[:, qc, :], P[:, :], rden[:, qc:qc + 1])
                        return Pn

                    def head_stage_b(h, Pn):
                        """transpose Pn -> UT, then O^T = V^T @ U^T."""
                        qrow = (h % 2) * 64
                        qm = h // 2
                        UT = pUT.tile([HC, 2, 2 * WW], F32R, name="UT")
                        ps_PT = [pps.tile([128, 392], F32, tag="ps",
                                          name="ps_PT") for _ in range(2)]
                        for qc in range(nch):     # qc-major: unblock early
                            for kc in range(2):
                                nc.tensor.transpose(
                                    ps_PT[kc][:HC, qc * HC:(qc + 1) * HC],
                                    Pn[:HC, qc, kc * HC:(kc + 1) * HC],
                                    ident32[:HC, :HC])
                        for kc in range(2):
                            nc.scalar.copy(UT[:, kc, :PW], ps_PT[kc][:HC, :PW])
                        for wl in range(nw):
                            ps_O = pps.tile([128, 392], F32, tag="ps", name="ps_O")
                            for kc in range(2):
                                nc.tensor.matmul(
                                    ps_O[:64, :PW],
                                    vT[:, 2 * wl + kc, h * 64:(h + 1) * 64],
                                    UT[:, kc, :PW],
                                    start=(kc == 0), stop=(kc == 1))
                            nc.vector.tensor_copy(
                                OT[qrow:qrow + 64, qm, wl * WW:(wl + 1) * WW],
                                ps_O[:64, wl * WW:(wl + 1) * WW])

                    # 2-stage pipeline over heads: S(h+1) issues before PT/O(h)
                    prev = None
                    for h in range(HEADS):
                        Pn = head_stage_a(h)
                        if prev is not None:
                            head_stage_b(h - 1, prev)
                        prev = Pn
                    head_stage_b(HEADS - 1, prev)

                    # proj + residual -> x2 (per chunk, scatter immediately)
                    for i in range(nch):
                        x2c = px2.tile([HC, DIM], F32, name="x2c")
                        for n in range(2):
                            ps_pr = pps.tile([128, 392], F32, tag="ps", name="ps_pr")
                            for k in range(6):
                                nc.tensor.matmul(
                                    ps_pr[:HC, :384],
                                    OT[:, k, i * HC:(i + 1) * HC],
                                    projw_sb[:, k, n * 384:(n + 1) * 384],
                                    start=(k == 0), stop=False)
                            nc.tensor.matmul(
                                ps_pr[:HC, :384],
                                ones_r[0:1, :HC],
                                bias2[0:1, n * 384:(n + 1) * 384],
                                start=False, stop=True)
                            nc.vector.tensor_tensor(
                                out=x2c[:, n * 384:(n + 1) * 384],
                                in0=ps_pr[:HC, :384],
                                in1=x_pair[:, i, n * 384:(n + 1) * 384],
                                op=OP.add)
                        _scatter_chunk(nc, x2_d, x2c[:, :], pair[i // 2], i % 2,
                                       eng=nc.sync)

            # ---------------- P4 + P5 (MLP) ----------------
            with ExitStack() as s5:
                # weight pool first so the half-0 prefetch DMAs overlap the
                # LN2-stats pass (no address reuse WAR on P4's pools)
                wB = s5.enter_context(tc.tile_pool(name="wB", bufs=1))

                def load_fc_weights(half):
                    engs = (nc.gpsimd, nc.sync, nc.scalar)
                    fc1w_sb = wB.tile([128, 6, MLP_H // 2], F32R, name="fc1w_sb")
                    for kk in range(6):
                        engs[kk % 3].dma_start(
                            fc1w_sb[:, kk, :],
                            fc1w_d[kk * 128:(kk + 1) * 128,
                                   half * 1536:(half + 1) * 1536])
                    fc2w_sb = wB.tile([128, 12, DIM], F32R, name="fc2w_sb")
                    for kk in range(4):
                        engs[kk % 3].dma_start(
                            fc2w_sb[:, 3 * kk:3 * (kk + 1), :],
                            fc2w_d[half * 1536 + kk * 384:
                                   half * 1536 + (kk + 1) * 384, :]
                            .rearrange("(a p) n -> p a n", p=128))
                    return fc1w_sb, fc2w_sb

                w_half0 = load_fc_weights(0)

                pxt = s5.enter_context(tc.tile_pool(name="pxt", bufs=4))
                pxn = s5.enter_context(tc.tile_pool(name="pxn", bufs=2))
                pxnT = s5.enter_context(tc.tile_pool(name="pxnT", bufs=2))
                pgT = s5.enter_context(tc.tile_pool(name="pgT", bufs=2))
                pout = s5.enter_context(tc.tile_pool(name="pout", bufs=4))
                pps5 = s5.enter_context(tc.tile_pool(name="pps5", bufs=8, space="PSUM"))


                for half in range(2):
                    fc1w_sb, fc2w_sb = w_half0 if half == 0 else load_fc_weights(1)

                    def mlp_prep(st):
                        """load x2 tiles, (half0: LN2 stats), apply, transpose."""
                        x2t = []
                        x2nT = pxnT.tile([128, 6, 256], F32R, name="x2nT")
                        for c in range(2):
                            t = st * 2 + c
                            xt = pxt.tile([128, DIM], F32, name="x2t")
                            eng = nc.scalar if t % 2 == 0 else nc.sync
                            eng.dma_start(xt[:, :], x2_d[t * 128:(t + 1) * 128, :])
                            x2t.append(xt)
                            if half == 0:
                                # LN2 stats inline; rsqrt on DVE only
                                # (quake seed + 3 Newton steps, ~1.5e-7 rel)
                                st5 = pxn.tile([128, 3, 6], F32, name="st5")
                                for gg in range(3):
                                    nc.vector.bn_stats(
                                        st5[:, gg, :],
                                        xt[:, gg * 256:(gg + 1) * 256])
                                mv5 = pxn.tile([128, 8], F32, name="mv5")
                                nc.vector.bn_aggr(mv5[:, 0:2], st5[:, :, :])
                                nc.gpsimd.tensor_copy(stats2M[:, t:t + 1],
                                                      mv5[:, 0:1])
                                ve = mv5[:, 2:3]
                                nc.vector.tensor_scalar(
                                    out=ve, in0=mv5[:, 1:2], scalar1=EPS,
                                    scalar2=None, op0=OP.add)
                                yi = mv5[:, 3:4].bitcast(mybir.dt.int32)
                                nc.vector.tensor_scalar(
                                    out=yi, in0=ve.bitcast(mybir.dt.int32),
                                    scalar1=1, scalar2=None,
                                    op0=OP.arith_shift_right)
                                y0 = mv5[:, 4:5].bitcast(mybir.dt.int32)
                                nc.vector.tensor_scalar(
                                    out=y0, in0=yi, scalar1=-1,
                                    scalar2=None, op0=OP.bitwise_xor)
                                nc.vector.tensor_scalar(
                                    out=y0, in0=y0, scalar1=0x5f3759e0,
                                    scalar2=None, op0=OP.add)
                                ya = mv5[:, 4:5]
                                yb = mv5[:, 5:6]
                                t2 = mv5[:, 6:7]
                                w5 = mv5[:, 7:8]
                                for _ in range(3):
                                    nc.vector.tensor_tensor(
                                        out=t2, in0=ya, in1=ya, op=OP.mult)
                                    nc.vector.scalar_tensor_tensor(
                                        out=w5, in0=ve, scalar=-0.5, in1=t2,
                                        op0=OP.mult, op1=OP.mult)
                                    nc.vector.tensor_scalar(
                                        out=w5, in0=w5, scalar1=1.5,
                                        scalar2=None, op0=OP.add)
                                    nc.vector.tensor_tensor(
                                        out=yb, in0=ya, in1=w5, op=OP.mult)
                                    ya, yb = yb, ya
                                nc.gpsimd.tensor_copy(stats2R[:, t:t + 1], ya)
                            xn = pxn.tile([128, DIM], F32R, name="x2n")
                            nc.vector.tensor_scalar(
                                out=xn[:, :], in0=xt[:, :],
                                scalar1=stats2M[:, t:t + 1],
                                scalar2=stats2R[:, t:t + 1],
                                op0=OP.subtract, op1=OP.mult)
                            for k in range(6):
                                ps_t2 = pps5.tile([128, 384], F32R, tag="ps5",
                                                  name="ps_t2")
                                nc.tensor.transpose(
                                    ps_t2[:, :128], xn[:, k * 128:(k + 1) * 128],
                                    ident_r[:, :])
                                nc.vector.tensor_copy(
                                    x2nT[:, k, c * 128:(c + 1) * 128],
                                    ps_t2[:, :128])
                        return x2t, x2nT

                    def mlp_fc1(x2nT):
                        gT = pgT.tile([128, 12, 256], F32R, name="gT")
                        for m in range(12):
                            ps_f1 = pps5.tile([128, 384], F32, tag="ps5", name="ps_f1")
                            for k in range(6):
                                nc.tensor.matmul(
                                    ps_f1[:, :256],
                                    fc1w_sb[:, k, m * 128:(m + 1) * 128],
                                    x2nT[:, k, :],
                                    start=(k == 0), stop=(k == 5))
                            nc.scalar.activation(
                                gT[:, m, :], ps_f1[:, :256], AF.Gelu,
                                bias=fc1b_sb[:, half * 12 + m:half * 12 + m + 1])
                        return gT

                    def mlp_fc2(st, x2t, gT):
                        for c in range(2):
                            outt = pout.tile([128, DIM], F32, name="outt")
                            for n in range(2):
                                ps_f2 = pps5.tile([128, 384], F32, tag="ps5",
                                                  name="ps_f2")
                                for m in range(12):
                                    nc.tensor.matmul(
                                        ps_f2[:, :384],
                                        gT[:, m, c * 128:(c + 1) * 128],
                                        fc2w_sb[:, m, n * 384:(n + 1) * 384],
                                        start=(m == 0), stop=(m == 11 and half == 1))
                                if half == 0:
                                    nc.tensor.matmul(
                                        ps_f2[:, :384],
                                        ones_r[32:33, :128],
                                        bias2[32:33, n * 384:(n + 1) * 384],
                                        start=False, stop=True)
                                    nc.vector.tensor_tensor(
                                        out=outt[:, n * 384:(n + 1) * 384],
                                        in0=ps_f2[:, :384],
                                        in1=x2t[c][:, n * 384:(n + 1) * 384],
                                        op=OP.add)
                                else:
                                    nc.vector.tensor_copy(
                                        outt[:, n * 384:(n + 1) * 384],
                                        ps_f2[:, :384])
                            t = st * 2 + c
                            if half == 0:
                                nc.sync.dma_start(
                                    out_d[t * 128:(t + 1) * 128, :], outt[:, :])
                            else:
                                nc.gpsimd.dma_start(
                                    out_d[t * 128:(t + 1) * 128, :], outt[:, :],
                                    accum_op=OP.add)

                    # 2-stage pipeline over super-tiles: prep(st+1) is emitted
                    # between fc1(st) and fc2(st) so its PE transposes fill
                    # the gelu wait.
                    cur = mlp_prep(0)
                    for st in range(16):
                        gT = mlp_fc1(cur[1])
                        nxt = mlp_prep(st + 1) if st < 15 else None
                        mlp_fc2(st, cur[0], gT)
                        cur = nxt

    nc.compile()
    return nc


_NC_CACHE = {}


def _get_nc():
    if "nc" not in _NC_CACHE:
        _NC_CACHE["nc"] = build_program()
    return _NC_CACHE["nc"]


def _prep_weights(inputs):
    f = lambda k: np.asarray(inputs[k], np.float32)
    x = f("x")
    ln1_g, ln1_b = f("ln1_g"), f("ln1_b")
    ln2_g, ln2_b = f("ln2_g"), f("ln2_b")
    qkv_w, qkv_b = f("qkv_w"), f("qkv_b")
    proj_w, proj_b = f("proj_w"), f("proj_b")
    fc1_w, fc1_b = f("fc1_w"), f("fc1_b")
    fc2_w, fc2_b = f("fc2_w"), f("fc2_b")
    rel = f("rel_pos_bias")
    SCALE = np.float32((DIM // HEADS) ** -0.5)

    # fold ln1 into qkv
    qkv_w_f = qkv_w * ln1_g[None, :]
    qkv_b_f = qkv_w @ ln1_b + qkv_b
    # fold attention scale into q block
    qkv_w_f[:DIM] *= SCALE
    qkv_b_f2 = qkv_b_f.copy()
    qkv_b_f2[:DIM] *= SCALE
    # fold v bias into proj bias (softmax rows sum to 1)
    projb = proj_b + proj_w @ qkv_b_f[2 * DIM:]
    # fold ln2 into fc1
    fc1_w_f = fc1_w * ln2_g[None, :]
    fc1_b_f = fc1_w @ ln2_b + fc1_b

    expb = np.exp(rel).astype(np.float32)          # [12, 196, 196]
    expb = expb.reshape(HEADS, 2, HC, WW).transpose(2, 0, 1, 3).copy()

    return {
        "qkvw": np.ascontiguousarray(qkv_w_f.T),    # [768, 2304]
        "qkvb": qkv_b_f2,
        "projw": np.ascontiguousarray(proj_w.T),    # [768, 768]
        "projb": projb,
        "expb": expb,
        "fc1w": np.ascontiguousarray(fc1_w_f.T),    # [768, 3072]
        "fc1b": fc1_b_f,
        "fc2w": np.ascontiguousarray(fc2_w.T),      # [3072, 768]
        "fc2b": fc2_b,
    }, x


PROFILE = False
LAST_RESULT = None


def kernel(**inputs):
    global LAST_RESULT
    weights, x = _prep_weights(inputs)
    nc = _get_nc()
    in_maps = [dict(weights, x=np.ascontiguousarray(x[i])) for i in range(B)]
    res = run_bass_kernel_spmd(nc, in_maps, core_ids=list(range(B)),
                               trace=PROFILE)
    LAST_RESULT = res
    out = np.stack([res.results[i]["out"] for i in range(B)], axis=0)
    return out.astype(np.float32)


if __name__ == "__main__":
    rng = np.random.default_rng(0)
    ins = {
        "x": rng.standard_normal((B, NTOK, DIM), dtype=np.float32),
        "rel_pos_bias": (rng.standard_normal((HEADS, WW, WW)) * 0.1).astype(np.float32),
        "ln1_g": np.ones(DIM, np.float32), "ln1_b": np.zeros(DIM, np.float32),
        "qkv_w": (rng.standard_normal((3 * DIM, DIM)) * 0.02).astype(np.float32),
        "qkv_b": np.zeros(3 * DIM, np.float32),
        "proj_w": (rng.standard_normal((DIM, DIM)) * 0.02).astype(np.float32),
        "proj_b": np.zeros(DIM, np.float32),
        "ln2_g": np.ones(DIM, np.float32), "ln2_b": np.zeros(DIM, np.float32),
        "fc1_w": (rng.standard_normal((MLP_H, DIM)) * 0.02).astype(np.float32),
        "fc1_b": np.zeros(MLP_H, np.float32),
        "fc2_w": (rng.standard_normal((DIM, MLP_H)) * 0.02).astype(np.float32),
        "fc2_b": np.zeros(DIM, np.float32),
        "H": np.int64(64), "W": np.int64(64),
    }
    out = kernel(**ins)
    print("out", out.shape, out.dtype, np.abs(out).max())


# revision 23
# speedup vs baseline: 1.9994x; 1.2038x over previous
"""Swin-style transformer block on 8 Trainium2 NeuronCores.

Sharding: data-parallel over batch — each of the 8 cores processes one image
([4096, 768] tokens). All weights replicated per core. No collectives.

Per-core pipeline:
  P1: LN1 statistics over the 50 window-half-chunks (98 tokens each)
  P2: 13 window-pairs: LN1-apply -> PE transpose -> fused QKV (fp32r, N=392)
      -> per-head scores (fp32r window-paired) -> softmax via exp(S)*exp(B)
      with deferred 1/sum -> transpose P -> O^T -> proj + residual -> x2 DRAM
  P4: LN2 statistics (32 token tiles)
  P5: MLP in two hidden-halves: fc1 (feature-major out) + erf-GELU -> fc2 +
      residual; second half accumulated into the output via gpsimd accum-DMA.

Host-side folds (all exact for the given inputs; SCALE=0.125 is binary-exact):
  - ln1 gamma/beta folded into qkv weights/bias; attention SCALE folded into
    the q block of qkv; v-bias folded into the proj bias (softmax rows sum
    to 1); ln2 gamma/beta folded into fc1; rel_pos_bias shipped as
    exp(rel_pos_bias) so softmax(S+B) = exp(S)*expB / sum(...).
"""

import numpy as np
from contextlib import ExitStack

import concourse.bass as bass
import concourse.mybir as mybir
import concourse.tile as tile
from concourse import bacc
from concourse.bass_utils import run_bass_kernel_spmd
from concourse.masks import make_identity

F32 = mybir.dt.float32
F32R = mybir.dt.float32r
AF = mybir.ActivationFunctionType
OP = mybir.AluOpType

DIM, HEADS, WIN, MLP_H = 768, 12, 14, 3072
B, H0, W0 = 8, 64, 64
NTOK = H0 * W0
NW = 5            # windows per image axis (70/14)
NWIN = NW * NW    # 25 windows
WW = WIN * WIN    # 196 tokens per window
HC = 98           # half-window chunk (7 rows x 14 cols)
DH = DIM // HEADS # 64
EPS = 1e-5

# window pairing: 12 pairs + 1 single
PAIRS = [(2 * i, 2 * i + 1) for i in range(12)] + [(24,)]


def _chunk_geom(w, c):
    """Valid-row/col geometry of half-chunk c (0/1) of window w."""
    wi, wj = divmod(w, NW)
    r0 = wi * WIN + c * 7          # first padded-image row of this chunk
    c0 = wj * WIN
    vr = 7 if (wi < 4 or c == 0) else 1   # wi==4 -> rows 56..63 valid (8)
    vc = 14 if wj < 4 else 8
    return r0, c0, vr, vc


def _gather_chunk(nc, dst, dram, w, c, eng=None):
    """DMA image tokens of half-chunk (w, c) from [4096,768] DRAM into
    dst [98, 768] SBUF tile (partition p = 14*row + col). Pads with zeros."""
    eng = eng or nc.sync
    r0, c0, vr, vc = _chunk_geom(w, c)
    if vr < 7 or vc < 14:
        nc.gpsimd.memset(dst[:, :], 0.0)
    if vc == 14:
        src = bass.AP(tensor=dram, offset=(r0 * W0 + c0) * DIM,
                      ap=[[W0 * DIM, vr], [DIM, 14], [1, DIM]])
        eng.dma_start(dst[0:vr * 14, :], src)
    else:
        for r in range(vr):
            src = bass.AP(tensor=dram, offset=((r0 + r) * W0 + c0) * DIM,
                          ap=[[DIM, vc], [1, DIM]])
            eng.dma_start(dst[r * 14:r * 14 + vc, :], src)


def _scatter_chunk(nc, dram, src, w, c, eng=None):
    """DMA the valid tokens of half-chunk (w, c) from src [98,768] SBUF back
    to token-major [4096,768] DRAM."""
    eng = eng or nc.sync
    r0, c0, vr, vc = _chunk_geom(w, c)
    if vc == 14:
        dst = bass.AP(tensor=dram, offset=(r0 * W0 + c0) * DIM,
                      ap=[[W0 * DIM, vr], [DIM, 14], [1, DIM]])
        eng.dma_start(dst, src[0:vr * 14, :])
    else:
        for r in range(vr):
            dst = bass.AP(tensor=dram, offset=((r0 + r) * W0 + c0) * DIM,
                          ap=[[DIM, vc], [1, DIM]])
            eng.dma_start(dst, src[r * 14:r * 14 + vc, :])


def _ln_stats(nc, pool, xt, n_part, statsM, statsR, col, eps_t):
    """bn_stats/bn_aggr over 768 features -> mean/rstd columns."""
    st = pool.tile([128, 3, 6], F32, name="bnstats")
    for g in range(3):
        nc.vector.bn_stats(st[:n_part, g, :], xt[:n_part, g * 256:(g + 1) * 256])
    mv = pool.tile([128, 2], F32, name="bnaggr")
    nc.vector.bn_aggr(mv[:n_part, :], st[:n_part, :, :])
    nc.gpsimd.tensor_copy(statsM[:n_part, col:col + 1], mv[:n_part, 0:1])
    std = pool.tile([128, 1], F32, name="std")
    nc.scalar.activation(std[:n_part, :], mv[:n_part, 1:2], AF.Sqrt,
                         bias=eps_t[:n_part, :], scale=1.0)
    nc.vector.reciprocal(statsR[:n_part, col:col + 1], std[:n_part, :])


def build_program():
    nc = bacc.Bacc(None, target_bir_lowering=False, debug=False)

    x_d = nc.dram_tensor("x", [NTOK, DIM], F32, kind="ExternalInput")
    qkvw_d = nc.dram_tensor("qkvw", [DIM, 3 * DIM], F32R, kind="ExternalInput")
    qkvb_d = nc.dram_tensor("qkvb", [3 * DIM], F32, kind="ExternalInput")
    projw_d = nc.dram_tensor("projw", [DIM, DIM], F32R, kind="ExternalInput")
    projb_d = nc.dram_tensor("projb", [DIM], F32R, kind="ExternalInput")
    expb_d = nc.dram_tensor("expb", [HC, HEADS, 2, WW], F32, kind="ExternalInput")
    fc1w_d = nc.dram_tensor("fc1w", [DIM, MLP_H], F32R, kind="ExternalInput")
    fc1b_d = nc.dram_tensor("fc1b", [MLP_H], F32, kind="ExternalInput")
    fc2w_d = nc.dram_tensor("fc2w", [MLP_H, DIM], F32R, kind="ExternalInput")
    fc2b_d = nc.dram_tensor("fc2b", [DIM], F32R, kind="ExternalInput")

    out_d = nc.dram_tensor("out", [NTOK, DIM], F32, kind="ExternalOutput")
    x2_d = nc.dram_tensor("x2", [NTOK, DIM], F32)  # internal scratch

    with tile.TileContext(nc) as tc:
        with ExitStack() as g:
            # ---------------- global constants / stats ----------------
            consts = g.enter_context(tc.tile_pool(name="consts", bufs=1))
            ident32 = consts.tile([128, 128], F32)
            make_identity(nc, ident32)
            ident_r = consts.tile([128, 128], F32R)
            nc.vector.tensor_copy(ident_r[:, :], ident32[:, :])
            ones32 = consts.tile([33, 128], F32)
            nc.vector.memset(ones32[:, :], 1.0)
            ones_r = consts.tile([33, 128], F32R)
            nc.vector.tensor_copy(ones_r[:, :], ones32[:, :])
            eps_t = consts.tile([128, 1], F32)
            nc.vector.memset(eps_t[:, :], EPS)
            qkvb_sb = consts.tile([128, 12], F32)
            nc.sync.dma_start(
                qkvb_sb[:, :],
                bass.AP(tensor=qkvb_d, offset=0, ap=[[1, 128], [128, 12]]))
            bias2 = consts.tile([33, DIM], F32R)
            nc.sync.dma_start(bias2[0:1, :],
                              bass.AP(tensor=projb_d, offset=0, ap=[[1, DIM]]))
            nc.sync.dma_start(bias2[32:33, :],
                              bass.AP(tensor=fc2b_d, offset=0, ap=[[1, DIM]]))
            fc1b_sb = consts.tile([128, 24], F32)
            nc.sync.dma_start(
                fc1b_sb[:, :],
                bass.AP(tensor=fc1b_d, offset=0, ap=[[1, 128], [128, 24]]))
            stats2M = consts.tile([128, 32], F32)
            stats2R = consts.tile([128, 32], F32)

            # ---------------- P2: attention over window pairs ----------------
            with ExitStack() as s2:
                wA = s2.enter_context(tc.tile_pool(name="wA", bufs=1))
                qkvw_sb = wA.tile([128, 6, 3 * DIM], F32R)
                for kk in range(6):
                    for hh in range(2):
                        eng = (nc.sync, nc.scalar, nc.gpsimd)[(2 * kk + hh) % 3]
                        eng.dma_start(
                            qkvw_sb[:, kk, hh * 1152:(hh + 1) * 1152],
                            qkvw_d[kk * 128:(kk + 1) * 128,
                                   hh * 1152:(hh + 1) * 1152])
                projw_sb = wA.tile([128, 6, DIM], F32R)
                for kk in range(3):
                    eng = (nc.sync, nc.scalar, nc.gpsimd)[kk]
                    nc.sync.dma_start(
                        projw_sb[:, 2 * kk:2 * kk + 2, :],
                        projw_d[kk * 256:(kk + 1) * 256, :]
                        .rearrange("(a p) n -> p a n", p=128))
                expb_sb = wA.tile([HC, HEADS, 2, WW], F32)
                for hh in range(3):
                    eng = (nc.sync, nc.scalar, nc.gpsimd)[hh]
                    eng.dma_start(expb_sb[:, 4 * hh:4 * (hh + 1), :, :],
                                  expb_d[:, 4 * hh:4 * (hh + 1), :, :])

                # natural_log_exp_and_others: exp (softmax) + ln/exp (rstd)
                nc.scalar.add_instruction(mybir.InstLoadActFuncSet(
                    name=nc.get_next_instruction_name(), ins=[], outs=[],
                    act_func_set_id=6))

                pxp = s2.enter_context(tc.tile_pool(name="pxp", bufs=2))
                pln = s2.enter_context(tc.tile_pool(name="pln", bufs=1))
                phT = s2.enter_context(tc.tile_pool(name="phT", bufs=1))
                pqk = s2.enter_context(tc.tile_pool(name="pqk", bufs=1))
                pvT = s2.enter_context(tc.tile_pool(name="pvT", bufs=1))
                psm = s2.enter_context(tc.tile_pool(name="psm", bufs=3))
                ppn = s2.enter_context(tc.tile_pool(name="ppn", bufs=2))
                pUT = s2.enter_context(tc.tile_pool(name="pUT", bufs=1))
                pOT = s2.enter_context(tc.tile_pool(name="pOT", bufs=1))
                px2 = s2.enter_context(tc.tile_pool(name="px2", bufs=3))
                pps = s2.enter_context(tc.tile_pool(name="pps", bufs=8, space="PSUM"))

                for pair in PAIRS:
                    nw = len(pair)
                    PW = nw * WW          # 392 or 196
                    nch = 2 * nw          # chunks in pair

                    x_pair = pxp.tile([HC, 4, DIM], F32, name="x_pair")
                    ln_pair = pln.tile([HC, 4, DIM], F32R, name="ln_pair")
                    for wl, w in enumerate(pair):
                        for c in range(2):
                            i = 2 * wl + c
                            _gather_chunk(nc, x_pair[:, i, :], x_d, w, c,
                                          eng=nc.gpsimd)
                            # LN1 stats inline: rstd = exp(-0.5*ln(var+eps))
                            st = psm.tile([HC, 3, 6], F32, name="bst")
                            for gg in range(3):
                                nc.vector.bn_stats(
                                    st[:, gg, :],
                                    x_pair[:, i, gg * 256:(gg + 1) * 256])
                            mv = psm.tile([HC, 2], F32, name="bmv")
                            nc.vector.bn_aggr(mv[:, :], st[:, :, :])
                            rstd = psm.tile([HC, 2], F32, name="rstd")
                            nc.scalar.activation(rstd[:, 0:1], mv[:, 1:2],
                                                 AF.Ln, bias=eps_t[:HC, :])
                            nc.scalar.activation(rstd[:, 1:2], rstd[:, 0:1],
                                                 AF.Exp, scale=-0.5)
                            nc.vector.tensor_scalar(
                                out=ln_pair[:, i, :], in0=x_pair[:, i, :],
                                scalar1=mv[:, 0:1],
                                scalar2=rstd[:, 1:2],
                                op0=OP.subtract, op1=OP.mult)

                    # transpose ln -> hT [128, 6, PW] (feature-major)
                    hT = phT.tile([128, 6, 2 * WW], F32R, name="hT")
                    for k in range(6):
                        ps_t = pps.tile([128, 392], F32R, tag="ps", name="ps_t")
                        for i in range(nch):
                            nc.tensor.transpose(
                                ps_t[:, i * HC:(i + 1) * HC],
                                ln_pair[:HC, i, k * 128:(k + 1) * 128],
                                ident_r[:HC, :HC])
                        nc.vector.tensor_copy(hT[:, k, :PW], ps_t[:, :PW])

                    # fused qk^T: [1536 feats, PW] (q pre-scaled on host)
                    qkT = pqk.tile([128, 12, 2 * WW], F32R, name="qkT")
                    for m in range(12):
                        ps_qk = pps.tile([128, 392], F32, tag="ps", name="ps_qk")
                        for k in range(6):
                            nc.tensor.matmul(
                                ps_qk[:, :PW],
                                qkvw_sb[:, k, m * 128:(m + 1) * 128],
                                hT[:, k, :PW],
                                start=(k == 0), stop=(k == 5))
                        nc.scalar.activation(qkT[:, m, :PW], ps_qk[:, :PW],
                                             AF.Identity, bias=qkvb_sb[:, m:m + 1])

                    # v token-major: [PW, 768]
                    vT = pvT.tile([HC, 4, DIM], F32R, name="vT")
                    for i in range(nch):
                        for n in range(2):
                            ps_v = pps.tile([128, 392], F32, tag="ps", name="ps_v")
                            for k in range(6):
                                nc.tensor.matmul(
                                    ps_v[:HC, :384],
                                    hT[:, k, i * HC:(i + 1) * HC],
                                    qkvw_sb[:, k, 2 * DIM + n * 384:2 * DIM + (n + 1) * 384],
                                    start=(k == 0), stop=(k == 5))
                            nc.vector.tensor_copy(vT[:, i, n * 384:(n + 1) * 384],
                                                  ps_v[:HC, :384])

                    OT = pOT.tile([128, 6, 2 * WW], F32R, name="OT")

                    def head_stage_a(h):
                        """scores + softmax -> normalized Pn for head h."""
                        qrow = (h % 2) * 64
                        qm = h // 2
                        km = 6 + h // 2
                        Pn = ppn.tile([HC, 4, WW], F32, name="Pn")
                        den = ppn.tile([HC, 4], F32, name="den")
                        rden = ppn.tile([HC, 4], F32, name="rden")
                        for qc in range(nch):
                            wl = qc // 2
                            ps_S = pps.tile([128, 392], F32, tag="ps", name="ps_S")
                            nc.tensor.matmul(
                                ps_S[:HC, :PW],
                                qkT[qrow:qrow + 64, qm, qc * HC:(qc + 1) * HC],
                                qkT[qrow:qrow + 64, km, :PW],
                                start=True, stop=True)
                            U = psm.tile([HC, 2 * WW], F32, name="U")
                            nc.scalar.activation(U[:, :PW], ps_S[:HC, :PW], AF.Exp)
                            P = psm.tile([HC, WW], F32, name="P")
                            nc.vector.scalar_tensor_tensor(
                                out=P[:, :], in0=U[:, wl * WW:(wl + 1) * WW],
                                scalar=1.0, in1=expb_sb[:, h, qc % 2, :],
                                op0=OP.mult, op1=OP.mult,
                                accum_out=den[:, qc:qc + 1])
                            nc.vector.reciprocal(rden[:, qc:qc + 1],
                                                 den[:, qc:qc + 1])
                            nc.vector.tensor_scalar_mul(
                                Pn[:, qc, :], P[:, :], rden[:, qc:qc + 1])
                        return Pn

                    def head_stage_b(h, Pn):
                        """transpose Pn -> UT, then O^T = V^T @ U^T."""
                        qrow = (h % 2) * 64
                        qm = h // 2
                        UT = pUT.tile([HC, 2, 2 * WW], F32R, name="UT")
                        ps_PT = [pps.tile([128, 392], F32, tag="ps",
                                          name="ps_PT") for _ in range(2)]
                        for qc in range(nch):     # qc-major: unblock early
                            for kc in range(2):
                                nc.tensor.transpose(
                                    ps_PT[kc][:HC, qc * HC:(qc + 1) * HC],
                                    Pn[:HC, qc, kc * HC:(kc + 1) * HC],
                                    ident32[:HC, :HC])
                        for kc in range(2):
                            nc.scalar.copy(UT[:, kc, :PW], ps_PT[kc][:HC, :PW])
                        for wl in range(nw):
                            ps_O = pps.tile([128, 392], F32, tag="ps", name="ps_O")
                            for kc in range(2):
                                nc.tensor.matmul(
                                    ps_O[:64, :PW],
                                    vT[:, 2 * wl + kc, h * 64:(h + 1) * 64],
                                    UT[:, kc, :PW],
                                    start=(kc == 0), stop=(kc == 1))
                            nc.vector.tensor_copy(
                                OT[qrow:qrow + 64, qm, wl * WW:(wl + 1) * WW],
                                ps_O[:64, wl * WW:(wl + 1) * WW])

                    # 2-stage pipeline over heads: S(h+1) issues before PT/O(h)
                    prev = None
                    for h in range(HEADS):
                        Pn = head_stage_a(h)
                        if prev is not None:
                            head_stage_b(h - 1, prev)
                        prev = Pn
                    head_stage_b(HEADS - 1, prev)

                    # proj + residual -> x2 (per chunk, scatter immediately)
                    for i in range(nch):
                        x2c = px2.tile([HC, DIM], F32, name="x2c")
                        for n in range(2):
                            ps_pr = pps.tile([128, 392], F32, tag="ps", name="ps_pr")
                            for k in range(6):
                                nc.tensor.matmul(
                                    ps_pr[:HC, :384],
                                    OT[:, k, i * HC:(i + 1) * HC],
                                    projw_sb[:, k, n * 384:(n + 1) * 384],
                                    start=(k == 0), stop=False)
                            nc.tensor.matmul(
                                ps_pr[:HC, :384],
                                ones_r[0:1, :HC],
                                bias2[0:1, n * 384:(n + 1) * 384],
                                start=False, stop=True)
                            nc.vector.tensor_tensor(
                                out=x2c[:, n * 384:(n + 1) * 384],
                                in0=ps_pr[:HC, :384],
                                in1=x_pair[:, i, n * 384:(n + 1) * 384],
                                op=OP.add)
                        _scatter_chunk(nc, x2_d, x2c[:, :], pair[i // 2], i % 2,
                                       eng=nc.sync)

            # ---------------- P4 + P5 (MLP) ----------------
            with ExitStack() as s5:
                # weight pool first so the half-0 prefetch DMAs overlap the
                # LN2-stats pass (no address reuse WAR on P4's pools)
                wB = s5.enter_context(tc.tile_pool(name="wB", bufs=1))

                def load_fc_weights(half):
                    engs = (nc.gpsimd, nc.sync, nc.scalar)
                    fc1w_sb = wB.tile([128, 6, MLP_H // 2], F32R, name="fc1w_sb")
                    for kk in range(6):
                        engs[kk % 3].dma_start(
                            fc1w_sb[:, kk, :],
                            fc1w_d[kk * 128:(kk + 1) * 128,
                                   half * 1536:(half + 1) * 1536])
                    fc2w_sb = wB.tile([128, 12, DIM], F32R, name="fc2w_sb")
                    for kk in range(4):
                        engs[kk % 3].dma_start(
                            fc2w_sb[:, 3 * kk:3 * (kk + 1), :],
                            fc2w_d[half * 1536 + kk * 384:
                                   half * 1536 + (kk + 1) * 384, :]
                            .rearrange("(a p) n -> p a n", p=128))
                    return fc1w_sb, fc2w_sb

                w_half0 = load_fc_weights(0)

                pxt = s5.enter_context(tc.tile_pool(name="pxt", bufs=4))
                pxn = s5.enter_context(tc.tile_pool(name="pxn", bufs=2))
                pxnT = s5.enter_context(tc.tile_pool(name="pxnT", bufs=2))
                pgT = s5.enter_context(tc.tile_pool(name="pgT", bufs=2))
                pout = s5.enter_context(tc.tile_pool(name="pout", bufs=4))
                pps5 = s5.enter_context(tc.tile_pool(name="pps5", bufs=8, space="PSUM"))


                for half in range(2):
                    fc1w_sb, fc2w_sb = w_half0 if half == 0 else load_fc_weights(1)

                    def mlp_prep(st):
                        """load x2 tiles, (half0: LN2 stats), apply, transpose."""
                        x2t = []
                        x2nT = pxnT.tile([128, 6, 256], F32R, name="x2nT")
                        for c in range(2):
                            t = st * 2 + c
                            xt = pxt.tile([128, DIM], F32, name="x2t")
                            eng = nc.scalar if t % 2 == 0 else nc.sync
                            eng.dma_start(xt[:, :], x2_d[t * 128:(t + 1) * 128, :])
                            x2t.append(xt)
                            if half == 0:
                                # LN2 stats inline; rsqrt on DVE only
                                # (quake seed + 3 Newton steps, ~1.5e-7 rel)
                                st5 = pxn.tile([128, 3, 6], F32, name="st5")
                                for gg in range(3):
                                    nc.vector.bn_stats(
                                        st5[:, gg, :],
                                        xt[:, gg * 256:(gg + 1) * 256])
                                mv5 = pxn.tile([128, 8], F32, name="mv5")
                                nc.vector.bn_aggr(mv5[:, 0:2], st5[:, :, :])
                                nc.gpsimd.tensor_copy(stats2M[:, t:t + 1],
                                                      mv5[:, 0:1])
                                ve = mv5[:, 2:3]
                                nc.vector.tensor_scalar(
                                    out=ve, in0=mv5[:, 1:2], scalar1=EPS,
                                    scalar2=None, op0=OP.add)
                                yi = mv5[:, 3:4].bitcast(mybir.dt.int32)
                                nc.vector.tensor_scalar(
                                    out=yi, in0=ve.bitcast(mybir.dt.int32),
                                    scalar1=1, scalar2=None,
                                    op0=OP.arith_shift_right)
                                y0 = mv5[:, 4:5].bitcast(mybir.dt.int32)
                                nc.vector.tensor_scalar(
                                    out=y0, in0=yi, scalar1=-1,
                                    scalar2=None, op0=OP.bitwise_xor)
                                nc.vector.tensor_scalar(
                                    out=y0, in0=y0, scalar1=0x5f3759e0,
                                    scalar2=None, op0=OP.add)
                                ya = mv5[:, 4:5]
                                yb = mv5[:, 5:6]
                                t2 = mv5[:, 6:7]
                                w5 = mv5[:, 7:8]
                                for _ in range(3):
                                    nc.vector.tensor_tensor(
                                        out=t2, in0=ya, in1=ya, op=OP.mult)
                                    nc.vector.scalar_tensor_tensor(
                                        out=w5, in0=ve, scalar=-0.5, in1=t2,
                                        op0=OP.mult, op1=OP.mult)
                                    nc.vector.tensor_scalar(
                                        out=w5, in0=w5, scalar1=1.5,
                                        scalar2=None, op0=OP.add)
                                    nc.vector.tensor_tensor(
                                        out=yb, in0=ya, in1=w5, op=OP.mult)
                                    ya, yb = yb, ya
                                nc.gpsimd.tensor_copy(stats2R[:, t:t + 1], ya)
                            xn = pxn.tile([128, DIM], F32R, name="x2n")
                            nc.vector.tensor_scalar(
                                out=xn[:, :], in0=xt[:, :],
                                scalar1=stats2M[:, t:t + 1],
                                scalar2=stats2R[:, t:t + 1],
                                op0=OP.subtract, op1=OP.mult)
                            for k in range(6):
                                ps_t2 = pps5.tile([128, 384], F32R, tag="ps5",
                                                  name="ps_t2")
                                nc.tensor.transpose(
                                    ps_t2[:, :128], xn[:, k * 128:(k + 1) * 128],
                                    ident_r[:, :])
                                nc.vector.tensor_copy(
                                    x2nT[:, k, c * 128:(c + 1) * 128],
                                    ps_t2[:, :128])
                        return x2t, x2nT

                    def mlp_fc1(x2nT):
                        gT = pgT.tile([128, 12, 256], F32R, name="gT")
                        for m in range(12):
                            ps_f1 = pps5.tile([128, 384], F32, tag="ps5", name="ps_f1")
                            for k in range(6):
                                nc.tensor.matmul(
                                    ps_f1[:, :256],
                                    fc1w_sb[:, k, m * 128:(m + 1) * 128],
                                    x2nT[:, k, :],
                                    start=(k == 0), stop=(k == 5))
                            nc.scalar.activation(
                                gT[:, m, :], ps_f1[:, :256], AF.Gelu,
                                bias=fc1b_sb[:, half * 12 + m:half * 12 + m + 1])
                        return gT

                    def mlp_fc2(st, x2t, gT):
                        for c in range(2):
                            outt = pout.tile([128, DIM], F32, name="outt")
                            for n in range(2):
                                ps_f2 = pps5.tile([128, 384], F32, tag="ps5",
                                                  name="ps_f2")
                                for m in range(12):
                                    nc.tensor.matmul(
                                        ps_f2[:, :384],
                                        gT[:, m, c * 128:(c + 1) * 128],
                                        fc2w_sb[:, m, n * 384:(n + 1) * 384],
                                        start=(m == 0), stop=(m == 11 and half == 1))
                                if half == 0:
                                    nc.tensor.matmul(
                                        ps_f2[:, :384],
                                        ones_r[32:33, :128],
                                        bias2[32:33, n * 384:(n + 1) * 384],
                                        start=False, stop=True)
                                    nc.vector.tensor_tensor(
                                        out=outt[:, n * 384:(n + 1) * 384],
                                        in0=ps_f2[:, :384],
                                        in1=x2t[c][:, n * 384:(n + 1) * 384],
                                        op=OP.add)
                                else:
                                    nc.vector.tensor_copy(
                                        outt[:, n * 384:(n + 1) * 384],
                                        ps_f2[:, :384])
                            t = st * 2 + c
                            if half == 0:
                                nc.sync.dma_start(
                                    out_d[t * 128:(t + 1) * 128, :], outt[:, :])
                            else:
                                nc.gpsimd.dma_start(
                                    out_d[t * 128:(t + 1) * 128, :], outt[:, :],
                                    accum_op=OP.add)

                    # 2-stage pipeline over super-tiles: prep(st+1) is emitted
                    # between fc1(st) and fc2(st) so its PE transposes fill
                    # the gelu wait.
                    cur = mlp_prep(0)
                    for st in range(16):
                        gT = mlp_fc1(cur[1])
                        nxt = mlp_prep(st + 1) if st < 15 else None
                        mlp_fc2(st, cur[0], gT)
                        cur = nxt

    nc.compile()
    return nc


_NC_CACHE = {}


def _get_nc():
    if "nc" not in _NC_CACHE:
        _NC_CACHE["nc"] = build_program()
    return _NC_CACHE["nc"]


def _prep_weights(inputs):
    f = lambda k: np.asarray(inputs[k], np.float32)
    x = f("x")
    ln1_g, ln1_b = f("ln1_g"), f("ln1_b")
    ln2_g, ln2_b = f("ln2_g"), f("ln2_b")
    qkv_w, qkv_b = f("qkv_w"), f("qkv_b")
    proj_w, proj_b = f("proj_w"), f("proj_b")
    fc1_w, fc1_b = f("fc1_w"), f("fc1_b")
    fc2_w, fc2_b = f("fc2_w"), f("fc2_b")
    rel = f("rel_pos_bias")
    SCALE = np.float32((DIM // HEADS) ** -0.5)

    # fold ln1 into qkv
    qkv_w_f = qkv_w * ln1_g[None, :]
    qkv_b_f = qkv_w @ ln1_b + qkv_b
    # fold attention scale into q block
    qkv_w_f[:DIM] *= SCALE
    qkv_b_f2 = qkv_b_f.copy()
    qkv_b_f2[:DIM] *= SCALE
    # fold v bias into proj bias (softmax rows sum to 1)
    projb = proj_b + proj_w @ qkv_b_f[2 * DIM:]
    # fold ln2 into fc1
    fc1_w_f = fc1_w * ln2_g[None, :]
    fc1_b_f = fc1_w @ ln2_b + fc1_b

    expb = np.exp(rel).astype(np.float32)          # [12, 196, 196]
    expb = expb.reshape(HEADS, 2, HC, WW).transpose(2, 0, 1, 3).copy()

    return {
        "qkvw": np.ascontiguousarray(qkv_w_f.T),    # [768, 2304]
        "qkvb": qkv_b_f2,
        "projw": np.ascontiguousarray(proj_w.T),    # [768, 768]
        "projb": projb,
        "expb": expb,
        "fc1w": np.ascontiguousarray(fc1_w_f.T),    # [768, 3072]
        "fc1b": fc1_b_f,
        "fc2w": np.ascontiguousarray(fc2_w.T),      # [3072, 768]
        "fc2b": fc2_b,
    }, x


PROFILE = False
LAST_RESULT = None


def kernel(**inputs):
    global LAST_RESULT
    weights, x = _prep_weights(inputs)
    nc = _get_nc()
    in_maps = [dict(weights, x=np.ascontiguousarray(x[i])) for i in range(B)]
    res = run_bass_kernel_spmd(nc, in_maps, core_ids=list(range(B)),
                               trace=PROFILE)
    LAST_RESULT = res
    out = np.stack([res.results[i]["out"] for i in range(B)], axis=0)
    return out.astype(np.float32)


if __name__ == "__main__":
    rng = np.random.default_rng(0)
    ins = {
        "x": rng.standard_normal((B, NTOK, DIM), dtype=np.float32),
        "rel_pos_bias": (rng.standard_normal((HEADS, WW, WW)) * 0.1).astype(np.float32),
        "ln1_g": np.ones(DIM, np.float32), "ln1_b": np.zeros(DIM, np.float32),
        "qkv_w": (rng.standard_normal((3 * DIM, DIM)) * 0.02).astype(np.float32),
        "qkv_b": np.zeros(3 * DIM, np.float32),
        "proj_w": (rng.standard_normal((DIM, DIM)) * 0.02).astype(np.float32),
        "proj_b": np.zeros(DIM, np.float32),
        "ln2_g": np.ones(DIM, np.float32), "ln2_b": np.zeros(DIM, np.float32),
        "fc1_w": (rng.standard_normal((MLP_H, DIM)) * 0.02).astype(np.float32),
        "fc1_b": np.zeros(MLP_H, np.float32),
        "fc2_w": (rng.standard_normal((DIM, MLP_H)) * 0.02).astype(np.float32),
        "fc2_b": np.zeros(DIM, np.float32),
        "H": np.int64(64), "W": np.int64(64),
    }
    out = kernel(**ins)
    print("out", out.shape, out.dtype, np.abs(out).max())


# revision 24
# speedup vs baseline: 2.0646x; 1.0326x over previous
"""Swin-style transformer block on 8 Trainium2 NeuronCores.

Sharding: data-parallel over batch — each of the 8 cores processes one image
([4096, 768] tokens). All weights replicated per core. No collectives.

Per-core pipeline:
  Attention, 13 window-pairs (pairing makes the matmul moving dim 392 >= 256
  so fp32r runs at full PE rate): gather + inline LN1 stats (rstd via
  ln/exp, same ACT table set as softmax) -> LN1-apply -> PE transpose ->
  fused QKV -> per-head scores -> softmax exp(S)*exp(B) with deferred 1/sum
  -> transpose P -> O^T = V^T P^T -> proj + residual -> scatter x2 to DRAM.
  Heads run in a 2-stage software pipeline (scores of head h+1 issue before
  transpose/O of head h).
  MLP in two hidden-halves of 1536 (weights fit SBUF): per 256-token
  super-tile: LN2 (stats inline on half 0; rsqrt = quake seed + 3 Newton
  steps, DVE-only, no ACT table switch) -> transpose -> fc1 (feature-major
  out) + erf-GELU -> fc2 + residual; half 1 accumulates into the output via
  gpsimd accum-DMA. Super-tiles are software-pipelined.

Host-side folds (all exact for the given inputs; SCALE=0.125 is binary-exact):
  - ln1 gamma/beta folded into qkv weights/bias; attention SCALE folded into
    the q block of qkv; v-bias folded into the proj bias (softmax rows sum
    to 1); ln2 gamma/beta folded into fc1; rel_pos_bias shipped as
    exp(rel_pos_bias) so softmax(S+B) = exp(S)*expB / sum(...).
"""

import numpy as np
from contextlib import ExitStack

import concourse.bass as bass
import concourse.mybir as mybir
import concourse.tile as tile
from concourse import bacc
from concourse.bass_utils import run_bass_kernel_spmd
from concourse.masks import make_identity

F32 = mybir.dt.float32
F32R = mybir.dt.float32r
AF = mybir.ActivationFunctionType
OP = mybir.AluOpType

DIM, HEADS, WIN, MLP_H = 768, 12, 14, 3072
B, H0, W0 = 8, 64, 64
NTOK = H0 * W0
NW = 5            # windows per image axis (70/14)
NWIN = NW * NW    # 25 windows
WW = WIN * WIN    # 196 tokens per window
HC = 98           # half-window chunk (7 rows x 14 cols)
DH = DIM // HEADS # 64
EPS = 1e-5

# window pairing: 12 pairs + 1 single
PAIRS = [(2 * i, 2 * i + 1) for i in range(12)] + [(24,)]


def _chunk_geom(w, c):
    """Valid-row/col geometry of half-chunk c (0/1) of window w."""
    wi, wj = divmod(w, NW)
    r0 = wi * WIN + c * 7          # first padded-image row of this chunk
    c0 = wj * WIN
    vr = 7 if (wi < 4 or c == 0) else 1   # wi==4 -> rows 56..63 valid (8)
    vc = 14 if wj < 4 else 8
    return r0, c0, vr, vc


def _gather_chunk(nc, dst, dram, w, c, eng=None):
    """DMA image tokens of half-chunk (w, c) from [4096,768] DRAM into
    dst [98, 768] SBUF tile (partition p = 14*row + col). Pads with zeros."""
    eng = eng or nc.sync
    r0, c0, vr, vc = _chunk_geom(w, c)
    if vr < 7 or vc < 14:
        nc.gpsimd.memset(dst[:, :], 0.0)
    if vc == 14:
        src = bass.AP(tensor=dram, offset=(r0 * W0 + c0) * DIM,
                      ap=[[W0 * DIM, vr], [DIM, 14], [1, DIM]])
        eng.dma_start(dst[0:vr * 14, :], src)
    else:
        for r in range(vr):
            src = bass.AP(tensor=dram, offset=((r0 + r) * W0 + c0) * DIM,
                          ap=[[DIM, vc], [1, DIM]])
            eng.dma_start(dst[r * 14:r * 14 + vc, :], src)


def _scatter_chunk(nc, dram, src, w, c, eng=None):
    """DMA the valid tokens of half-chunk (w, c) from src [98,768] SBUF back
    to token-major [4096,768] DRAM."""
    eng = eng or nc.sync
    r0, c0, vr, vc = _chunk_geom(w, c)
    if vc == 14:
        dst = bass.AP(tensor=dram, offset=(r0 * W0 + c0) * DIM,
                      ap=[[W0 * DIM, vr], [DIM, 14], [1, DIM]])
        eng.dma_start(dst, src[0:vr * 14, :])
    else:
        for r in range(vr):
            dst = bass.AP(tensor=dram, offset=((r0 + r) * W0 + c0) * DIM,
                          ap=[[DIM, vc], [1, DIM]])
            eng.dma_start(dst, src[r * 14:r * 14 + vc, :])


def build_program():
    nc = bacc.Bacc(None, target_bir_lowering=False, debug=False)

    x_d = nc.dram_tensor("x", [NTOK, DIM], F32, kind="ExternalInput")
    qkvw_d = nc.dram_tensor("qkvw", [DIM, 3 * DIM], F32R, kind="ExternalInput")
    qkvb_d = nc.dram_tensor("qkvb", [3 * DIM], F32, kind="ExternalInput")
    projw_d = nc.dram_tensor("projw", [DIM, DIM], F32R, kind="ExternalInput")
    projb_d = nc.dram_tensor("projb", [DIM], F32R, kind="ExternalInput")
    expb_d = nc.dram_tensor("expb", [HC, HEADS, 2, WW], F32, kind="ExternalInput")
    fc1w_d = nc.dram_tensor("fc1w", [DIM, MLP_H], F32R, kind="ExternalInput")
    fc1b_d = nc.dram_tensor("fc1b", [MLP_H], F32, kind="ExternalInput")
    fc2w_d = nc.dram_tensor("fc2w", [MLP_H, DIM], F32R, kind="ExternalInput")
    fc2b_d = nc.dram_tensor("fc2b", [DIM], F32R, kind="ExternalInput")

    out_d = nc.dram_tensor("out", [NTOK, DIM], F32, kind="ExternalOutput")
    x2_d = nc.dram_tensor("x2", [NTOK, DIM], F32)  # internal scratch

    with tile.TileContext(nc) as tc:
        with ExitStack() as g:
            # ---------------- global constants / stats ----------------
            consts = g.enter_context(tc.tile_pool(name="consts", bufs=1))
            ident32 = consts.tile([128, 128], F32)
            make_identity(nc, ident32)
            ident_r = consts.tile([128, 128], F32R)
            nc.vector.tensor_copy(ident_r[:, :], ident32[:, :])
            ones32 = consts.tile([33, 128], F32)
            nc.vector.memset(ones32[:, :], 1.0)
            ones_r = consts.tile([33, 128], F32R)
            nc.vector.tensor_copy(ones_r[:, :], ones32[:, :])
            eps_t = consts.tile([128, 1], F32)
            nc.vector.memset(eps_t[:, :], EPS)
            qkvb_sb = consts.tile([128, 12], F32)
            nc.sync.dma_start(
                qkvb_sb[:, :],
                bass.AP(tensor=qkvb_d, offset=0, ap=[[1, 128], [128, 12]]))
            bias2 = consts.tile([33, DIM], F32R)
            nc.sync.dma_start(bias2[0:1, :],
                              bass.AP(tensor=projb_d, offset=0, ap=[[1, DIM]]))
            nc.sync.dma_start(bias2[32:33, :],
                              bass.AP(tensor=fc2b_d, offset=0, ap=[[1, DIM]]))
            fc1b_sb = consts.tile([128, 24], F32)
            nc.sync.dma_start(
                fc1b_sb[:, :],
                bass.AP(tensor=fc1b_d, offset=0, ap=[[1, 128], [128, 24]]))
            stats2M = consts.tile([128, 32], F32)
            stats2R = consts.tile([128, 32], F32)

            # ---------------- P2: attention over window pairs ----------------
            with ExitStack() as s2:
                wA = s2.enter_context(tc.tile_pool(name="wA", bufs=1))
                qkvw_sb = wA.tile([128, 6, 3 * DIM], F32R)
                for kk in range(6):
                    for hh in range(2):
                        eng = (nc.sync, nc.scalar, nc.gpsimd)[(2 * kk + hh) % 3]
                        eng.dma_start(
                            qkvw_sb[:, kk, hh * 1152:(hh + 1) * 1152],
                            qkvw_d[kk * 128:(kk + 1) * 128,
                                   hh * 1152:(hh + 1) * 1152])
                projw_sb = wA.tile([128, 6, DIM], F32R)
                for kk in range(3):
                    eng = (nc.sync, nc.scalar, nc.gpsimd)[kk]
                    nc.sync.dma_start(
                        projw_sb[:, 2 * kk:2 * kk + 2, :],
                        projw_d[kk * 256:(kk + 1) * 256, :]
                        .rearrange("(a p) n -> p a n", p=128))
                expb_sb = wA.tile([HC, HEADS, 2, WW], F32)
                for hh in range(3):
                    eng = (nc.sync, nc.scalar, nc.gpsimd)[hh]
                    eng.dma_start(expb_sb[:, 4 * hh:4 * (hh + 1), :, :],
                                  expb_d[:, 4 * hh:4 * (hh + 1), :, :])

                # natural_log_exp_and_others: exp (softmax) + ln/exp (rstd)
                nc.scalar.add_instruction(mybir.InstLoadActFuncSet(
                    name=nc.get_next_instruction_name(), ins=[], outs=[],
                    act_func_set_id=6))

                pxp = s2.enter_context(tc.tile_pool(name="pxp", bufs=2))
                pln = s2.enter_context(tc.tile_pool(name="pln", bufs=1))
                phT = s2.enter_context(tc.tile_pool(name="phT", bufs=1))
                pqk = s2.enter_context(tc.tile_pool(name="pqk", bufs=1))
                pvT = s2.enter_context(tc.tile_pool(name="pvT", bufs=1))
                psm = s2.enter_context(tc.tile_pool(name="psm", bufs=3))
                ppn = s2.enter_context(tc.tile_pool(name="ppn", bufs=2))
                pUT = s2.enter_context(tc.tile_pool(name="pUT", bufs=1))
                pOT = s2.enter_context(tc.tile_pool(name="pOT", bufs=1))
                px2 = s2.enter_context(tc.tile_pool(name="px2", bufs=3))
                pps = s2.enter_context(tc.tile_pool(name="pps", bufs=8, space="PSUM"))

                for pair in PAIRS:
                    nw = len(pair)
                    PW = nw * WW          # 392 or 196
                    nch = 2 * nw          # chunks in pair

                    x_pair = pxp.tile([HC, 4, DIM], F32, name="x_pair")
                    ln_pair = pln.tile([HC, 4, DIM], F32R, name="ln_pair")
                    for wl, w in enumerate(pair):
                        for c in range(2):
                            i = 2 * wl + c
                            _gather_chunk(nc, x_pair[:, i, :], x_d, w, c,
                                          eng=nc.gpsimd)
                            # LN1 stats inline: rstd = exp(-0.5*ln(var+eps))
                            st = psm.tile([HC, 3, 6], F32, name="bst")
                            for gg in range(3):
                                nc.vector.bn_stats(
                                    st[:, gg, :],
                                    x_pair[:, i, gg * 256:(gg + 1) * 256])
                            mv = psm.tile([HC, 2], F32, name="bmv")
                            nc.vector.bn_aggr(mv[:, :], st[:, :, :])
                            rstd = psm.tile([HC, 2], F32, name="rstd")
                            nc.scalar.activation(rstd[:, 0:1], mv[:, 1:2],
                                                 AF.Ln, bias=eps_t[:HC, :])
                            nc.scalar.activation(rstd[:, 1:2], rstd[:, 0:1],
                                                 AF.Exp, scale=-0.5)
                            nc.vector.tensor_scalar(
                                out=ln_pair[:, i, :], in0=x_pair[:, i, :],
                                scalar1=mv[:, 0:1],
                                scalar2=rstd[:, 1:2],
                                op0=OP.subtract, op1=OP.mult)

                    # transpose ln -> hT [128, 6, PW] (feature-major)
                    hT = phT.tile([128, 6, 2 * WW], F32R, name="hT")
                    for k in range(6):
                        ps_t = pps.tile([128, 392], F32R, tag="ps", name="ps_t")
                        for i in range(nch):
                            nc.tensor.transpose(
                                ps_t[:, i * HC:(i + 1) * HC],
                                ln_pair[:HC, i, k * 128:(k + 1) * 128],
                                ident_r[:HC, :HC])
                        nc.vector.tensor_copy(hT[:, k, :PW], ps_t[:, :PW])

                    # fused qk^T: [1536 feats, PW] (q pre-scaled on host)
                    qkT = pqk.tile([128, 12, 2 * WW], F32R, name="qkT")
                    for m in range(12):
                        ps_qk = pps.tile([128, 392], F32, tag="ps", name="ps_qk")
                        for k in range(6):
                            nc.tensor.matmul(
                                ps_qk[:, :PW],
                                qkvw_sb[:, k, m * 128:(m + 1) * 128],
                                hT[:, k, :PW],
                                start=(k == 0), stop=(k == 5))
                        nc.scalar.activation(qkT[:, m, :PW], ps_qk[:, :PW],
                                             AF.Identity, bias=qkvb_sb[:, m:m + 1])

                    # v token-major: [PW, 768]
                    vT = pvT.tile([HC, 4, DIM], F32R, name="vT")
                    for i in range(nch):
                        for n in range(2):
                            ps_v = pps.tile([128, 392], F32, tag="ps", name="ps_v")
                            for k in range(6):
                                nc.tensor.matmul(
                                    ps_v[:HC, :384],
                                    hT[:, k, i * HC:(i + 1) * HC],
                                    qkvw_sb[:, k, 2 * DIM + n * 384:2 * DIM + (n + 1) * 384],
                                    start=(k == 0), stop=(k == 5))
                            nc.vector.tensor_copy(vT[:, i, n * 384:(n + 1) * 384],
                                                  ps_v[:HC, :384])

                    OT = pOT.tile([128, 6, 2 * WW], F32R, name="OT")

                    def head_stage_a(h):
                        """scores + softmax -> normalized Pn for head h."""
                        qrow = (h % 2) * 64
                        qm = h // 2
                        km = 6 + h // 2
                        Pn = ppn.tile([HC, 4, WW], F32, name="Pn")
                        den = ppn.tile([HC, 4], F32, name="den")
                        rden = ppn.tile([HC, 4], F32, name="rden")
                        for qc in range(nch):
                            wl = qc // 2
                            ps_S = pps.tile([128, 392], F32, tag="ps", name="ps_S")
                            nc.tensor.matmul(
                                ps_S[:HC, :PW],
                                qkT[qrow:qrow + 64, qm, qc * HC:(qc + 1) * HC],
                                qkT[qrow:qrow + 64, km, :PW],
                                start=True, stop=True)
                            U = psm.tile([HC, 2 * WW], F32, name="U")
                            nc.scalar.activation(U[:, :PW], ps_S[:HC, :PW], AF.Exp)
                            P = psm.tile([HC, WW], F32, name="P")
                            nc.vector.scalar_tensor_tensor(
                                out=P[:, :], in0=U[:, wl * WW:(wl + 1) * WW],
                                scalar=1.0, in1=expb_sb[:, h, qc % 2, :],
                                op0=OP.mult, op1=OP.mult,
                                accum_out=den[:, qc:qc + 1])
                            nc.vector.reciprocal(rden[:, qc:qc + 1],
                                                 den[:, qc:qc + 1])
                            nc.vector.tensor_scalar_mul(
                                Pn[:, qc, :], P[:, :], rden[:, qc:qc + 1])
                        return Pn

                    def head_stage_b(h, Pn):
                        """transpose Pn -> UT, then O^T = V^T @ U^T."""
                        qrow = (h % 2) * 64
                        qm = h // 2
                        UT = pUT.tile([HC, 2, 2 * WW], F32R, name="UT")
                        ps_PT = [pps.tile([128, 392], F32, tag="ps",
                                          name="ps_PT") for _ in range(2)]
                        for qc in range(nch):     # qc-major: unblock early
                            for kc in range(2):
                                nc.tensor.transpose(
                                    ps_PT[kc][:HC, qc * HC:(qc + 1) * HC],
                                    Pn[:HC, qc, kc * HC:(kc + 1) * HC],
                                    ident32[:HC, :HC])
                        for kc in range(2):
                            nc.scalar.copy(UT[:, kc, :PW], ps_PT[kc][:HC, :PW])
                        for wl in range(nw):
                            ps_O = pps.tile([128, 392], F32, tag="ps", name="ps_O")
                            for kc in range(2):
                                nc.tensor.matmul(
                                    ps_O[:64, :PW],
                                    vT[:, 2 * wl + kc, h * 64:(h + 1) * 64],
                                    UT[:, kc, :PW],
                                    start=(kc == 0), stop=(kc == 1))
                            nc.vector.tensor_copy(
                                OT[qrow:qrow + 64, qm, wl * WW:(wl + 1) * WW],
                                ps_O[:64, wl * WW:(wl + 1) * WW])

                    # 2-stage pipeline over heads: S(h+1) issues before PT/O(h)
                    prev = None
                    for h in range(HEADS):
                        Pn = head_stage_a(h)
                        if prev is not None:
                            head_stage_b(h - 1, prev)
                        prev = Pn
                    head_stage_b(HEADS - 1, prev)

                    # proj + residual -> x2 (per chunk, scatter immediately)
                    for i in range(nch):
                        x2c = px2.tile([HC, DIM], F32, name="x2c")
                        for n in range(2):
                            ps_pr = pps.tile([128, 392], F32, tag="ps", name="ps_pr")
                            for k in range(6):
                                nc.tensor.matmul(
                                    ps_pr[:HC, :384],
                                    OT[:, k, i * HC:(i + 1) * HC],
                                    projw_sb[:, k, n * 384:(n + 1) * 384],
                                    start=(k == 0), stop=False)
                            nc.tensor.matmul(
                                ps_pr[:HC, :384],
                                ones_r[0:1, :HC],
                                bias2[0:1, n * 384:(n + 1) * 384],
                                start=False, stop=True)
                            nc.vector.tensor_tensor(
                                out=x2c[:, n * 384:(n + 1) * 384],
                                in0=ps_pr[:HC, :384],
                                in1=x_pair[:, i, n * 384:(n + 1) * 384],
                                op=OP.add)
                        _scatter_chunk(nc, x2_d, x2c[:, :], pair[i // 2], i % 2,
                                       eng=nc.sync)

            # ---------------- P4 + P5 (MLP) ----------------
            with ExitStack() as s5:
                # weight pool first so the half-0 prefetch DMAs overlap the
                # LN2-stats pass (no address reuse WAR on P4's pools)
                wB = s5.enter_context(tc.tile_pool(name="wB", bufs=1))

                def load_fc_weights(half):
                    engs = (nc.gpsimd, nc.sync, nc.scalar)
                    fc1w_sb = wB.tile([128, 6, MLP_H // 2], F32R, name="fc1w_sb")
                    for kk in range(6):
                        engs[kk % 3].dma_start(
                            fc1w_sb[:, kk, :],
                            fc1w_d[kk * 128:(kk + 1) * 128,
                                   half * 1536:(half + 1) * 1536])
                    fc2w_sb = wB.tile([128, 12, DIM], F32R, name="fc2w_sb")
                    for kk in range(4):
                        engs[kk % 3].dma_start(
                            fc2w_sb[:, 3 * kk:3 * (kk + 1), :],
                            fc2w_d[half * 1536 + kk * 384:
                                   half * 1536 + (kk + 1) * 384, :]
                            .rearrange("(a p) n -> p a n", p=128))
                    return fc1w_sb, fc2w_sb

                w_half0 = load_fc_weights(0)

                pxt = s5.enter_context(tc.tile_pool(name="pxt", bufs=4))
                pxn = s5.enter_context(tc.tile_pool(name="pxn", bufs=2))
                pxnT = s5.enter_context(tc.tile_pool(name="pxnT", bufs=2))
                pgT = s5.enter_context(tc.tile_pool(name="pgT", bufs=2))
                pout = s5.enter_context(tc.tile_pool(name="pout", bufs=4))
                pps5 = s5.enter_context(tc.tile_pool(name="pps5", bufs=8, space="PSUM"))


                for half in range(2):
                    fc1w_sb, fc2w_sb = w_half0 if half == 0 else load_fc_weights(1)

                    def mlp_prep(st):
                        """load x2 tiles, (half0: LN2 stats), apply, transpose."""
                        x2t = []
                        x2nT = pxnT.tile([128, 6, 256], F32R, name="x2nT")
                        for c in range(2):
                            t = st * 2 + c
                            xt = pxt.tile([128, DIM], F32, name="x2t")
                            eng = nc.scalar if t % 2 == 0 else nc.sync
                            eng.dma_start(xt[:, :], x2_d[t * 128:(t + 1) * 128, :])
                            x2t.append(xt)
                            if half == 0:
                                # LN2 stats inline; rsqrt on DVE only
                                # (quake seed + 3 Newton steps, ~1.5e-7 rel)
                                st5 = pxn.tile([128, 3, 6], F32, name="st5")
                                for gg in range(3):
                                    nc.vector.bn_stats(
                                        st5[:, gg, :],
                                        xt[:, gg * 256:(gg + 1) * 256])
                                mv5 = pxn.tile([128, 8], F32, name="mv5")
                                nc.vector.bn_aggr(mv5[:, 0:2], st5[:, :, :])
                                nc.gpsimd.tensor_copy(stats2M[:, t:t + 1],
                                                      mv5[:, 0:1])
                                ve = mv5[:, 2:3]
                                nc.vector.tensor_scalar(
                                    out=ve, in0=mv5[:, 1:2], scalar1=EPS,
                                    scalar2=None, op0=OP.add)
                                yi = mv5[:, 3:4].bitcast(mybir.dt.int32)
                                nc.vector.tensor_scalar(
                                    out=yi, in0=ve.bitcast(mybir.dt.int32),
                                    scalar1=1, scalar2=None,
                                    op0=OP.arith_shift_right)
                                y0 = mv5[:, 4:5].bitcast(mybir.dt.int32)
                                nc.vector.tensor_scalar(
                                    out=y0, in0=yi, scalar1=-1,
                                    scalar2=None, op0=OP.bitwise_xor)
                                nc.vector.tensor_scalar(
                                    out=y0, in0=y0, scalar1=0x5f3759e0,
                                    scalar2=None, op0=OP.add)
                                ya = mv5[:, 4:5]
                                yb = mv5[:, 5:6]
                                t2 = mv5[:, 6:7]
                                w5 = mv5[:, 7:8]
                                for _ in range(3):
                                    nc.vector.tensor_tensor(
                                        out=t2, in0=ya, in1=ya, op=OP.mult)
                                    nc.vector.scalar_tensor_tensor(
                                        out=w5, in0=ve, scalar=-0.5, in1=t2,
                                        op0=OP.mult, op1=OP.mult)
                                    nc.vector.tensor_scalar(
                                        out=w5, in0=w5, scalar1=1.5,
                                        scalar2=None, op0=OP.add)
                                    nc.vector.tensor_tensor(
                                        out=yb, in0=ya, in1=w5, op=OP.mult)
                                    ya, yb = yb, ya
                                nc.gpsimd.tensor_copy(stats2R[:, t:t + 1], ya)
                            xn = pxn.tile([128, DIM], F32R, name="x2n")
                            nc.vector.tensor_scalar(
                                out=xn[:, :], in0=xt[:, :],
                                scalar1=stats2M[:, t:t + 1],
                                scalar2=stats2R[:, t:t + 1],
                                op0=OP.subtract, op1=OP.mult)
                            for k in range(6):
                                ps_t2 = pps5.tile([128, 384], F32R, tag="ps5",
                                                  name="ps_t2")
                                nc.tensor.transpose(
                                    ps_t2[:, :128], xn[:, k * 128:(k + 1) * 128],
                                    ident_r[:, :])
                                nc.vector.tensor_copy(
                                    x2nT[:, k, c * 128:(c + 1) * 128],
                                    ps_t2[:, :128])
                        return x2t, x2nT

                    def mlp_fc1(x2nT):
                        gT = pgT.tile([128, 12, 256], F32R, name="gT")
                        for m in range(12):
                            ps_f1 = pps5.tile([128, 384], F32, tag="ps5", name="ps_f1")
                            for k in range(6):
                                nc.tensor.matmul(
                                    ps_f1[:, :256],
                                    fc1w_sb[:, k, m * 128:(m + 1) * 128],
                                    x2nT[:, k, :],
                                    start=(k == 0), stop=(k == 5))
                            nc.scalar.activation(
                                gT[:, m, :], ps_f1[:, :256], AF.Gelu,
                                bias=fc1b_sb[:, half * 12 + m:half * 12 + m + 1])
                        return gT

                    def mlp_fc2(st, x2t, gT):
                        for c in range(2):
                            outt = pout.tile([128, DIM], F32, name="outt")
                            for n in range(2):
                                ps_f2 = pps5.tile([128, 384], F32, tag="ps5",
                                                  name="ps_f2")
                                for m in range(12):
                                    nc.tensor.matmul(
                                        ps_f2[:, :384],
                                        gT[:, m, c * 128:(c + 1) * 128],
                                        fc2w_sb[:, m, n * 384:(n + 1) * 384],
                                        start=(m == 0), stop=(m == 11 and half == 1))
                                if half == 0:
                                    nc.tensor.matmul(
                                        ps_f2[:, :384],
                                        ones_r[32:33, :128],
                                        bias2[32:33, n * 384:(n + 1) * 384],
                                        start=False, stop=True)
                                    nc.vector.tensor_tensor(
                                        out=outt[:, n * 384:(n + 1) * 384],
                                        in0=ps_f2[:, :384],
                                        in1=x2t[c][:, n * 384:(n + 1) * 384],
                                        op=OP.add)
                                else:
                                    nc.vector.tensor_copy(
                                        outt[:, n * 384:(n + 1) * 384],
                                        ps_f2[:, :384])
                            t = st * 2 + c
                            if half == 0:
                                nc.sync.dma_start(
                                    out_d[t * 128:(t + 1) * 128, :], outt[:, :])
                            else:
                                nc.gpsimd.dma_start(
                                    out_d[t * 128:(t + 1) * 128, :], outt[:, :],
                                    accum_op=OP.add)

                    # 2-stage pipeline over super-tiles: prep(st+1) is emitted
                    # between fc1(st) and fc2(st) so its PE transposes fill
                    # the gelu wait.
                    cur = mlp_prep(0)
                    for st in range(16):
                        gT = mlp_fc1(cur[1])
                        nxt = mlp_prep(st + 1) if st < 15 else None
                        mlp_fc2(st, cur[0], gT)
                        cur = nxt

    nc.compile()
    return nc


_NC_CACHE = {}


def _get_nc():
    if "nc" not in _NC_CACHE:
        _NC_CACHE["nc"] = build_program()
    return _NC_CACHE["nc"]


def _prep_weights(inputs):
    f = lambda k: np.asarray(inputs[k], np.float32)
    x = f("x")
    ln1_g, ln1_b = f("ln1_g"), f("ln1_b")
    ln2_g, ln2_b = f("ln2_g"), f("ln2_b")
    qkv_w, qkv_b = f("qkv_w"), f("qkv_b")
    proj_w, proj_b = f("proj_w"), f("proj_b")
    fc1_w, fc1_b = f("fc1_w"), f("fc1_b")
    fc2_w, fc2_b = f("fc2_w"), f("fc2_b")
    rel = f("rel_pos_bias")
    SCALE = np.float32((DIM // HEADS) ** -0.5)

    # fold ln1 into qkv
    qkv_w_f = qkv_w * ln1_g[None, :]
    qkv_b_f = qkv_w @ ln1_b + qkv_b
    # fold attention scale into q block
    qkv_w_f[:DIM] *= SCALE
    qkv_b_f2 = qkv_b_f.copy()
    qkv_b_f2[:DIM] *= SCALE
    # fold v bias into proj bias (softmax rows sum to 1)
    projb = proj_b + proj_w @ qkv_b_f[2 * DIM:]
    # fold ln2 into fc1
    fc1_w_f = fc1_w * ln2_g[None, :]
    fc1_b_f = fc1_w @ ln2_b + fc1_b

    expb = np.exp(rel).astype(np.float32)          # [12, 196, 196]
    expb = expb.reshape(HEADS, 2, HC, WW).transpose(2, 0, 1, 3).copy()

    return {
        "qkvw": np.ascontiguousarray(qkv_w_f.T),    # [768, 2304]
        "qkvb": qkv_b_f2,
        "projw": np.ascontiguousarray(proj_w.T),    # [768, 768]
        "projb": projb,
        "expb": expb,
        "fc1w": np.ascontiguousarray(fc1_w_f.T),    # [768, 3072]
        "fc1b": fc1_b_f,
        "fc2w": np.ascontiguousarray(fc2_w.T),      # [3072, 768]
        "fc2b": fc2_b,
    }, x


PROFILE = False
LAST_RESULT = None


def kernel(**inputs):
    global LAST_RESULT
    weights, x = _prep_weights(inputs)
    nc = _get_nc()
    in_maps = [dict(weights, x=np.ascontiguousarray(x[i])) for i in range(B)]
    res = run_bass_kernel_spmd(nc, in_maps, core_ids=list(range(B)),
                               trace=PROFILE)
    LAST_RESULT = res
    out = np.stack([res.results[i]["out"] for i in range(B)], axis=0)
    return out.astype(np.float32)


if __name__ == "__main__":
    rng = np.random.default_rng(0)
    ins = {
        "x": rng.standard_normal((B, NTOK, DIM), dtype=np.float32),
        "rel_pos_bias": (rng.standard_normal((HEADS, WW, WW)) * 0.1).astype(np.float32),
        "ln1_g": np.ones(DIM, np.float32), "ln1_b": np.zeros(DIM, np.float32),
        "qkv_w": (rng.standard_normal((3 * DIM, DIM)) * 0.02).astype(np.float32),
        "qkv_b": np.zeros(3 * DIM, np.float32),
        "proj_w": (rng.standard_normal((DIM, DIM)) * 0.02).astype(np.float32),
        "proj_b": np.zeros(DIM, np.float32),
        "ln2_g": np.ones(DIM, np.float32), "ln2_b": np.zeros(DIM, np.float32),
        "fc1_w": (rng.standard_normal((MLP_H, DIM)) * 0.02).astype(np.float32),
        "fc1_b": np.zeros(MLP_H, np.float32),
        "fc2_w": (rng.standard_normal((DIM, MLP_H)) * 0.02).astype(np.float32),
        "fc2_b": np.zeros(DIM, np.float32),
        "H": np.int64(64), "W": np.int64(64),
    }
    out = kernel(**ins)
    print("out", out.shape, out.dtype, np.abs(out).max())


# revision 25
# speedup vs baseline: 2.0715x; 1.0034x over previous
"""Swin-style transformer block on 8 Trainium2 NeuronCores.

Sharding: data-parallel over batch — each of the 8 cores processes one image
([4096, 768] tokens). All weights replicated per core. No collectives.

Per-core pipeline:
  Attention, 13 window-pairs (pairing makes the matmul moving dim 392 >= 256
  so fp32r runs at full PE rate): gather + inline LN1 stats (rstd via
  ln/exp, same ACT table set as softmax) -> LN1-apply -> PE transpose ->
  fused QKV -> per-head scores -> softmax exp(S)*exp(B) with deferred 1/sum
  -> transpose P -> O^T = V^T P^T -> proj + residual -> scatter x2 to DRAM.
  Heads run in a 2-stage software pipeline (scores of head h+1 issue before
  transpose/O of head h).
  MLP in two hidden-halves of 1536 (weights fit SBUF): per 256-token
  super-tile: LN2 (stats inline on half 0; rsqrt = quake seed + 3 Newton
  steps, DVE-only, no ACT table switch) -> transpose -> fc1 (feature-major
  out) + erf-GELU -> fc2 + residual; half 1 accumulates into the output via
  gpsimd accum-DMA. Super-tiles are software-pipelined.

Host-side folds (all exact for the given inputs; SCALE=0.125 is binary-exact):
  - ln1 gamma/beta folded into qkv weights/bias; attention SCALE folded into
    the q block of qkv; v-bias folded into the proj bias (softmax rows sum
    to 1); ln2 gamma/beta folded into fc1; rel_pos_bias shipped as
    exp(rel_pos_bias) so softmax(S+B) = exp(S)*expB / sum(...).
"""

import numpy as np
from contextlib import ExitStack

import concourse.bass as bass
import concourse.mybir as mybir
import concourse.tile as tile
from concourse import bacc
from concourse.bass_utils import run_bass_kernel_spmd
from concourse.masks import make_identity

F32 = mybir.dt.float32
F32R = mybir.dt.float32r
AF = mybir.ActivationFunctionType
OP = mybir.AluOpType

DIM, HEADS, WIN, MLP_H = 768, 12, 14, 3072
B, H0, W0 = 8, 64, 64
NTOK = H0 * W0
NW = 5            # windows per image axis (70/14)
NWIN = NW * NW    # 25 windows
WW = WIN * WIN    # 196 tokens per window
HC = 98           # half-window chunk (7 rows x 14 cols)
DH = DIM // HEADS # 64
EPS = 1e-5

# window pairing: 12 pairs + 1 single
PAIRS = [(2 * i, 2 * i + 1) for i in range(12)] + [(24,)]


def _chunk_geom(w, c):
    """Valid-row/col geometry of half-chunk c (0/1) of window w."""
    wi, wj = divmod(w, NW)
    r0 = wi * WIN + c * 7          # first padded-image row of this chunk
    c0 = wj * WIN
    vr = 7 if (wi < 4 or c == 0) else 1   # wi==4 -> rows 56..63 valid (8)
    vc = 14 if wj < 4 else 8
    return r0, c0, vr, vc


def _gather_chunk(nc, dst, dram, w, c, eng=None):
    """DMA image tokens of half-chunk (w, c) from [4096,768] DRAM into
    dst [98, 768] SBUF tile (partition p = 14*row + col). Pads with zeros."""
    eng = eng or nc.sync
    r0, c0, vr, vc = _chunk_geom(w, c)
    if vr < 7 or vc < 14:
        nc.gpsimd.memset(dst[:, :], 0.0)
    if vc == 14:
        src = bass.AP(tensor=dram, offset=(r0 * W0 + c0) * DIM,
                      ap=[[W0 * DIM, vr], [DIM, 14], [1, DIM]])
        eng.dma_start(dst[0:vr * 14, :], src)
    else:
        for r in range(vr):
            src = bass.AP(tensor=dram, offset=((r0 + r) * W0 + c0) * DIM,
                          ap=[[DIM, vc], [1, DIM]])
            eng.dma_start(dst[r * 14:r * 14 + vc, :], src)


def _scatter_chunk(nc, dram, src, w, c, eng=None):
    """DMA the valid tokens of half-chunk (w, c) from src [98,768] SBUF back
    to token-major [4096,768] DRAM."""
    eng = eng or nc.sync
    r0, c0, vr, vc = _chunk_geom(w, c)
    if vc == 14:
        dst = bass.AP(tensor=dram, offset=(r0 * W0 + c0) * DIM,
                      ap=[[W0 * DIM, vr], [DIM, 14], [1, DIM]])
        eng.dma_start(dst, src[0:vr * 14, :])
    else:
        for r in range(vr):
            dst = bass.AP(tensor=dram, offset=((r0 + r) * W0 + c0) * DIM,
                          ap=[[DIM, vc], [1, DIM]])
            eng.dma_start(dst, src[r * 14:r * 14 + vc, :])


def build_program():
    nc = bacc.Bacc(None, target_bir_lowering=False, debug=False)

    x_d = nc.dram_tensor("x", [NTOK, DIM], F32, kind="ExternalInput")
    qkvw_d = nc.dram_tensor("qkvw", [DIM, 3 * DIM], F32R, kind="ExternalInput")
    qkvb_d = nc.dram_tensor("qkvb", [3 * DIM], F32, kind="ExternalInput")
    projw_d = nc.dram_tensor("projw", [DIM, DIM], F32R, kind="ExternalInput")
    projb_d = nc.dram_tensor("projb", [DIM], F32R, kind="ExternalInput")
    expb_d = nc.dram_tensor("expb", [HC, HEADS, 2, WW], F32, kind="ExternalInput")
    fc1w_d = nc.dram_tensor("fc1w", [DIM, MLP_H], F32R, kind="ExternalInput")
    fc1b_d = nc.dram_tensor("fc1b", [MLP_H], F32, kind="ExternalInput")
    fc2w_d = nc.dram_tensor("fc2w", [MLP_H, DIM], F32R, kind="ExternalInput")
    fc2b_d = nc.dram_tensor("fc2b", [DIM], F32R, kind="ExternalInput")

    out_d = nc.dram_tensor("out", [NTOK, DIM], F32, kind="ExternalOutput")
    x2_d = nc.dram_tensor("x2", [NTOK, DIM], F32)  # internal scratch

    with tile.TileContext(nc) as tc:
        with ExitStack() as g:
            # ---------------- global constants / stats ----------------
            consts = g.enter_context(tc.tile_pool(name="consts", bufs=1))
            ident32 = consts.tile([128, 128], F32)
            make_identity(nc, ident32)
            ident_r = consts.tile([128, 128], F32R)
            nc.vector.tensor_copy(ident_r[:, :], ident32[:, :])
            ones32 = consts.tile([33, 128], F32)
            nc.vector.memset(ones32[:, :], 1.0)
            ones_r = consts.tile([33, 128], F32R)
            nc.vector.tensor_copy(ones_r[:, :], ones32[:, :])
            eps_t = consts.tile([128, 1], F32)
            nc.vector.memset(eps_t[:, :], EPS)
            qkvb_sb = consts.tile([128, 12], F32)
            nc.sync.dma_start(
                qkvb_sb[:, :],
                bass.AP(tensor=qkvb_d, offset=0, ap=[[1, 128], [128, 12]]))
            bias2 = consts.tile([33, DIM], F32R)
            nc.sync.dma_start(bias2[0:1, :],
                              bass.AP(tensor=projb_d, offset=0, ap=[[1, DIM]]))
            nc.sync.dma_start(bias2[32:33, :],
                              bass.AP(tensor=fc2b_d, offset=0, ap=[[1, DIM]]))
            fc1b_sb = consts.tile([128, 24], F32)
            nc.sync.dma_start(
                fc1b_sb[:, :],
                bass.AP(tensor=fc1b_d, offset=0, ap=[[1, 128], [128, 24]]))
            stats2M = consts.tile([128, 32], F32)
            stats2R = consts.tile([128, 32], F32)

            # ---------------- P2: attention over window pairs ----------------
            with ExitStack() as s2:
                wA = s2.enter_context(tc.tile_pool(name="wA", bufs=1))
                qkvw_sb = wA.tile([128, 6, 3 * DIM], F32R)
                for kk in range(6):
                    for hh in range(2):
                        eng = (nc.sync, nc.scalar, nc.gpsimd)[(2 * kk + hh) % 3]
                        eng.dma_start(
                            qkvw_sb[:, kk, hh * 1152:(hh + 1) * 1152],
                            qkvw_d[kk * 128:(kk + 1) * 128,
                                   hh * 1152:(hh + 1) * 1152])
                projw_sb = wA.tile([128, 6, DIM], F32R)
                for kk in range(3):
                    eng = (nc.sync, nc.scalar, nc.gpsimd)[kk]
                    nc.sync.dma_start(
                        projw_sb[:, 2 * kk:2 * kk + 2, :],
                        projw_d[kk * 256:(kk + 1) * 256, :]
                        .rearrange("(a p) n -> p a n", p=128))
                expb_sb = wA.tile([HC, HEADS, 2, WW], F32)
                for hh in range(3):
                    eng = (nc.sync, nc.scalar, nc.gpsimd)[hh]
                    eng.dma_start(expb_sb[:, 4 * hh:4 * (hh + 1), :, :],
                                  expb_d[:, 4 * hh:4 * (hh + 1), :, :])

                # natural_log_exp_and_others: exp (softmax) + ln/exp (rstd)
                nc.scalar.add_instruction(mybir.InstLoadActFuncSet(
                    name=nc.get_next_instruction_name(), ins=[], outs=[],
                    act_func_set_id=6))

                pxp = s2.enter_context(tc.tile_pool(name="pxp", bufs=2))
                pln = s2.enter_context(tc.tile_pool(name="pln", bufs=1))
                phT = s2.enter_context(tc.tile_pool(name="phT", bufs=1))
                pqk = s2.enter_context(tc.tile_pool(name="pqk", bufs=1))
                pvT = s2.enter_context(tc.tile_pool(name="pvT", bufs=1))
                psm = s2.enter_context(tc.tile_pool(name="psm", bufs=3))
                ppn = s2.enter_context(tc.tile_pool(name="ppn", bufs=2))
                pUT = s2.enter_context(tc.tile_pool(name="pUT", bufs=1))
                pOT = s2.enter_context(tc.tile_pool(name="pOT", bufs=1))
                px2 = s2.enter_context(tc.tile_pool(name="px2", bufs=3))
                pps = s2.enter_context(tc.tile_pool(name="pps", bufs=8, space="PSUM"))

                for pair in PAIRS:
                    nw = len(pair)
                    PW = nw * WW          # 392 or 196
                    nch = 2 * nw          # chunks in pair

                    x_pair = pxp.tile([HC, 4, DIM], F32, name="x_pair")
                    ln_pair = pln.tile([HC, 4, DIM], F32R, name="ln_pair")
                    for wl, w in enumerate(pair):
                        for c in range(2):
                            i = 2 * wl + c
                            _gather_chunk(nc, x_pair[:, i, :], x_d, w, c,
                                          eng=nc.gpsimd)
                            # LN1 stats inline: rstd = exp(-0.5*ln(var+eps))
                            st = psm.tile([HC, 3, 6], F32, name="bst")
                            for gg in range(3):
                                nc.vector.bn_stats(
                                    st[:, gg, :],
                                    x_pair[:, i, gg * 256:(gg + 1) * 256])
                            mv = psm.tile([HC, 2], F32, name="bmv")
                            nc.vector.bn_aggr(mv[:, :], st[:, :, :])
                            rstd = psm.tile([HC, 2], F32, name="rstd")
                            nc.scalar.activation(rstd[:, 0:1], mv[:, 1:2],
                                                 AF.Ln, bias=eps_t[:HC, :])
                            nc.scalar.activation(rstd[:, 1:2], rstd[:, 0:1],
                                                 AF.Exp, scale=-0.5)
                            nc.vector.tensor_scalar(
                                out=ln_pair[:, i, :], in0=x_pair[:, i, :],
                                scalar1=mv[:, 0:1],
                                scalar2=rstd[:, 1:2],
                                op0=OP.subtract, op1=OP.mult)

                    # transpose ln -> hT [128, 6, PW] (feature-major)
                    hT = phT.tile([128, 6, 2 * WW], F32R, name="hT")
                    for k in range(6):
                        ps_t = pps.tile([128, 392], F32R, tag="ps", name="ps_t")
                        for i in range(nch):
                            nc.tensor.transpose(
                                ps_t[:, i * HC:(i + 1) * HC],
                                ln_pair[:HC, i, k * 128:(k + 1) * 128],
                                ident_r[:HC, :HC])
                        nc.vector.tensor_copy(hT[:, k, :PW], ps_t[:, :PW])

                    # fused qk^T: [1536 feats, PW] (q pre-scaled on host).
                    # Emit q/k chunks interleaved (0,6,1,7,..) so head h's
                    # operands (chunks h//2 and 6+h//2) finish earliest.
                    qkT = pqk.tile([128, 12, 2 * WW], F32R, name="qkT")
                    for m in [0, 6, 1, 7, 2, 8, 3, 9, 4, 10, 5, 11]:
                        ps_qk = pps.tile([128, 392], F32, tag="ps", name="ps_qk")
                        for k in range(6):
                            nc.tensor.matmul(
                                ps_qk[:, :PW],
                                qkvw_sb[:, k, m * 128:(m + 1) * 128],
                                hT[:, k, :PW],
                                start=(k == 0), stop=(k == 5))
                        nc.scalar.activation(qkT[:, m, :PW], ps_qk[:, :PW],
                                             AF.Identity, bias=qkvb_sb[:, m:m + 1])

                    # v token-major: [PW, 768]
                    vT = pvT.tile([HC, 4, DIM], F32R, name="vT")
                    for i in range(nch):
                        for n in range(2):
                            ps_v = pps.tile([128, 392], F32, tag="ps", name="ps_v")
                            for k in range(6):
                                nc.tensor.matmul(
                                    ps_v[:HC, :384],
                                    hT[:, k, i * HC:(i + 1) * HC],
                                    qkvw_sb[:, k, 2 * DIM + n * 384:2 * DIM + (n + 1) * 384],
                                    start=(k == 0), stop=(k == 5))
                            nc.vector.tensor_copy(vT[:, i, n * 384:(n + 1) * 384],
                                                  ps_v[:HC, :384])

                    OT = pOT.tile([128, 6, 2 * WW], F32R, name="OT")

                    def head_stage_a(h):
                        """scores + softmax -> normalized Pn for head h."""
                        qrow = (h % 2) * 64
                        qm = h // 2
                        km = 6 + h // 2
                        Pn = ppn.tile([HC, 4, WW], F32, name="Pn")
                        den = ppn.tile([HC, 4], F32, name="den")
                        rden = ppn.tile([HC, 4], F32, name="rden")
                        for qc in range(nch):
                            wl = qc // 2
                            ps_S = pps.tile([128, 392], F32, tag="ps", name="ps_S")
                            nc.tensor.matmul(
                                ps_S[:HC, :PW],
                                qkT[qrow:qrow + 64, qm, qc * HC:(qc + 1) * HC],
                                qkT[qrow:qrow + 64, km, :PW],
                                start=True, stop=True)
                            U = psm.tile([HC, 2 * WW], F32, name="U")
                            nc.scalar.activation(U[:, :PW], ps_S[:HC, :PW], AF.Exp)
                            P = psm.tile([HC, WW], F32, name="P")
                            nc.vector.scalar_tensor_tensor(
                                out=P[:, :], in0=U[:, wl * WW:(wl + 1) * WW],
                                scalar=1.0, in1=expb_sb[:, h, qc % 2, :],
                                op0=OP.mult, op1=OP.mult,
                                accum_out=den[:, qc:qc + 1])
                            nc.vector.reciprocal(rden[:, qc:qc + 1],
                                                 den[:, qc:qc + 1])
                            nc.vector.tensor_scalar_mul(
                                Pn[:, qc, :], P[:, :], rden[:, qc:qc + 1])
                        return Pn

                    def head_stage_b(h, Pn):
                        """transpose Pn -> UT, then O^T = V^T @ U^T."""
                        qrow = (h % 2) * 64
                        qm = h // 2
                        UT = pUT.tile([HC, 2, 2 * WW], F32R, name="UT")
                        ps_PT = [pps.tile([128, 392], F32, tag="ps",
                                          name="ps_PT") for _ in range(2)]
                        for qc in range(nch):     # qc-major: unblock early
                            for kc in range(2):
                                nc.tensor.transpose(
                                    ps_PT[kc][:HC, qc * HC:(qc + 1) * HC],
                                    Pn[:HC, qc, kc * HC:(kc + 1) * HC],
                                    ident32[:HC, :HC])
                        for kc in range(2):
                            nc.scalar.copy(UT[:, kc, :PW], ps_PT[kc][:HC, :PW])
                        for wl in range(nw):
                            ps_O = pps.tile([128, 392], F32, tag="ps", name="ps_O")
                            for kc in range(2):
                                nc.tensor.matmul(
                                    ps_O[:64, :PW],
                                    vT[:, 2 * wl + kc, h * 64:(h + 1) * 64],
                                    UT[:, kc, :PW],
                                    start=(kc == 0), stop=(kc == 1))
                            nc.vector.tensor_copy(
                                OT[qrow:qrow + 64, qm, wl * WW:(wl + 1) * WW],
                                ps_O[:64, wl * WW:(wl + 1) * WW])

                    # 2-stage pipeline over heads: S(h+1) issues before PT/O(h)
                    prev = None
                    for h in range(HEADS):
                        Pn = head_stage_a(h)
                        if prev is not None:
                            head_stage_b(h - 1, prev)
                        prev = Pn
                    head_stage_b(HEADS - 1, prev)

                    # proj + residual -> x2 (per chunk, scatter immediately)
                    for i in range(nch):
                        x2c = px2.tile([HC, DIM], F32, name="x2c")
                        for n in range(2):
                            ps_pr = pps.tile([128, 392], F32, tag="ps", name="ps_pr")
                            for k in range(6):
                                nc.tensor.matmul(
                                    ps_pr[:HC, :384],
                                    OT[:, k, i * HC:(i + 1) * HC],
                                    projw_sb[:, k, n * 384:(n + 1) * 384],
                                    start=(k == 0), stop=False)
                            nc.tensor.matmul(
                                ps_pr[:HC, :384],
                                ones_r[0:1, :HC],
                                bias2[0:1, n * 384:(n + 1) * 384],
                                start=False, stop=True)
                            nc.vector.tensor_tensor(
                                out=x2c[:, n * 384:(n + 1) * 384],
                                in0=ps_pr[:HC, :384],
                                in1=x_pair[:, i, n * 384:(n + 1) * 384],
                                op=OP.add)
                        _scatter_chunk(nc, x2_d, x2c[:, :], pair[i // 2], i % 2,
                                       eng=nc.sync)

            # ---------------- P4 + P5 (MLP) ----------------
            with ExitStack() as s5:
                # weight pool first so the half-0 prefetch DMAs overlap the
                # LN2-stats pass (no address reuse WAR on P4's pools)
                wB = s5.enter_context(tc.tile_pool(name="wB", bufs=1))

                def load_fc_weights(half):
                    engs = (nc.gpsimd, nc.sync, nc.scalar)
                    fc1w_sb = wB.tile([128, 6, MLP_H // 2], F32R, name="fc1w_sb")
                    for kk in range(6):
                        engs[kk % 3].dma_start(
                            fc1w_sb[:, kk, :],
                            fc1w_d[kk * 128:(kk + 1) * 128,
                                   half * 1536:(half + 1) * 1536])
                    fc2w_sb = wB.tile([128, 12, DIM], F32R, name="fc2w_sb")
                    for kk in range(4):
                        engs[kk % 3].dma_start(
                            fc2w_sb[:, 3 * kk:3 * (kk + 1), :],
                            fc2w_d[half * 1536 + kk * 384:
                                   half * 1536 + (kk + 1) * 384, :]
                            .rearrange("(a p) n -> p a n", p=128))
                    return fc1w_sb, fc2w_sb

                w_half0 = load_fc_weights(0)

                pxt = s5.enter_context(tc.tile_pool(name="pxt", bufs=4))
                pxn = s5.enter_context(tc.tile_pool(name="pxn", bufs=2))
                pxnT = s5.enter_context(tc.tile_pool(name="pxnT", bufs=2))
                pgT = s5.enter_context(tc.tile_pool(name="pgT", bufs=2))
                pout = s5.enter_context(tc.tile_pool(name="pout", bufs=4))
                pps5 = s5.enter_context(tc.tile_pool(name="pps5", bufs=8, space="PSUM"))


                for half in range(2):
                    fc1w_sb, fc2w_sb = w_half0 if half == 0 else load_fc_weights(1)

                    def mlp_prep(st):
                        """load x2 tiles, (half0: LN2 stats), apply, transpose."""
                        x2t = []
                        x2nT = pxnT.tile([128, 6, 256], F32R, name="x2nT")
                        for c in range(2):
                            t = st * 2 + c
                            xt = pxt.tile([128, DIM], F32, name="x2t")
                            eng = nc.scalar if t % 2 == 0 else nc.sync
                            eng.dma_start(xt[:, :], x2_d[t * 128:(t + 1) * 128, :])
                            x2t.append(xt)
                            if half == 0:
                                # LN2 stats inline; rsqrt on DVE only
                                # (quake seed + 3 Newton steps, ~1.5e-7 rel)
                                st5 = pxn.tile([128, 3, 6], F32, name="st5")
                                for gg in range(3):
                                    nc.vector.bn_stats(
                                        st5[:, gg, :],
                                        xt[:, gg * 256:(gg + 1) * 256])
                                mv5 = pxn.tile([128, 8], F32, name="mv5")
                                nc.vector.bn_aggr(mv5[:, 0:2], st5[:, :, :])
                                nc.gpsimd.tensor_copy(stats2M[:, t:t + 1],
                                                      mv5[:, 0:1])
                                ve = mv5[:, 2:3]
                                nc.vector.tensor_scalar(
                                    out=ve, in0=mv5[:, 1:2], scalar1=EPS,
                                    scalar2=None, op0=OP.add)
                                yi = mv5[:, 3:4].bitcast(mybir.dt.int32)
                                nc.vector.tensor_scalar(
                                    out=yi, in0=ve.bitcast(mybir.dt.int32),
                                    scalar1=1, scalar2=None,
                                    op0=OP.arith_shift_right)
                                y0 = mv5[:, 4:5].bitcast(mybir.dt.int32)
                                nc.vector.tensor_scalar(
                                    out=y0, in0=yi, scalar1=-1,
                                    scalar2=None, op0=OP.bitwise_xor)
                                nc.vector.tensor_scalar(
                                    out=y0, in0=y0, scalar1=0x5f3759e0,
                                    scalar2=None, op0=OP.add)
                                ya = mv5[:, 4:5]
                                yb = mv5[:, 5:6]
                                t2 = mv5[:, 6:7]
                                w5 = mv5[:, 7:8]
                                for _ in range(3):
                                    nc.vector.tensor_tensor(
                                        out=t2, in0=ya, in1=ya, op=OP.mult)
                                    nc.vector.scalar_tensor_tensor(
                                        out=w5, in0=ve, scalar=-0.5, in1=t2,
                                        op0=OP.mult, op1=OP.mult)
                                    nc.vector.tensor_scalar(
                                        out=w5, in0=w5, scalar1=1.5,
                                        scalar2=None, op0=OP.add)
                                    nc.vector.tensor_tensor(
                                        out=yb, in0=ya, in1=w5, op=OP.mult)
                                    ya, yb = yb, ya
                                nc.gpsimd.tensor_copy(stats2R[:, t:t + 1], ya)
                            xn = pxn.tile([128, DIM], F32R, name="x2n")
                            nc.vector.tensor_scalar(
                                out=xn[:, :], in0=xt[:, :],
                                scalar1=stats2M[:, t:t + 1],
                                scalar2=stats2R[:, t:t + 1],
                                op0=OP.subtract, op1=OP.mult)
                            for k in range(6):
                                ps_t2 = pps5.tile([128, 384], F32R, tag="ps5",
                                                  name="ps_t2")
                                nc.tensor.transpose(
                                    ps_t2[:, :128], xn[:, k * 128:(k + 1) * 128],
                                    ident_r[:, :])
                                nc.vector.tensor_copy(
                                    x2nT[:, k, c * 128:(c + 1) * 128],
                                    ps_t2[:, :128])
                        return x2t, x2nT

                    def mlp_fc1(x2nT):
                        gT = pgT.tile([128, 12, 256], F32R, name="gT")
                        for m in range(12):
                            ps_f1 = pps5.tile([128, 384], F32, tag="ps5", name="ps_f1")
                            for k in range(6):
                                nc.tensor.matmul(
                                    ps_f1[:, :256],
                                    fc1w_sb[:, k, m * 128:(m + 1) * 128],
                                    x2nT[:, k, :],
                                    start=(k == 0), stop=(k == 5))
                            nc.scalar.activation(
                                gT[:, m, :], ps_f1[:, :256], AF.Gelu,
                                bias=fc1b_sb[:, half * 12 + m:half * 12 + m + 1])
                        return gT

                    def mlp_fc2(st, x2t, gT):
                        for c in range(2):
                            outt = pout.tile([128, DIM], F32, name="outt")
                            for n in range(2):
                                ps_f2 = pps5.tile([128, 384], F32, tag="ps5",
                                                  name="ps_f2")
                                for m in range(12):
                                    nc.tensor.matmul(
                                        ps_f2[:, :384],
                                        gT[:, m, c * 128:(c + 1) * 128],
                                        fc2w_sb[:, m, n * 384:(n + 1) * 384],
                                        start=(m == 0), stop=(m == 11 and half == 1))
                                if half == 0:
                                    nc.tensor.matmul(
                                        ps_f2[:, :384],
                                        ones_r[32:33, :128],
                                        bias2[32:33, n * 384:(n + 1) * 384],
                                        start=False, stop=True)
                                    nc.vector.tensor_tensor(
                                        out=outt[:, n * 384:(n + 1) * 384],
                                        in0=ps_f2[:, :384],
                                        in1=x2t[c][:, n * 384:(n + 1) * 384],
                                        op=OP.add)
                                else:
                                    nc.vector.tensor_copy(
                                        outt[:, n * 384:(n + 1) * 384],
                                        ps_f2[:, :384])
                            t = st * 2 + c
                            if half == 0:
                                nc.sync.dma_start(
                                    out_d[t * 128:(t + 1) * 128, :], outt[:, :])
                            else:
                                nc.gpsimd.dma_start(
                                    out_d[t * 128:(t + 1) * 128, :], outt[:, :],
                                    accum_op=OP.add)

                    # 2-stage pipeline over super-tiles: prep(st+1) is emitted
                    # between fc1(st) and fc2(st) so its PE transposes fill
                    # the gelu wait.
                    cur = mlp_prep(0)
                    for st in range(16):
                        gT = mlp_fc1(cur[1])
                        nxt = mlp_prep(st + 1) if st < 15 else None
                        mlp_fc2(st, cur[0], gT)
                        cur = nxt

    nc.compile()
    return nc


_NC_CACHE = {}


def _get_nc():
    if "nc" not in _NC_CACHE:
        _NC_CACHE["nc"] = build_program()
    return _NC_CACHE["nc"]


def _prep_weights(inputs):
    f = lambda k: np.asarray(inputs[k], np.float32)
    x = f("x")
    ln1_g, ln1_b = f("ln1_g"), f("ln1_b")
    ln2_g, ln2_b = f("ln2_g"), f("ln2_b")
    qkv_w, qkv_b = f("qkv_w"), f("qkv_b")
    proj_w, proj_b = f("proj_w"), f("proj_b")
    fc1_w, fc1_b = f("fc1_w"), f("fc1_b")
    fc2_w, fc2_b = f("fc2_w"), f("fc2_b")
    rel = f("rel_pos_bias")
    SCALE = np.float32((DIM // HEADS) ** -0.5)

    # fold ln1 into qkv
    qkv_w_f = qkv_w * ln1_g[None, :]
    qkv_b_f = qkv_w @ ln1_b + qkv_b
    # fold attention scale into q block
    qkv_w_f[:DIM] *= SCALE
    qkv_b_f2 = qkv_b_f.copy()
    qkv_b_f2[:DIM] *= SCALE
    # fold v bias into proj bias (softmax rows sum to 1)
    projb = proj_b + proj_w @ qkv_b_f[2 * DIM:]
    # fold ln2 into fc1
    fc1_w_f = fc1_w * ln2_g[None, :]
    fc1_b_f = fc1_w @ ln2_b + fc1_b

    expb = np.exp(rel).astype(np.float32)          # [12, 196, 196]
    expb = expb.reshape(HEADS, 2, HC, WW).transpose(2, 0, 1, 3).copy()

    return {
        "qkvw": np.ascontiguousarray(qkv_w_f.T),    # [768, 2304]
        "qkvb": qkv_b_f2,
        "projw": np.ascontiguousarray(proj_w.T),    # [768, 768]
        "projb": projb,
        "expb": expb,
        "fc1w": np.ascontiguousarray(fc1_w_f.T),    # [768, 3072]
        "fc1b": fc1_b_f,
        "fc2w": np.ascontiguousarray(fc2_w.T),      # [3072, 768]
        "fc2b": fc2_b,
    }, x


PROFILE = False
LAST_RESULT = None


def kernel(**inputs):
    global LAST_RESULT
    weights, x = _prep_weights(inputs)
    nc = _get_nc()
    in_maps = [dict(weights, x=np.ascontiguousarray(x[i])) for i in range(B)]
    res = run_bass_kernel_spmd(nc, in_maps, core_ids=list(range(B)),
                               trace=PROFILE)
    LAST_RESULT = res
    out = np.stack([res.results[i]["out"] for i in range(B)], axis=0)
    return out.astype(np.float32)


if __name__ == "__main__":
    rng = np.random.default_rng(0)
    ins = {
        "x": rng.standard_normal((B, NTOK, DIM), dtype=np.float32),
        "rel_pos_bias": (rng.standard_normal((HEADS, WW, WW)) * 0.1).astype(np.float32),
        "ln1_g": np.ones(DIM, np.float32), "ln1_b": np.zeros(DIM, np.float32),
        "qkv_w": (rng.standard_normal((3 * DIM, DIM)) * 0.02).astype(np.float32),
        "qkv_b": np.zeros(3 * DIM, np.float32),
        "proj_w": (rng.standard_normal((DIM, DIM)) * 0.02).astype(np.float32),
        "proj_b": np.zeros(DIM, np.float32),
        "ln2_g": np.ones(DIM, np.float32), "ln2_b": np.zeros(DIM, np.float32),
        "fc1_w": (rng.standard_normal((MLP_H, DIM)) * 0.02).astype(np.float32),
        "fc1_b": np.zeros(MLP_H, np.float32),
        "fc2_w": (rng.standard_normal((DIM, MLP_H)) * 0.02).astype(np.float32),
        "fc2_b": np.zeros(DIM, np.float32),
        "H": np.int64(64), "W": np.int64(64),
    }
    out = kernel(**ins)
    print("out", out.shape, out.dtype, np.abs(out).max())


# revision 27
# speedup vs baseline: 2.0780x; 1.0031x over previous
"""Swin-style transformer block on 8 Trainium2 NeuronCores.

Sharding: data-parallel over batch — each of the 8 cores processes one image
([4096, 768] tokens). All weights replicated per core. No collectives.

Per-core pipeline:
  Attention, 13 window-pairs (pairing makes the matmul moving dim 392 >= 256
  so fp32r runs at full PE rate): gather + inline LN1 stats (rstd via
  ln/exp, same ACT table set as softmax) -> LN1-apply -> PE transpose ->
  fused QKV -> per-head scores -> softmax exp(S)*exp(B) with deferred 1/sum
  -> transpose P -> O^T = V^T P^T -> proj + residual -> scatter x2 to DRAM.
  Heads run in a 2-stage software pipeline (scores of head h+1 issue before
  transpose/O of head h).
  MLP in two hidden-halves of 1536 (weights fit SBUF): per 256-token
  super-tile: LN2 (stats inline on half 0; rsqrt = quake seed + 3 Newton
  steps, DVE-only, no ACT table switch) -> transpose -> fc1 (feature-major
  out) + erf-GELU -> fc2 + residual; half 1 accumulates into the output via
  gpsimd accum-DMA. Super-tiles are software-pipelined.

Host-side folds (all exact for the given inputs; SCALE=0.125 is binary-exact):
  - ln1 gamma/beta folded into qkv weights/bias; attention SCALE folded into
    the q block of qkv; v-bias folded into the proj bias (softmax rows sum
    to 1); ln2 gamma/beta folded into fc1; rel_pos_bias shipped as
    exp(rel_pos_bias) so softmax(S+B) = exp(S)*expB / sum(...).
"""

import numpy as np
from contextlib import ExitStack

import concourse.bass as bass
import concourse.mybir as mybir
import concourse.tile as tile
from concourse import bacc
from concourse.bass_utils import run_bass_kernel_spmd
from concourse.masks import make_identity

F32 = mybir.dt.float32
F32R = mybir.dt.float32r
AF = mybir.ActivationFunctionType
OP = mybir.AluOpType

DIM, HEADS, WIN, MLP_H = 768, 12, 14, 3072
B, H0, W0 = 8, 64, 64
NTOK = H0 * W0
NW = 5            # windows per image axis (70/14)
NWIN = NW * NW    # 25 windows
WW = WIN * WIN    # 196 tokens per window
HC = 98           # half-window chunk (7 rows x 14 cols)
DH = DIM // HEADS # 64
EPS = 1e-5

# window pairing: 12 pairs + 1 single
PAIRS = [(2 * i, 2 * i + 1) for i in range(12)] + [(24,)]


def _chunk_geom(w, c):
    """Valid-row/col geometry of half-chunk c (0/1) of window w."""
    wi, wj = divmod(w, NW)
    r0 = wi * WIN + c * 7          # first padded-image row of this chunk
    c0 = wj * WIN
    vr = 7 if (wi < 4 or c == 0) else 1   # wi==4 -> rows 56..63 valid (8)
    vc = 14 if wj < 4 else 8
    return r0, c0, vr, vc


def _gather_chunk(nc, dst, dram, w, c, eng=None):
    """DMA image tokens of half-chunk (w, c) from [4096,768] DRAM into
    dst [98, 768] SBUF tile (partition p = 14*row + col). Pads with zeros."""
    eng = eng or nc.sync
    r0, c0, vr, vc = _chunk_geom(w, c)
    if vr < 7 or vc < 14:
        nc.gpsimd.memset(dst[:, :], 0.0)
    if vc == 14:
        src = bass.AP(tensor=dram, offset=(r0 * W0 + c0) * DIM,
                      ap=[[W0 * DIM, vr], [DIM, 14], [1, DIM]])
        eng.dma_start(dst[0:vr * 14, :], src)
    else:
        for r in range(vr):
            src = bass.AP(tensor=dram, offset=((r0 + r) * W0 + c0) * DIM,
                          ap=[[DIM, vc], [1, DIM]])
            eng.dma_start(dst[r * 14:r * 14 + vc, :], src)


def _scatter_chunk(nc, dram, src, w, c, eng=None):
    """DMA the valid tokens of half-chunk (w, c) from src [98,768] SBUF back
    to token-major [4096,768] DRAM."""
    eng = eng or nc.sync
    r0, c0, vr, vc = _chunk_geom(w, c)
    if vc == 14:
        dst = bass.AP(tensor=dram, offset=(r0 * W0 + c0) * DIM,
                      ap=[[W0 * DIM, vr], [DIM, 14], [1, DIM]])
        eng.dma_start(dst, src[0:vr * 14, :])
    else:
        for r in range(vr):
            dst = bass.AP(tensor=dram, offset=((r0 + r) * W0 + c0) * DIM,
                          ap=[[DIM, vc], [1, DIM]])
            eng.dma_start(dst, src[r * 14:r * 14 + vc, :])


def build_program():
    nc = bacc.Bacc(None, target_bir_lowering=False, debug=False)

    x_d = nc.dram_tensor("x", [NTOK, DIM], F32, kind="ExternalInput")
    qkvw_d = nc.dram_tensor("qkvw", [DIM, 3 * DIM], F32R, kind="ExternalInput")
    qkvb_d = nc.dram_tensor("qkvb", [3 * DIM], F32, kind="ExternalInput")
    projw_d = nc.dram_tensor("projw", [DIM, DIM], F32R, kind="ExternalInput")
    projb_d = nc.dram_tensor("projb", [DIM], F32R, kind="ExternalInput")
    expb_d = nc.dram_tensor("expb", [HC, HEADS, 2, WW], F32, kind="ExternalInput")
    fc1w_d = nc.dram_tensor("fc1w", [DIM, MLP_H], F32R, kind="ExternalInput")
    fc1b_d = nc.dram_tensor("fc1b", [MLP_H], F32, kind="ExternalInput")
    fc2w_d = nc.dram_tensor("fc2w", [MLP_H, DIM], F32R, kind="ExternalInput")
    fc2b_d = nc.dram_tensor("fc2b", [DIM], F32R, kind="ExternalInput")

    out_d = nc.dram_tensor("out", [NTOK, DIM], F32, kind="ExternalOutput")
    x2_d = nc.dram_tensor("x2", [NTOK, DIM], F32)  # internal scratch

    with tile.TileContext(nc) as tc:
        with ExitStack() as g:
            # ---------------- global constants / stats ----------------
            consts = g.enter_context(tc.tile_pool(name="consts", bufs=1))
            ident32 = consts.tile([128, 128], F32)
            make_identity(nc, ident32)
            ident_r = consts.tile([128, 128], F32R)
            nc.vector.tensor_copy(ident_r[:, :], ident32[:, :])
            ones32 = consts.tile([33, 128], F32)
            nc.vector.memset(ones32[:, :], 1.0)
            ones_r = consts.tile([33, 128], F32R)
            nc.vector.tensor_copy(ones_r[:, :], ones32[:, :])
            eps_t = consts.tile([128, 1], F32)
            nc.vector.memset(eps_t[:, :], EPS)
            qkvb_sb = consts.tile([128, 12], F32)
            nc.sync.dma_start(
                qkvb_sb[:, :],
                bass.AP(tensor=qkvb_d, offset=0, ap=[[1, 128], [128, 12]]))
            bias2 = consts.tile([33, DIM], F32R)
            nc.sync.dma_start(bias2[0:1, :],
                              bass.AP(tensor=projb_d, offset=0, ap=[[1, DIM]]))
            nc.sync.dma_start(bias2[32:33, :],
                              bass.AP(tensor=fc2b_d, offset=0, ap=[[1, DIM]]))
            fc1b_sb = consts.tile([128, 24], F32)
            nc.sync.dma_start(
                fc1b_sb[:, :],
                bass.AP(tensor=fc1b_d, offset=0, ap=[[1, 128], [128, 24]]))
            stats2M = consts.tile([128, 32], F32)
            stats2R = consts.tile([128, 32], F32)

            # ---------------- P2: attention over window pairs ----------------
            with ExitStack() as s2:
                wA = s2.enter_context(tc.tile_pool(name="wA", bufs=1))
                qkvw_sb = wA.tile([128, 6, 3 * DIM], F32R)
                for kk in range(6):
                    for hh in range(2):
                        eng = (nc.sync, nc.scalar, nc.gpsimd)[(2 * kk + hh) % 3]
                        eng.dma_start(
                            qkvw_sb[:, kk, hh * 1152:(hh + 1) * 1152],
                            qkvw_d[kk * 128:(kk + 1) * 128,
                                   hh * 1152:(hh + 1) * 1152])
                projw_sb = wA.tile([128, 6, DIM], F32R)
                for kk in range(3):
                    eng = (nc.sync, nc.scalar, nc.gpsimd)[kk]
                    nc.sync.dma_start(
                        projw_sb[:, 2 * kk:2 * kk + 2, :],
                        projw_d[kk * 256:(kk + 1) * 256, :]
                        .rearrange("(a p) n -> p a n", p=128))
                expb_sb = wA.tile([HC, HEADS, 2, WW], F32)
                for hh in range(3):
                    eng = (nc.sync, nc.scalar, nc.gpsimd)[hh]
                    eng.dma_start(expb_sb[:, 4 * hh:4 * (hh + 1), :, :],
                                  expb_d[:, 4 * hh:4 * (hh + 1), :, :])

                # natural_log_exp_and_others: exp (softmax) + ln/exp (rstd)
                nc.scalar.add_instruction(mybir.InstLoadActFuncSet(
                    name=nc.get_next_instruction_name(), ins=[], outs=[],
                    act_func_set_id=6))

                pxp = s2.enter_context(tc.tile_pool(name="pxp", bufs=2))
                pln = s2.enter_context(tc.tile_pool(name="pln", bufs=1))
                phT = s2.enter_context(tc.tile_pool(name="phT", bufs=1))
                pqk = s2.enter_context(tc.tile_pool(name="pqk", bufs=1))
                pvT = s2.enter_context(tc.tile_pool(name="pvT", bufs=1))
                psm = s2.enter_context(tc.tile_pool(name="psm", bufs=3))
                ppn = s2.enter_context(tc.tile_pool(name="ppn", bufs=2))
                pUT = s2.enter_context(tc.tile_pool(name="pUT", bufs=1))
                pOT = s2.enter_context(tc.tile_pool(name="pOT", bufs=1))
                px2 = s2.enter_context(tc.tile_pool(name="px2", bufs=3))
                pps = s2.enter_context(tc.tile_pool(name="pps", bufs=8, space="PSUM"))

                for pair in PAIRS:
                    nw = len(pair)
                    PW = nw * WW          # 392 or 196
                    nch = 2 * nw          # chunks in pair

                    x_pair = pxp.tile([HC, 4, DIM], F32, name="x_pair")
                    ln_pair = pln.tile([HC, 4, DIM], F32R, name="ln_pair")
                    for wl, w in enumerate(pair):
                        for c in range(2):
                            i = 2 * wl + c
                            _gather_chunk(nc, x_pair[:, i, :], x_d, w, c,
                                          eng=nc.gpsimd)
                            # LN1 stats inline: rstd = exp(-0.5*ln(var+eps))
                            st = psm.tile([HC, 3, 6], F32, name="bst")
                            for gg in range(3):
                                nc.vector.bn_stats(
                                    st[:, gg, :],
                                    x_pair[:, i, gg * 256:(gg + 1) * 256])
                            mv = psm.tile([HC, 2], F32, name="bmv")
                            nc.vector.bn_aggr(mv[:, :], st[:, :, :])
                            rstd = psm.tile([HC, 2], F32, name="rstd")
                            nc.scalar.activation(rstd[:, 0:1], mv[:, 1:2],
                                                 AF.Ln, bias=eps_t[:HC, :])
                            nc.scalar.activation(rstd[:, 1:2], rstd[:, 0:1],
                                                 AF.Exp, scale=-0.5)
                            nc.vector.tensor_scalar(
                                out=ln_pair[:, i, :], in0=x_pair[:, i, :],
                                scalar1=mv[:, 0:1],
                                scalar2=rstd[:, 1:2],
                                op0=OP.subtract, op1=OP.mult)

                    # transpose ln -> hT [128, 6, PW] (feature-major)
                    hT = phT.tile([128, 6, 2 * WW], F32R, name="hT")
                    for k in range(6):
                        ps_t = pps.tile([128, 392], F32R, tag="ps", name="ps_t")
                        for i in range(nch):
                            nc.tensor.transpose(
                                ps_t[:, i * HC:(i + 1) * HC],
                                ln_pair[:HC, i, k * 128:(k + 1) * 128],
                                ident_r[:HC, :HC])
                        nc.vector.tensor_copy(hT[:, k, :PW], ps_t[:, :PW])

                    # fused qk^T: [1536 feats, PW] (q pre-scaled on host).
                    # Emit q/k chunks interleaved (0,6,1,7,..) so head h's
                    # operands (chunks h//2 and 6+h//2) finish earliest.
                    qkT = pqk.tile([128, 12, 2 * WW], F32R, name="qkT")
                    for m in [0, 6, 1, 7, 2, 8, 3, 9, 4, 10, 5, 11]:
                        ps_qk = pps.tile([128, 392], F32, tag="ps", name="ps_qk")
                        for k in range(6):
                            nc.tensor.matmul(
                                ps_qk[:, :PW],
                                qkvw_sb[:, k, m * 128:(m + 1) * 128],
                                hT[:, k, :PW],
                                start=(k == 0), stop=(k == 5))
                        nc.scalar.activation(qkT[:, m, :PW], ps_qk[:, :PW],
                                             AF.Identity, bias=qkvb_sb[:, m:m + 1])

                    # v token-major: [PW, 768]
                    vT = pvT.tile([HC, 4, DIM], F32R, name="vT")
                    for i in range(nch):
                        for n in range(2):
                            ps_v = pps.tile([128, 392], F32, tag="ps", name="ps_v")
                            for k in range(6):
                                nc.tensor.matmul(
                                    ps_v[:HC, :384],
                                    hT[:, k, i * HC:(i + 1) * HC],
                                    qkvw_sb[:, k, 2 * DIM + n * 384:2 * DIM + (n + 1) * 384],
                                    start=(k == 0), stop=(k == 5))
                            nc.vector.tensor_copy(vT[:, i, n * 384:(n + 1) * 384],
                                                  ps_v[:HC, :384])

                    OT = pOT.tile([128, 6, 2 * WW], F32R, name="OT")

                    def head_stage_a(h):
                        """scores + softmax -> normalized Pn for head h."""
                        qrow = (h % 2) * 64
                        qm = h // 2
                        km = 6 + h // 2
                        Pn = ppn.tile([HC, 4, WW], F32, name="Pn")
                        den = ppn.tile([HC, 4], F32, name="den")
                        rden = ppn.tile([HC, 4], F32, name="rden")
                        for qc in range(nch):
                            wl = qc // 2
                            ps_S = pps.tile([128, 392], F32, tag="ps", name="ps_S")
                            nc.tensor.matmul(
                                ps_S[:HC, :PW],
                                qkT[qrow:qrow + 64, qm, qc * HC:(qc + 1) * HC],
                                qkT[qrow:qrow + 64, km, :PW],
                                start=True, stop=True)
                            U = psm.tile([HC, 2 * WW], F32, name="U")
                            nc.scalar.activation(U[:, :PW], ps_S[:HC, :PW], AF.Exp)
                            P = psm.tile([HC, WW], F32, name="P")
                            nc.vector.scalar_tensor_tensor(
                                out=P[:, :], in0=U[:, wl * WW:(wl + 1) * WW],
                                scalar=1.0, in1=expb_sb[:, h, qc % 2, :],
                                op0=OP.mult, op1=OP.mult,
                                accum_out=den[:, qc:qc + 1])
                            nc.vector.reciprocal(rden[:, qc:qc + 1],
                                                 den[:, qc:qc + 1])
                            nc.vector.tensor_scalar_mul(
                                Pn[:, qc, :], P[:, :], rden[:, qc:qc + 1])
                        return Pn

                    def head_stage_b(h, Pn):
                        """transpose Pn -> UT, then O^T = V^T @ U^T."""
                        qrow = (h % 2) * 64
                        qm = h // 2
                        UT = pUT.tile([HC, 2, 2 * WW], F32R, name="UT")
                        ps_PT = [pps.tile([128, 392], F32, tag="ps",
                                          name="ps_PT") for _ in range(2)]
                        for qc in range(nch):     # qc-major: unblock early
                            for kc in range(2):
                                nc.tensor.transpose(
                                    ps_PT[kc][:HC, qc * HC:(qc + 1) * HC],
                                    Pn[:HC, qc, kc * HC:(kc + 1) * HC],
                                    ident32[:HC, :HC])
                        for kc in range(2):
                            nc.scalar.copy(UT[:, kc, :PW], ps_PT[kc][:HC, :PW])
                        for wl in range(nw):
                            ps_O = pps.tile([128, 392], F32, tag="ps", name="ps_O")
                            for kc in range(2):
                                nc.tensor.matmul(
                                    ps_O[:64, :PW],
                                    vT[:, 2 * wl + kc, h * 64:(h + 1) * 64],
                                    UT[:, kc, :PW],
                                    start=(kc == 0), stop=(kc == 1))
                            nc.vector.tensor_copy(
                                OT[qrow:qrow + 64, qm, wl * WW:(wl + 1) * WW],
                                ps_O[:64, wl * WW:(wl + 1) * WW])

                    # 2-stage pipeline over heads: S(h+1) issues before PT/O(h)
                    prev = None
                    for h in range(HEADS):
                        Pn = head_stage_a(h)
                        if prev is not None:
                            head_stage_b(h - 1, prev)
                        prev = Pn
                    head_stage_b(HEADS - 1, prev)

                    # proj + residual -> x2 (per chunk, scatter immediately)
                    for i in range(nch):
                        x2c = px2.tile([HC, DIM], F32, name="x2c")
                        for n in range(2):
                            ps_pr = pps.tile([128, 392], F32, tag="ps", name="ps_pr")
                            for k in range(6):
                                nc.tensor.matmul(
                                    ps_pr[:HC, :384],
                                    OT[:, k, i * HC:(i + 1) * HC],
                                    projw_sb[:, k, n * 384:(n + 1) * 384],
                                    start=(k == 0), stop=False)
                            nc.tensor.matmul(
                                ps_pr[:HC, :384],
                                ones_r[0:1, :HC],
                                bias2[0:1, n * 384:(n + 1) * 384],
                                start=False, stop=True)
                            nc.vector.tensor_tensor(
                                out=x2c[:, n * 384:(n + 1) * 384],
                                in0=ps_pr[:HC, :384],
                                in1=x_pair[:, i, n * 384:(n + 1) * 384],
                                op=OP.add)
                        _scatter_chunk(nc, x2_d, x2c[:, :], pair[i // 2], i % 2,
                                       eng=nc.sync)

            # ---------------- P4 + P5 (MLP) ----------------
            with ExitStack() as s5:
                # weight pool first so the half-0 prefetch DMAs overlap the
                # LN2-stats pass (no address reuse WAR on P4's pools)
                wB = s5.enter_context(tc.tile_pool(name="wB", bufs=1))

                def load_fc_weights(half):
                    engs = (nc.gpsimd, nc.sync, nc.scalar)
                    fc1w_sb = wB.tile([128, 6, MLP_H // 2], F32R, name="fc1w_sb")
                    for kk in range(6):
                        engs[kk % 3].dma_start(
                            fc1w_sb[:, kk, :],
                            fc1w_d[kk * 128:(kk + 1) * 128,
                                   half * 1536:(half + 1) * 1536])
                    fc2w_sb = wB.tile([128, 12, DIM], F32R, name="fc2w_sb")
                    for kk in range(4):
                        engs[kk % 3].dma_start(
                            fc2w_sb[:, 3 * kk:3 * (kk + 1), :],
                            fc2w_d[half * 1536 + kk * 384:
                                   half * 1536 + (kk + 1) * 384, :]
                            .rearrange("(a p) n -> p a n", p=128))
                    return fc1w_sb, fc2w_sb

                w_half0 = load_fc_weights(0)

                pxt = s5.enter_context(tc.tile_pool(name="pxt", bufs=4))
                pxn = s5.enter_context(tc.tile_pool(name="pxn", bufs=2))
                pxnT = s5.enter_context(tc.tile_pool(name="pxnT", bufs=2))
                pgT = s5.enter_context(tc.tile_pool(name="pgT", bufs=2))
                pout = s5.enter_context(tc.tile_pool(name="pout", bufs=4))
                pps5 = s5.enter_context(tc.tile_pool(name="pps5", bufs=8, space="PSUM"))


                for half in range(2):
                    fc1w_sb, fc2w_sb = w_half0 if half == 0 else load_fc_weights(1)

                    def mlp_prep(st):
                        """load x2 tiles, (half0: LN2 stats), apply, transpose."""
                        x2t = []
                        x2nT = pxnT.tile([128, 6, 256], F32R, name="x2nT")
                        for c in range(2):
                            t = st * 2 + c
                            xt = pxt.tile([128, DIM], F32, name="x2t")
                            eng = nc.scalar if t % 2 == 0 else nc.sync
                            eng.dma_start(xt[:, :], x2_d[t * 128:(t + 1) * 128, :])
                            x2t.append(xt)
                            if half == 0:
                                # LN2 stats inline; rsqrt on DVE only
                                # (quake seed + 3 Newton steps, ~1.5e-7 rel)
                                st5 = pxn.tile([128, 3, 6], F32, name="st5")
                                for gg in range(3):
                                    nc.vector.bn_stats(
                                        st5[:, gg, :],
                                        xt[:, gg * 256:(gg + 1) * 256])
                                mv5 = pxn.tile([128, 8], F32, name="mv5")
                                nc.vector.bn_aggr(mv5[:, 0:2], st5[:, :, :])
                                nc.gpsimd.tensor_copy(stats2M[:, t:t + 1],
                                                      mv5[:, 0:1])
                                ve = mv5[:, 2:3]
                                nc.vector.tensor_scalar(
                                    out=ve, in0=mv5[:, 1:2], scalar1=EPS,
                                    scalar2=None, op0=OP.add)
                                yi = mv5[:, 3:4].bitcast(mybir.dt.int32)
                                nc.vector.tensor_scalar(
                                    out=yi, in0=ve.bitcast(mybir.dt.int32),
                                    scalar1=1, scalar2=None,
                                    op0=OP.arith_shift_right)
                                y0 = mv5[:, 4:5].bitcast(mybir.dt.int32)
                                nc.vector.tensor_scalar(
                                    out=y0, in0=yi, scalar1=-1,
                                    scalar2=None, op0=OP.bitwise_xor)
                                nc.vector.tensor_scalar(
                                    out=y0, in0=y0, scalar1=0x5f3759e0,
                                    scalar2=None, op0=OP.add)
                                ya = mv5[:, 4:5]
                                yb = mv5[:, 5:6]
                                t2 = mv5[:, 6:7]
                                w5 = mv5[:, 7:8]
                                for _ in range(3):
                                    nc.vector.tensor_tensor(
                                        out=t2, in0=ya, in1=ya, op=OP.mult)
                                    nc.vector.scalar_tensor_tensor(
                                        out=w5, in0=ve, scalar=-0.5, in1=t2,
                                        op0=OP.mult, op1=OP.mult)
                                    nc.vector.tensor_scalar(
                                        out=w5, in0=w5, scalar1=1.5,
                                        scalar2=None, op0=OP.add)
                                    nc.vector.tensor_tensor(
                                        out=yb, in0=ya, in1=w5, op=OP.mult)
                                    ya, yb = yb, ya
                                nc.gpsimd.tensor_copy(stats2R[:, t:t + 1], ya)
                            xn = pxn.tile([128, DIM], F32R, name="x2n")
                            nc.vector.tensor_scalar(
                                out=xn[:, :], in0=xt[:, :],
                                scalar1=stats2M[:, t:t + 1],
                                scalar2=stats2R[:, t:t + 1],
                                op0=OP.subtract, op1=OP.mult)
                            for k in range(6):
                                ps_t2 = pps5.tile([128, 384], F32R, tag="ps5",
                                                  name="ps_t2")
                                nc.tensor.transpose(
                                    ps_t2[:, :128], xn[:, k * 128:(k + 1) * 128],
                                    ident_r[:, :])
                                nc.vector.tensor_copy(
                                    x2nT[:, k, c * 128:(c + 1) * 128],
                                    ps_t2[:, :128])
                        return x2t, x2nT

                    def mlp_fc1(x2nT):
                        gT = pgT.tile([128, 12, 256], F32R, name="gT")
                        for m in range(12):
                            ps_f1 = pps5.tile([128, 384], F32, tag="ps5", name="ps_f1")
                            for k in range(6):
                                nc.tensor.matmul(
                                    ps_f1[:, :256],
                                    fc1w_sb[:, k, m * 128:(m + 1) * 128],
                                    x2nT[:, k, :],
                                    start=(k == 0), stop=(k == 5))
                            nc.scalar.activation(
                                gT[:, m, :], ps_f1[:, :256], AF.Gelu,
                                bias=fc1b_sb[:, half * 12 + m:half * 12 + m + 1])
                        return gT

                    def mlp_fc2(st, x2t, gT):
                        for c in range(2):
                            outt = pout.tile([128, DIM], F32, name="outt")
                            for n in range(2):
                                ps_f2 = pps5.tile([128, 384], F32, tag="ps5",
                                                  name="ps_f2")
                                for m in range(12):
                                    nc.tensor.matmul(
                                        ps_f2[:, :384],
                                        gT[:, m, c * 128:(c + 1) * 128],
                                        fc2w_sb[:, m, n * 384:(n + 1) * 384],
                                        start=(m == 0), stop=(m == 11 and half == 1))
                                if half == 0:
                                    nc.tensor.matmul(
                                        ps_f2[:, :384],
                                        ones_r[32:33, :128],
                                        bias2[32:33, n * 384:(n + 1) * 384],
                                        start=False, stop=True)
                                    nc.vector.tensor_tensor(
                                        out=outt[:, n * 384:(n + 1) * 384],
                                        in0=ps_f2[:, :384],
                                        in1=x2t[c][:, n * 384:(n + 1) * 384],
                                        op=OP.add)
                                else:
                                    nc.vector.tensor_copy(
                                        outt[:, n * 384:(n + 1) * 384],
                                        ps_f2[:, :384])
                            t = st * 2 + c
                            if half == 0:
                                nc.sync.dma_start(
                                    out_d[t * 128:(t + 1) * 128, :], outt[:, :])
                            else:
                                nc.gpsimd.dma_start(
                                    out_d[t * 128:(t + 1) * 128, :], outt[:, :],
                                    accum_op=OP.add)

                    # 2-stage pipeline over super-tiles: prep(st+1) is emitted
                    # between fc1(st) and fc2(st) so its PE transposes fill
                    # the gelu wait.
                    cur = mlp_prep(0)
                    for st in range(16):
                        gT = mlp_fc1(cur[1])
                        nxt = mlp_prep(st + 1) if st < 15 else None
                        mlp_fc2(st, cur[0], gT)
                        cur = nxt

    nc.compile()
    return nc


_NC_CACHE = {}


def _get_nc():
    if "nc" not in _NC_CACHE:
        _NC_CACHE["nc"] = build_program()
    return _NC_CACHE["nc"]


def _prep_weights(inputs):
    f = lambda k: np.asarray(inputs[k], np.float32)
    x = f("x")
    ln1_g, ln1_b = f("ln1_g"), f("ln1_b")
    ln2_g, ln2_b = f("ln2_g"), f("ln2_b")
    qkv_w, qkv_b = f("qkv_w"), f("qkv_b")
    proj_w, proj_b = f("proj_w"), f("proj_b")
    fc1_w, fc1_b = f("fc1_w"), f("fc1_b")
    fc2_w, fc2_b = f("fc2_w"), f("fc2_b")
    rel = f("rel_pos_bias")
    SCALE = np.float32((DIM // HEADS) ** -0.5)

    # fold ln1 into qkv
    qkv_w_f = qkv_w * ln1_g[None, :]
    qkv_b_f = qkv_w @ ln1_b + qkv_b
    # fold attention scale into q block
    qkv_w_f[:DIM] *= SCALE
    qkv_b_f2 = qkv_b_f.copy()
    qkv_b_f2[:DIM] *= SCALE
    # fold v bias into proj bias (softmax rows sum to 1)
    projb = proj_b + proj_w @ qkv_b_f[2 * DIM:]
    # fold ln2 into fc1
    fc1_w_f = fc1_w * ln2_g[None, :]
    fc1_b_f = fc1_w @ ln2_b + fc1_b

    expb = np.exp(rel).astype(np.float32)          # [12, 196, 196]
    expb = expb.reshape(HEADS, 2, HC, WW).transpose(2, 0, 1, 3).copy()

    return {
        "qkvw": np.ascontiguousarray(qkv_w_f.T),    # [768, 2304]
        "qkvb": qkv_b_f2,
        "projw": np.ascontiguousarray(proj_w.T),    # [768, 768]
        "projb": projb,
        "expb": expb,
        "fc1w": np.ascontiguousarray(fc1_w_f.T),    # [768, 3072]
        "fc1b": fc1_b_f,
        "fc2w": np.ascontiguousarray(fc2_w.T),      # [3072, 768]
        "fc2b": fc2_b,
    }, x


PROFILE = False
LAST_RESULT = None


def kernel(**inputs):
    global LAST_RESULT
    weights, x = _prep_weights(inputs)
    nc = _get_nc()
    in_maps = [dict(weights, x=np.ascontiguousarray(x[i])) for i in range(B)]
    res = run_bass_kernel_spmd(nc, in_maps, core_ids=list(range(B)),
                               trace=PROFILE)
    LAST_RESULT = res
    out = np.stack([res.results[i]["out"] for i in range(B)], axis=0)
    return out.astype(np.float32)


if __name__ == "__main__":
    rng = np.random.default_rng(0)
    ins = {
        "x": rng.standard_normal((B, NTOK, DIM), dtype=np.float32),
        "rel_pos_bias": (rng.standard_normal((HEADS, WW, WW)) * 0.1).astype(np.float32),
        "ln1_g": np.ones(DIM, np.float32), "ln1_b": np.zeros(DIM, np.float32),
        "qkv_w": (rng.standard_normal((3 * DIM, DIM)) * 0.02).astype(np.float32),
        "qkv_b": np.zeros(3 * DIM, np.float32),
        "proj_w": (rng.standard_normal((DIM, DIM)) * 0.02).astype(np.float32),
        "proj_b": np.zeros(DIM, np.float32),
        "ln2_g": np.ones(DIM, np.float32), "ln2_b": np.zeros(DIM, np.float32),
        "fc1_w": (rng.standard_normal((MLP_H, DIM)) * 0.02).astype(np.float32),
        "fc1_b": np.zeros(MLP_H, np.float32),
        "fc2_w": (rng.standard_normal((DIM, MLP_H)) * 0.02).astype(np.float32),
        "fc2_b": np.zeros(DIM, np.float32),
        "H": np.int64(64), "W": np.int64(64),
    }
    out = kernel(**ins)
    print("out", out.shape, out.dtype, np.abs(out).max())
